# revision 38
# baseline (speedup 1.0000x reference)
"""Trainium2 Bass kernel for nn_ASISNativeAttention (B=2,S=2048,D=1024,H=16).

Sharding: 8 cores = 2 batches x 4 head-groups (4 heads each); host splits
inputs per core and sums the 4 partial output projections per batch (+bo).

v2 design (vs the all-bf16 v1): the two elementwise engines (ACT, DVE) are
the roofline -- 131K partition-lines of exp per core -- so every matmul that
can cheaply move to fp8 DoubleRow mode does, freeing PE far below the
elementwise roof, and the exp work is split across BOTH elementwise engines:

  PE    x (fp8, host-transposed) -> q,k,v projections in fp8 DoubleRow
        (2 d-tiles per matmul); scores per head via DoubleRow with a
        stride-0 replicated k-tile pair (computes 2*q.k, folded into the
        exp scale 1/32... -> 1/16 of the x2 psum); PV and out-proj in bf16;
        mean-pool via fp8 DoubleRow ones-column matmuls.
  ACT   exact exp (scale=1/16) psum->bf16 for ~half the score tiles; the
        qk/v/out drains it is assigned; 2 sigmoids.
  DVE   Schraudolph exp for the other tiles: one tensor_scalar
        (psum*A+B -> int16) whose int16 bits ARE the bf16 probs; ctx
        normalize (strided reciprocal + stride-0-broadcast tensor_tensor);
        gated ctxT drain (2x mode); its share of drains.
  Pool  output DMA via SWDGE (keeps HWDGE/SP free); no tensor work (GPSIMD
        cannot access PSUM on TRN2).

Accuracy (numpy study vs reference, same seed): rel err ~1.2e-2 < 2e-2 gate;
fp8 exposure limited to x / Wq,Wk,Wv / stored qT,kT (probs, v, ctx, Wo stay
bf16). Host sums partial outputs in fp32 and adds bo.

kernel.py is self-contained: numpy/ml_dtypes + the concourse tree at
/opt/trn_rl_repo.
"""

import os
import sys
import numpy as np
import ml_dtypes

BF16 = ml_dtypes.bfloat16
F8 = ml_dtypes.float8_e4m3

sys.path.insert(0, "/opt/trn_rl_repo")

B, S, D, H = 2, 2048, 1024, 16
HD = 64          # head dim
NCORES = 8
HG = 4           # head groups = cores per batch
HL = H // HG     # heads per core (4)
DL = D // HG     # local width (256)
ST = S // 128    # 16 s-tiles
DT = D // 128    # 8 d-tiles
SC = 512         # sq chunk width for scores
NSC = S // SC    # 4 chunks

# Schraudolph exp constants: int16 = rint(psum * A16 + B16); psum holds
# 2*q.k so the effective exp argument is psum/16 = q.k/8.
_C_SCH = 360000.0
A16 = float(2.0**7 / np.log(2.0) / 16.0)
B16 = float(127 * 2**7 - _C_SCH / 65536.0)

# --- engine-assignment knobs (tuned against TimelineSim) ---
# exp engine per chunk (16 chars: a-stretch skp 0-7 then b-stretch skp 0-7):
# 'A' = ACT exact exp, 'D' = DVE Schraudolph
EXP_PAT = [
    "AADAADAD" "AADAADAD",   # 10A chunks
    "AADAADAD" "ADAADADD",   # 9A chunks
] * 3 + ["AADAADAD" "AADAADAD", "AADAADAD" "ADAADDDA"]
if os.environ.get("KB_ALLACT"):
    EXP_PAT = ["A" * 16] * 8
KB_NOSWDGE = bool(os.environ.get("KB_NOSWDGE"))
KB_CHUNKS = int(os.environ.get("KB_CHUNKS", "8"))
KB_LEADIN = int(os.environ.get("KB_LEADIN", "99"))
# qk projection drain engines, one char per unit (k-i0 x4, q-i0 x4, then 8 i1)
QK_DRAIN = "ADADADADADADADAD"
# v drain engines, one per psum group (8)
V_DRAIN = "ADADADAD"
# out-proj drain engines, one per po half (32)
OUT_DRAIN = "AD" * 16
FILLER_NS = 1500   # filler budget popped per b-stretch skp

_CACHE = {}


def _build_nc():
    import concourse.bass as bass
    import concourse.mybir as mybir
    from concourse.tile import TileContext

    fp32 = mybir.dt.float32
    bf16 = mybir.dt.bfloat16
    f8 = mybir.dt.float8e4
    i16 = mybir.dt.int16
    AF = mybir.ActivationFunctionType
    ALU = mybir.AluOpType
    DRm = mybir.MatmulPerfMode.DoubleRow

    nc = bass.Bass()

    xt8_d = nc.declare_dram_parameter("xt8", [64, 2 * DT * S], f8, isOutput=False)
    x8_d = nc.declare_dram_parameter("x8", [128, ST * D], f8, isOutput=False)
    wq_d = nc.declare_dram_parameter("wq8", [64, 2 * DT * DL], f8, isOutput=False)
    wk_d = nc.declare_dram_parameter("wk8", [64, 2 * DT * DL], f8, isOutput=False)
    wv_d = nc.declare_dram_parameter("wv8", [64, 2 * DT * DL], f8, isOutput=False)
    wo_d = nc.declare_dram_parameter("wo", [128, 2 * D], bf16, isOutput=False)
    bqk_d = nc.declare_dram_parameter("bqk", [128, 4], fp32, isOutput=False)
    bv8_d = nc.declare_dram_parameter("bv8p", [1, 2 * DL], f8, isOutput=False)
    wes_d = nc.declare_dram_parameter("wes", [128, DT * 2 * HL], fp32, isOutput=False)
    bes_d = nc.declare_dram_parameter("bes", [HL, 2], fp32, isOutput=False)
    gexp_d = nc.declare_dram_parameter("gexp", [HL, DL], fp32, isOutput=False)
    id_d = nc.declare_dram_parameter("ident", [128, 128], bf16, isOutput=False)
    out_d = nc.declare_dram_parameter("out", [S, D], bf16, isOutput=True)

    with TileContext(nc) as tc:
        with tc.tile_pool(name="persist", bufs=1) as P:
            xt8 = P.tile([64, 2 * DT * S], f8, tag="xt8")
            wq8 = P.tile([64, 2 * DT * DL], f8, tag="wq8")
            wk8 = P.tile([64, 2 * DT * DL], f8, tag="wk8")
            wv8 = P.tile([64, 2 * DT * DL], f8, tag="wv8")
            wo = P.tile([128, 2 * D], bf16, tag="wo")
            qT8 = P.tile([128, 2 * S], f8, tag="qT8")
            kT8 = P.tile([128, 2 * S], f8, tag="kT8")
            v = P.tile([128, ST * HL * 65], bf16, tag="v")
            ctx = P.tile([128, ST * DL], bf16, tag="ctx")
            ctxT = P.tile([128, 2 * S], bf16, tag="ctxT")
            bqk = P.tile([128, 4], fp32, tag="bqk")
            bv8p = P.tile([1, 2 * DL], f8, tag="bv8p")
            ones82 = P.tile([128, 2, 64], f8, tag="ones82")
            ones8r = P.tile([1, 2 * 128], f8, tag="ones8r")
            z8row = P.tile([1, 2 * 260], f8, tag="z8row")
            wes = P.tile([128, DT * 2 * HL], fp32, tag="wes")
            bes = P.tile([HL, 2], fp32, tag="bes")
            gexp = P.tile([HL, DL], fp32, tag="gexp")
            ident = P.tile([128, 128], bf16, tag="ident")
            ones1 = P.tile([1, 1], fp32, tag="ones1")
            xmrow = P.tile([1, D], fp32, tag="xmrow")
            xm_col = P.tile([128, DT], fp32, tag="xm_col")
            gcol = P.tile([128, 2], fp32, tag="gcol")

            dma = nc.sync.dma_start

            def vview(t):
                return v[:].rearrange("p (t h c) -> p t h c", h=HL, c=65)[:, t]

            nc.vector.memset(ones82[:], 1.0)
            nc.vector.memset(ones8r[:], 1.0)
            nc.vector.memset(z8row[:], 0.0)
            nc.vector.memset(ones1[:], 1.0)
            # constant softmax-denominator columns of v
            nc.vector.memset(
                v[:].rearrange("p (t h c) -> p t h c", h=HL, c=65)[:, :, :, 64:65], 1.0
            )

            xt8v = xt8[:].rearrange("p (j s) -> p j s", s=S)      # [64, 16, S]
            wq8v = wq8[:].rearrange("p (j m) -> p j m", m=DL)     # [64, 16, DL]
            wk8v = wk8[:].rearrange("p (j m) -> p j m", m=DL)
            wv8v = wv8[:].rearrange("p (j m) -> p j m", m=DL)

            with (
                tc.tile_pool(name="x8l", bufs=8) as XL,
                tc.tile_pool(name="pm", bufs=3, space="PSUM") as PM,
                tc.tile_pool(name="pcps", bufs=2, space="PSUM") as PCS,
                tc.tile_pool(name="ets", bufs=26) as EX,
                tc.tile_pool(name="rc", bufs=8) as RC,
                tc.tile_pool(name="ob", bufs=2) as OB,
                tc.tile_pool(name="gs", bufs=1) as GS,
            ):
                xmp = [None, None]

                xbs = []

                def load_x_pair(pr):
                    """DMA natural-x pair pr via SWDGE (Pool) off the HWDGE path."""
                    xb = XL.tile([128, 2, D], f8, tag="xb", name=f"xb{pr}")
                    eng = nc.scalar if KB_NOSWDGE else nc.gpsimd
                    eng.dma_start(
                        out=xb[:],
                        in_=x8_d[:].rearrange("p (r c d) -> p r c d", c=2, d=D)[:, pr])
                    xbs.append(xb)

                def meanpool_mm(pr, xmps):
                    lhs = ones82[:]  # [128, 2, 64]: dual-fp8 ldweights needs wide M
                    for qh in range(4):
                        half, qq = qh // 2, qh % 2
                        first = (pr == 0 and qq == 0)
                        nc.tensor.matmul(
                            xmps[half][:, qq * 256:(qq + 1) * 256],
                            lhsT=lhs,
                            rhs=xbs[pr][:, :, qh * 256:(qh + 1) * 256],
                            start=first, stop=first,
                            skip_group_check=not first,
                            perf_mode=DRm,
                        )

                def qk_unit(w8v, dst8, bcol, i, sc, eng):
                    """Project one [128, SC] chunk of qT or kT (fp8 out)."""
                    pp = PM.tile([128, SC], fp32, tag="pm",
                                 name=f"pp{dst8.tensor.name}_{i}_{sc}")
                    for qq in range(2):
                        # x / weights live on 64 partitions x 16 d-tiles:
                        # dual-fp8 DR caps 2*K*M at the PE array size and the
                        # PE hangs if ldweights base-partition changes inside
                        # an accumulation chain, so every chain stays K=64.
                        for dp in range(DT):
                            first = (qq == 0 and dp == 0)
                            nc.tensor.matmul(
                                pp[:, qq * 256:(qq + 1) * 256],
                                lhsT=w8v[:, 2 * dp:2 * dp + 2, i * 128:(i + 1) * 128],
                                rhs=xt8v[:, 2 * dp:2 * dp + 2,
                                         sc * SC + qq * 256: sc * SC + (qq + 1) * 256],
                                start=first, stop=first,
                                skip_group_check=not first,
                                perf_mode=DRm,
                            )
                    dst = dst8[:, i * S + sc * SC: i * S + (sc + 1) * SC]
                    if eng == "A":
                        nc.scalar.activation(dst, pp[:], AF.Identity, bias=bcol)
                    else:
                        nc.vector.tensor_scalar(
                            out=dst, in0=pp[:], scalar1=bcol, scalar2=None,
                            op0=ALU.add,
                        )

                def v_unit(g, eng):
                    """Project v for s-tiles 2g, 2g+1 (one psum bank)."""
                    pv = PM.tile([128, 2 * DL], fp32, tag="pm", name=f"pv{g}")
                    for t2 in range(2):
                        t = 2 * g + t2
                        sl = pv[:, t2 * DL:(t2 + 1) * DL]
                        nc.tensor.matmul(
                            sl, lhsT=ones8r[:].rearrange("o (c m) -> o c m", c=2),
                            rhs=bv8p[:].rearrange("o (c m) -> o c m", c=2),
                            start=(t2 == 0), stop=(t2 == 0),
                            skip_group_check=(t2 == 1), perf_mode=DRm,
                        )
                        for dp in range(DT):
                            nc.tensor.matmul(
                                sl,
                                lhsT=xt8v[:, 2 * dp:2 * dp + 2, t * 128:(t + 1) * 128],
                                rhs=wv8v[:, 2 * dp:2 * dp + 2, :],
                                start=False, stop=False,
                                skip_group_check=True,
                                perf_mode=DRm,
                            )
                    for t2 in range(2):
                        t = 2 * g + t2
                        src = pv[:, t2 * DL:(t2 + 1) * DL].rearrange(
                            "p (h c) -> p h c", c=HD)
                        dst = vview(t)[:, :, 0:HD]
                        if eng == "A":
                            nc.scalar.copy(dst, src)
                        else:
                            nc.vector.tensor_copy(dst, src)

                def score_exp(i, sc, hh, skp, eng):
                    """Scores for sk-tiles (2skp, 2skp+1) x [sc*SC, (sc+1)*SC) of
                    head 2i+hh; one wide exp. psum holds 2*q.k (stride-0 DR)."""
                    r = hh * 64
                    sp = PM.tile([128, 2 * SC], fp32, tag="pm",
                                 name=f"sp{i}_{sc}_{hh}_{skp}")
                    for half in range(2):
                        sk = 2 * skp + half
                        lhsT = kT8[r:r + 64, i * S + sk * 128: i * S + (sk + 1) * 128] \
                            .unsqueeze(1).broadcast_to([64, 2, 128])
                        for qq in range(2):
                            rhs = qT8[r:r + 64,
                                      i * S + sc * SC + qq * 256: i * S + sc * SC + (qq + 1) * 256] \
                                .unsqueeze(1).broadcast_to([64, 2, 256])
                            # qq0 opens the bank's zero region; qq1 assigns
                            # into still-pending bytes (no second group)
                            nc.tensor.matmul(
                                sp[:, half * SC + qq * 256: half * SC + (qq + 1) * 256],
                                lhsT=lhsT, rhs=rhs, start=(qq == 0), stop=(qq == 0),
                                skip_group_check=(qq == 1),
                                perf_mode=DRm,
                            )
                    if eng == "A":
                        et = EX.tile([128, 2 * SC], bf16, tag="et",
                                     name=f"et{i}_{sc}_{hh}_{skp}")
                        nc.scalar.activation(et[:], sp[:], AF.Exp, scale=1.0 / 16.0)
                        return et[:]
                    et = EX.tile([128, 2 * SC], i16, tag="et",
                                 name=f"et{i}_{sc}_{hh}_{skp}")
                    nc.vector.tensor_scalar(
                        out=et[:], in0=sp[:], scalar1=A16, scalar2=B16,
                        op0=ALU.mult, op1=ALU.add,
                    )
                    return et[:].bitcast(bf16)

                def pv_mm(cps, ets, i, hh, sk):
                    h = 2 * i + hh
                    skp, half = sk // 2, sk % 2
                    for u in range(SC // 128):
                        nc.tensor.matmul(
                            cps[u][:, hh * 65:(hh + 1) * 65],
                            lhsT=ets[skp][:, half * SC + u * 128: half * SC + (u + 1) * 128],
                            rhs=v[:, sk * HL * 65 + h * 65: sk * HL * 65 + (h + 1) * 65],
                            start=False, stop=False, skip_group_check=True,
                        )

                def outproj_units(t):
                    def unit():
                        ot = OB.tile([128, D], bf16, tag="ot", name=f"ot{t}")
                        po = PM.tile([128, D], fp32, tag="pm", name=f"po{t}")
                        for n2 in range(2):
                            for i2 in range(2):
                                nc.tensor.matmul(
                                    po[:, n2 * 512:(n2 + 1) * 512],
                                    lhsT=ctxT[:, i2 * S + t * 128: i2 * S + (t + 1) * 128],
                                    rhs=wo[:, i2 * D + n2 * 512: i2 * D + (n2 + 1) * 512],
                                    start=(i2 == 0), stop=(i2 == 1),
                                )
                        if OUT_DRAIN[t % len(OUT_DRAIN)] == "A":
                            nc.scalar.copy(ot[:], po[:])
                        else:
                            nc.vector.tensor_copy(ot[:], po[:])
                        (nc.scalar if KB_NOSWDGE else nc.gpsimd).dma_start(
                            out=out_d[t * 128:(t + 1) * 128, :], in_=ot[:])
                    return [(1000, unit)]

                fillers = []

                def pop_fillers(budget_ns):
                    spent = 0
                    while fillers and spent < budget_ns:
                        ns, unit = fillers.pop(0)
                        unit()
                        spent += ns

                # ---------------- lead-in ----------------
                # xt8 chunk 0 + wk8 first: they gate the whole pipeline
                def dma_xt(sc):
                    dma(xt8v[:, :, sc * SC:(sc + 1) * SC],
                        xt8_d[:].rearrange("p (j s) -> p j s", s=S)[:, :, sc * SC:(sc + 1) * SC])
                dma_xt(0)
                dma(wk8[:], wk_d[:])
                dma(bqk[:], bqk_d[:])
                dma(wq8[:], wq_d[:])
                dma_xt(1)
                dma(ident[:], id_d[:])
                dma_xt(2)
                dma_xt(3)
                dma(wv8[:], wv_d[:])
                dma(bv8p[:], bv8_d[:])
                dma(wes[:], wes_d[:])
                dma(bes[:], bes_d[:])
                dma(gexp[:], gexp_d[:])
                dma(wo[:], wo_d[:])
                # keep the pair transfers behind xt8-sc0/wk8/wq8 in the
                # shared DMA queue: they are not needed until the mean-pool
                with tc.tile_wait_until(0.0035):
                    for pr in range(ST // 2):
                        load_x_pair(pr)

                # minimal critical path: k i=0 sc=0 + q i=0 sc=0 lets chunk-0
                # scores start; the other k i=0 chunks interleave with the
                # first a-stretch (k-sc j emitted just before skp 2j).
                if KB_LEADIN >= 1:
                    qk_unit(wk8v, kT8, bqk[:, 2:3], 0, 0, QK_DRAIN[0])
                if KB_LEADIN >= 2:
                    qk_unit(wq8v, qT8, bqk[:, 0:1], 0, 0, QK_DRAIN[4])

                # deferred to fillers: all of i=1 (needed from chunk 4)
                for sc in range(NSC):
                    fillers.append((1100, (lambda s: lambda: qk_unit(
                        wk8v, kT8, bqk[:, 3:4], 1, s, QK_DRAIN[(8 + s) % 16]))(sc)))
                for sc in range(NSC):
                    fillers.append((1100, (lambda s: lambda: qk_unit(
                        wq8v, qT8, bqk[:, 1:2], 1, s, QK_DRAIN[(12 + s) % 16]))(sc)))

                def gates_block():
                    xcp = PCS.tile([128, DT], fp32, tag="cp", name="xcp")
                    for hf in range(2):
                        nc.vector.tensor_copy(
                            xmrow[:, hf * 512:(hf + 1) * 512], xmp[hf][0:1, :])
                    for j in range(DT):
                        nc.tensor.matmul(
                            xcp[:, j: j + 1],
                            lhsT=xmrow[:, j * 128:(j + 1) * 128],
                            rhs=ones1[:],
                            start=True, stop=True,
                        )
                    nc.vector.tensor_copy(xm_col[:], xcp[:])
                    gpe = PCS.tile([HL, 1], fp32, tag="cp", name="gpe")
                    gps = PCS.tile([HL, 1], fp32, tag="cp", name="gps")
                    for j in range(DT):
                        nc.tensor.matmul(
                            gpe[:], lhsT=wes[:, j * 8: j * 8 + 4],
                            rhs=xm_col[:, j: j + 1],
                            start=(j == 0), stop=(j == DT - 1),
                        )
                    for j in range(DT):
                        nc.tensor.matmul(
                            gps[:], lhsT=wes[:, j * 8 + 4: j * 8 + 8],
                            rhs=xm_col[:, j: j + 1],
                            start=(j == 0), stop=(j == DT - 1),
                        )
                    eth = GS.tile([HL, 1], fp32, tag="eth")
                    saf = GS.tile([HL, 1], fp32, tag="saf")
                    gate = GS.tile([HL, 1], fp32, tag="gate")
                    nc.scalar.activation(eth[:], gpe[:], AF.Sigmoid,
                                         bias=bes[:, 0:1], scale=1.0 / S)
                    nc.scalar.activation(saf[:], gps[:], AF.Sigmoid,
                                         bias=bes[:, 1:2], scale=1.0 / S)
                    nc.vector.tensor_mul(gate[:], eth[:], saf[:])
                    for i in range(2):
                        pgc = PCS.tile([128, 1], fp32, tag="cp", name=f"pgc{i}")
                        nc.tensor.matmul(
                            pgc[:], lhsT=gexp[:, i * 128:(i + 1) * 128], rhs=gate[:],
                            start=True, stop=True,
                        )
                        nc.vector.tensor_copy(gcol[:, i: i + 1], pgc[:])

                # ---------------- chunk loop ----------------
                def alloc_cps(i, sc):
                    # two 1-bank tiles, each holding two u-slots of [128, 130];
                    # a zero rank-1 matmul opens each bank's zero region so the
                    # interleaved PV accumulations need no group bookkeeping
                    pair = [PCS.tile([128, 2, 130], fp32, tag="cp",
                                     name=f"cp{i}_{sc}_{w}") for w in range(2)]
                    for w in range(2):
                        for a2 in range(2):
                            nc.tensor.matmul(
                                pair[w][:, a2, :],
                                lhsT=ones8r[:].rearrange("o (c m) -> o c m", c=2),
                                rhs=z8row[:].rearrange("o (c m) -> o c m", c=2)[:, :, 0:130],
                                start=True, stop=True,
                                skip_group_check=(a2 == 1),
                                perf_mode=DRm,
                            )
                    return [pair[u // 2][:, u % 2, :] for u in range(SC // 128)]

                def pv_mm_u(cps, ets, i, hh, u):
                    h = 2 * i + hh
                    for sk in range(ST):
                        skp, half = sk // 2, sk % 2
                        nc.tensor.matmul(
                            cps[u][:, hh * 65:(hh + 1) * 65],
                            lhsT=ets[skp][:, half * SC + u * 128: half * SC + (u + 1) * 128],
                            rhs=v[:, sk * HL * 65 + h * 65: sk * HL * 65 + (h + 1) * 65],
                            start=(sk == 0), stop=(sk == ST - 1),
                        )

                def make_tail_parts(i, sc, cps, last):
                    """Staggered per-u closures: partN(u) = DVE normalize only;
                    partT(u) = PE transpose (+ drain at odd u), emitted one
                    slot later so the transpose never queues on PE before its
                    normalize has finished on DVE. PV ran in the b-stretch."""
                    state = {"tp": None}

                    def partN(u):
                        def f():
                            t = sc * (SC // 128) + u
                            rec2 = RC.tile([128, 2], fp32, tag="rec",
                                           name=f"rec{i}_{t}")
                            cpv = cps[u].rearrange("p (h c) -> p h c", c=65)
                            nc.vector.reciprocal(rec2[:], cpv[:, :, 64])
                            with nc.allow_low_precision("softmax-normalized bf16 ctx"):
                                nc.vector.tensor_tensor(
                                    out=ctx[:, t * DL + i * 128: t * DL + (i + 1) * 128]
                                        .rearrange("p (h c) -> p h c", c=HD),
                                    in0=cpv[:, :, 0:HD],
                                    in1=rec2[:].unsqueeze(2).broadcast_to([128, 2, HD]),
                                    op=ALU.mult,
                                )
                        return f

                    def partT(u):
                        def f():
                            if u % 2 == 0:
                                state["tp"] = PM.tile([128, 256], bf16, tag="pm",
                                                      name=f"tp{i}_{sc}_{u // 2}")
                            tp = state["tp"]
                            t = sc * (SC // 128) + u
                            nc.tensor.transpose(
                                tp[:, (u % 2) * 128:(u % 2 + 1) * 128],
                                ctx[:, t * DL + i * 128: t * DL + (i + 1) * 128],
                                ident[:],
                            )
                            if last:
                                # per-u drain so the final out-projs pipeline
                                nc.vector.tensor_scalar(
                                    out=ctxT[:, i * S + t * 128: i * S + (t + 1) * 128],
                                    in0=tp[:, (u % 2) * 128:(u % 2 + 1) * 128],
                                    scalar1=gcol[:, i: i + 1],
                                    scalar2=None,
                                    op0=ALU.mult,
                                )
                                for _, unit in outproj_units(t):
                                    unit()
                            elif u % 2 == 1:
                                nc.vector.tensor_scalar(
                                    out=ctxT[:, i * S + (t - 1) * 128: i * S + (t + 1) * 128],
                                    in0=tp[:],
                                    scalar1=gcol[:, i: i + 1],
                                    scalar2=None,
                                    op0=ALU.mult,
                                )
                                if u == SC // 128 - 1 and i == 1:
                                    for t2 in range(sc * 4, sc * 4 + 4):
                                        fillers.extend(outproj_units(t2))
                        return f

                    parts = []
                    nn = [partN(u) for u in range(SC // 128)]
                    tt = [partT(u) for u in range(SC // 128)]
                    parts.append(nn[0])
                    for u in range(1, SC // 128):
                        parts.append(lambda a=nn[u], b=tt[u - 1]: (a(), b()))
                    parts.append(tt[SC // 128 - 1])
                    return parts

                nchunks = [(i, sc) for i in range(2) for sc in range(NSC)][:KB_CHUNKS]
                pending_parts = []
                for n, (i, sc) in enumerate(nchunks):
                    first = (n == 0)
                    if first:
                        # mean-pool runs here: PE is otherwise idle during the
                        # first a-stretch and the PC psum pool is free.
                        xmp[0] = PCS.tile([64, 512], fp32, tag="cp", name="xmp0")
                        xmp[1] = PCS.tile([64, 512], fp32, tag="cp", name="xmp1")
                    ets_a = []
                    for skp in range(ST // 2):
                        if first and skp in (2, 4, 6):
                            # k i=0 chunk j just ahead of the scores needing it
                            qk_unit(wk8v, kT8, bqk[:, 2:3], 0, skp // 2,
                                    QK_DRAIN[skp // 2])
                        ets_a.append(score_exp(i, sc, 0, skp,
                                               EXP_PAT[n % len(EXP_PAT)][skp]))
                        if skp >= 1 and pending_parts:
                            pending_parts.pop(0)()
                        elif skp >= 2 and not first:
                            pop_fillers(FILLER_NS)
                    while pending_parts:
                        pending_parts.pop(0)()
                    if first:
                        # keep these off the critical lead-in: the scheduler
                        # would otherwise hoist them ahead of the k/q units
                        # and head-block PE on the slow x8-pair DMAs.
                        with tc.tile_wait_until(0.012):
                            for pr in range(ST // 2):
                                meanpool_mm(pr, xmp)
                        with tc.tile_wait_until(0.014):
                            gates_block()
                    cps = alloc_cps(i, sc)
                    ets_b = []
                    for skp in range(ST // 2):
                        ets_b.append(score_exp(i, sc, 1, skp,
                                               EXP_PAT[n % len(EXP_PAT)][8 + skp]))
                        if first:
                            v_unit(skp, V_DRAIN[skp % 8])
                        # PV for both heads streams through the b-stretch.
                        # Head b lags one skp so PE never queues behind the
                        # exp that was just issued for this skp.
                        pv_mm(cps, ets_a, i, 0, 2 * skp)
                        pv_mm(cps, ets_a, i, 0, 2 * skp + 1)
                        if skp >= 1:
                            pv_mm(cps, ets_b, i, 1, 2 * (skp - 1))
                            pv_mm(cps, ets_b, i, 1, 2 * (skp - 1) + 1)
                        if skp == 5 and i == 0 and sc < NSC - 1:
                            # q i=0 chunk sc+1 mid-b-stretch, off the boundary
                            qk_unit(wq8v, qT8, bqk[:, 0:1], 0, sc + 1,
                                    QK_DRAIN[4 + sc + 1])
                        if not first:
                            pop_fillers(FILLER_NS)
                    pending_parts = make_tail_parts(
                        i, sc, cps, last=(n == len(nchunks) - 1))
                    # the last head-b PV pair rides into the next a-stretch so
                    # the chunk boundary never waits on the final exp
                    def last_pv(cps=cps, ets_b=ets_b, i=i):
                        pv_mm(cps, ets_b, i, 1, ST - 2)
                        pv_mm(cps, ets_b, i, 1, ST - 1)
                    pending_parts.insert(0, last_pv)
                if KB_CHUNKS == 8:
                    for p in pending_parts:
                        p()
                    pop_fillers(10**9)
                else:
                    pending_parts.clear()
                    fillers.clear()
                    # touch out so the output DMA graph exists
                    ot = OB.tile([128, D], bf16, tag="ot", name="ot_stub")
                    nc.vector.memset(ot[:], 0.0)
                    nc.sync.dma_start(out=out_d[0:128, :], in_=ot[:])

    _split_multi_waits(nc)
    return nc


def _split_multi_waits(nc, skip=("InstEventSemaphore",)):
    """Hoist extra sync waits onto preceding same-engine NoOps.

    Walrus codegen can attach only one sync wait to some instruction
    encodings, so any instruction carrying N>1 waits is rewritten as N-1
    single-wait NoOps followed by the instruction with the last wait.
    """
    import concourse.mybir as mybir

    eng = {
        "EngineType.PE": nc.tensor,
        "EngineType.DVE": nc.vector,
        "EngineType.Activation": nc.scalar,
        "EngineType.Pool": nc.gpsimd,
        "EngineType.SP": nc.sync,
    }

    def fresh_nop(engine_key):
        nop = eng[engine_key].nop(hint="wsplit").ins
        for fn in nc.m.functions:
            for bb in fn.blocks:
                for i, ins in enumerate(bb.instructions):
                    if ins.name == nop.name:
                        del bb.instructions[i]
                        return nop
        raise RuntimeError("fresh nop not found")

    for fn in nc.m.functions:
        for bb in fn.blocks:
            insertions = []
            for idx, ins in enumerate(bb.instructions):
                if type(ins).__name__ in skip:
                    continue
                si = ins.sync_info
                if si is None or len(si.on_wait) <= 1:
                    continue
                waits = list(si.on_wait)
                nops = []
                for w in waits[:-1]:
                    nop = fresh_nop(str(ins.engine))
                    nop.sync_info = mybir.SyncInfo(on_wait=[w], on_update=[])
                    nops.append(nop)
                ins.sync_info = mybir.SyncInfo(
                    on_wait=[waits[-1]], on_update=list(si.on_update)
                )
                insertions.append((idx, nops))
            for idx, nops in reversed(insertions):
                bb.instructions[idx:idx] = nops


def _in_maps(inputs):
    x = np.ascontiguousarray(inputs["x"], np.float32)
    maps = []
    ident = np.eye(128, dtype=np.float32)
    gexp = np.zeros((HL, DL), np.float32)
    for h in range(HL):
        gexp[h, h * HD:(h + 1) * HD] = 1.0
    x8 = x.astype(F8)          # [B, S, D]
    for c in range(NCORES):
        b, g = c // HG, c % HG
        sl = slice(g * DL, (g + 1) * DL)
        hsl = slice(g * HL, (g + 1) * HL)
        # xT fp8 on 64 partitions: [64, jj, s] = x[b][s, jj*64+p]
        xt8 = np.ascontiguousarray(
            x8[b].T.reshape(2 * DT, 64, S).transpose(1, 0, 2).reshape(64, 2 * DT * S))
        # natural-x pairs: [128, pr, c2, col] = x[b][pr*256 + c2*128 + p, col]
        x8n = np.ascontiguousarray(
            x8[b].reshape(ST // 2, 2, 128, D).transpose(2, 0, 1, 3).reshape(128, ST * D))
        def wtile(w):
            return np.ascontiguousarray(
                w[:, sl].reshape(2 * DT, 64, DL).transpose(1, 0, 2).reshape(64, 2 * DT * DL).astype(F8))
        wo = inputs["Wo"][sl, :].reshape(2, 128, D).transpose(1, 0, 2).reshape(128, 2 * D)
        bqk = np.stack([
            inputs["bq"][sl][0:128], inputs["bq"][sl][128:256],
            inputs["bk"][sl][0:128], inputs["bk"][sl][128:256],
        ], axis=1).astype(np.float32)
        bv8p = np.zeros((1, 2 * DL), F8)
        bv8p[0, 0:DL] = inputs["bv"][sl].astype(F8)
        wes = np.concatenate([inputs["We"][:, hsl], inputs["Ws"][:, hsl]], axis=1)
        wes = wes.reshape(DT, 128, 2 * HL).transpose(1, 0, 2).reshape(128, DT * 2 * HL)
        bes = np.stack([inputs["be"][hsl], inputs["bs"][hsl]], axis=1)
        maps.append({
            "xt8": xt8,
            "x8": x8n,
            "wq8": wtile(inputs["Wq"]),
            "wk8": wtile(inputs["Wk"]),
            "wv8": wtile(inputs["Wv"]),
            "wo": np.ascontiguousarray(wo.astype(BF16)),
            "bqk": np.ascontiguousarray(bqk),
            "bv8p": bv8p,
            "wes": np.ascontiguousarray(wes.astype(np.float32)),
            "bes": np.ascontiguousarray(bes.astype(np.float32)),
            "gexp": gexp,
            "ident": ident.astype(BF16),
        })
    return maps


def kernel(**inputs):
    if "nc" not in _CACHE:
        _CACHE["nc"] = _build_nc()
    nc = _CACHE["nc"]
    maps = _in_maps({k: np.asarray(v) for k, v in inputs.items()})

    from concourse.bass_utils import run_bass_kernel_spmd

    trace = bool(int(os.environ.get("KERNEL_TRACE", "0")))
    res = run_bass_kernel_spmd(
        nc, maps, list(range(NCORES)), trace=trace,
        tmpdir=os.environ.get("KERNEL_TRACE_DIR") if trace else None,
    )
    _CACHE["last_result"] = res
    bo = np.asarray(inputs["bo"], np.float32)
    out = np.zeros((B, S, D), np.float32)
    for b in range(B):
        acc = np.zeros((S, D), np.float32)
        for g in range(HG):
            acc += res.results[b * HG + g]["out"].astype(np.float32)
        out[b] = acc + bo
    return out


# revision 40
# speedup vs baseline: 1.0235x; 1.0235x over previous
"""Trainium2 Bass kernel for nn_ASISNativeAttention (B=2,S=2048,D=1024,H=16).

Sharding: 8 cores = 2 batches x 4 head-groups (4 heads each); host splits
inputs per core and sums the 4 partial output projections per batch (+bo).

v2 design (vs the all-bf16 v1 at ~208us modeled): the two elementwise
engines (ACT, DVE) are the roofline -- 131K partition-lines of exp per core
-- so the exp work is split across BOTH of them, and the big matmuls move
to fp8 DoubleRow mode (2 k-tiles per instruction at half the per-row cost)
to keep PE under that roof:

  PE    q/k/v projections in fp8 DoubleRow from a host-transposed fp8 xT
        laid out on 64 partitions x 16 d-tiles (the PE hangs if a dual-fp8
        ldweights base partition changes inside one accumulation chain, and
        2*K*M may not exceed the 128x128 array, so chains stay K=64);
        scores per head via DoubleRow with a stride-0 replicated k-tile
        pair (computes 2*q.k, folded into the exp scale 1/16); PV and
        out-proj in bf16; mean-pool via fp8 DoubleRow ones-matmuls.
  ACT   exact exp (scale=1/16, psum->bf16) for ~60% of score tiles; its
        share of the qk/v/out-proj drains; the two gate sigmoids.
  DVE   Schraudolph exp for the rest: one tensor_scalar
        (psum*A16+B16 -> int16, round-to-nearest) whose int16 bits ARE the
        bf16 probs (max rel err ~3.5%, invisible after softmax: the PV
        ones-column sums the same stored values for the denominator);
        ctx normalize (strided reciprocal + stride-0-broadcast
        tensor_tensor); gated ctxT drain in 2x mode; remaining drains.
  Pool  x8-pair and output DMA via SWDGE, keeping HWDGE/SP free.

PSUM (8 banks): scores double-buffer 2x2 banks in a 3-slot pool shared
with transient out-proj/transpose/projection tiles; PV accumulators 2
banks (zero-opened by a rank-1 matmul so the interleaved per-head
accumulations need no group bookkeeping); per-2KB-bank accumulation
groups follow the zero-region rules (one pending group per region).

Measured (test.py): modeled exec 169.7us (v1 baseline: 208.0us); hardware
rel err vs reference 1.33e-2 (gate 2e-2; fp8 exposure is limited to
x / Wq,Wk,Wv / stored qT,kT -- probs, v, ctx, Wo stay bf16).

kernel.py is self-contained: numpy/ml_dtypes + the concourse tree at
/opt/trn_rl_repo.
"""

import os
import sys
import numpy as np
import ml_dtypes

BF16 = ml_dtypes.bfloat16
F8 = ml_dtypes.float8_e4m3

sys.path.insert(0, "/opt/trn_rl_repo")

B, S, D, H = 2, 2048, 1024, 16
HD = 64          # head dim
NCORES = 8
HG = 4           # head groups = cores per batch
HL = H // HG     # heads per core (4)
DL = D // HG     # local width (256)
ST = S // 128    # 16 s-tiles
DT = D // 128    # 8 d-tiles
SC = 512         # sq chunk width for scores
NSC = S // SC    # 4 chunks

# Schraudolph exp constants: int16 = rint(psum * A16 + B16); psum holds
# 2*q.k so the effective exp argument is psum/16 = q.k/8.
_C_SCH = 360000.0
A16 = float(2.0**7 / np.log(2.0) / 16.0)
B16 = float(127 * 2**7 - _C_SCH / 65536.0)

# --- engine-assignment knobs (tuned against TimelineSim) ---
# exp engine per chunk (16 chars: a-stretch skp 0-7 then b-stretch skp 0-7):
# 'A' = ACT exact exp, 'D' = DVE Schraudolph
EXP_PAT = [
    "AADAADAD" "AADAADAD",   # 10A chunks
    "AADAADAD" "ADAADADD",   # 9A chunks
] * 3 + ["AADAADAD" "AADAADAD", "AADAADAD" "ADAADDDA"]
if os.environ.get("KB_ALLACT"):
    EXP_PAT = ["A" * 16] * 8
KB_NOSWDGE = bool(os.environ.get("KB_NOSWDGE"))
KB_CHUNKS = int(os.environ.get("KB_CHUNKS", "8"))
KB_LEADIN = int(os.environ.get("KB_LEADIN", "99"))
# qk projection drain engines, one char per unit (k-i0 x4, q-i0 x4, then 8 i1)
QK_DRAIN = "ADADADADADADADAD"
# v drain engines, one per psum group (8)
V_DRAIN = "ADADADAD"
# out-proj drain engines, one per po half (32)
OUT_DRAIN = "AD" * 16
FILLER_NS = 1500   # filler budget popped per b-stretch skp

_CACHE = {}


def _build_nc():
    import concourse.bass as bass
    import concourse.mybir as mybir
    from concourse.tile import TileContext

    fp32 = mybir.dt.float32
    bf16 = mybir.dt.bfloat16
    f8 = mybir.dt.float8e4
    i16 = mybir.dt.int16
    AF = mybir.ActivationFunctionType
    ALU = mybir.AluOpType
    DRm = mybir.MatmulPerfMode.DoubleRow

    nc = bass.Bass()

    xt8_d = nc.declare_dram_parameter("xt8", [64, 2 * DT * S], f8, isOutput=False)
    x8_d = nc.declare_dram_parameter("x8", [128, ST * D], f8, isOutput=False)
    wq_d = nc.declare_dram_parameter("wq8", [64, 2 * DT * DL], f8, isOutput=False)
    wk_d = nc.declare_dram_parameter("wk8", [64, 2 * DT * DL], f8, isOutput=False)
    wv_d = nc.declare_dram_parameter("wv8", [64, 2 * DT * DL], f8, isOutput=False)
    wo_d = nc.declare_dram_parameter("wo", [128, 2 * D], bf16, isOutput=False)
    bqk_d = nc.declare_dram_parameter("bqk", [128, 4], fp32, isOutput=False)
    bv8_d = nc.declare_dram_parameter("bv8p", [1, 2 * DL], f8, isOutput=False)
    wes_d = nc.declare_dram_parameter("wes", [128, DT * 2 * HL], fp32, isOutput=False)
    bes_d = nc.declare_dram_parameter("bes", [HL, 2], fp32, isOutput=False)
    gexp_d = nc.declare_dram_parameter("gexp", [HL, DL], fp32, isOutput=False)
    id_d = nc.declare_dram_parameter("ident", [128, 128], bf16, isOutput=False)
    out_d = nc.declare_dram_parameter("out", [S, D], bf16, isOutput=True)

    with TileContext(nc) as tc:
        with tc.tile_pool(name="persist", bufs=1) as P:
            xt8 = P.tile([64, 2 * DT * S], f8, tag="xt8")
            wq8 = P.tile([64, 2 * DT * DL], f8, tag="wq8")
            wk8 = P.tile([64, 2 * DT * DL], f8, tag="wk8")
            wv8 = P.tile([64, 2 * DT * DL], f8, tag="wv8")
            wo = P.tile([128, 2 * D], bf16, tag="wo")
            qT8 = P.tile([128, 2 * S], f8, tag="qT8")
            kT8 = P.tile([128, 2 * S], f8, tag="kT8")
            v = P.tile([128, ST * HL * 65], bf16, tag="v")
            ctx = P.tile([128, ST * DL], bf16, tag="ctx")
            ctxT = P.tile([128, 2 * S], bf16, tag="ctxT")
            bqk = P.tile([128, 4], fp32, tag="bqk")
            bv8p = P.tile([1, 2 * DL], f8, tag="bv8p")
            ones82 = P.tile([128, 2, 64], f8, tag="ones82")
            ones8r = P.tile([1, 2 * 128], f8, tag="ones8r")
            z8row = P.tile([1, 2 * 260], f8, tag="z8row")
            wes = P.tile([128, DT * 2 * HL], fp32, tag="wes")
            bes = P.tile([HL, 2], fp32, tag="bes")
            gexp = P.tile([HL, DL], fp32, tag="gexp")
            ident = P.tile([128, 128], bf16, tag="ident")
            ones1 = P.tile([1, 1], fp32, tag="ones1")
            xmrow = P.tile([1, D], fp32, tag="xmrow")
            xm_col = P.tile([128, DT], fp32, tag="xm_col")
            gcol = P.tile([128, 2], fp32, tag="gcol")

            dma = nc.sync.dma_start

            def vview(t):
                return v[:].rearrange("p (t h c) -> p t h c", h=HL, c=65)[:, t]

            nc.vector.memset(ones82[:], 1.0)
            nc.vector.memset(ones8r[:], 1.0)
            nc.vector.memset(z8row[:], 0.0)
            nc.vector.memset(ones1[:], 1.0)
            # constant softmax-denominator columns of v
            nc.vector.memset(
                v[:].rearrange("p (t h c) -> p t h c", h=HL, c=65)[:, :, :, 64:65], 1.0
            )

            xt8v = xt8[:].rearrange("p (j s) -> p j s", s=S)      # [64, 16, S]
            wq8v = wq8[:].rearrange("p (j m) -> p j m", m=DL)     # [64, 16, DL]
            wk8v = wk8[:].rearrange("p (j m) -> p j m", m=DL)
            wv8v = wv8[:].rearrange("p (j m) -> p j m", m=DL)

            with (
                tc.tile_pool(name="x8l", bufs=8) as XL,
                tc.tile_pool(name="pm", bufs=3, space="PSUM") as PM,
                tc.tile_pool(name="pcps", bufs=2, space="PSUM") as PCS,
                tc.tile_pool(name="ets", bufs=26) as EX,
                tc.tile_pool(name="rc", bufs=8) as RC,
                tc.tile_pool(name="ob", bufs=2) as OB,
                tc.tile_pool(name="gs", bufs=1) as GS,
            ):
                xmp = [None, None]

                xbs = []

                def load_x_pair(pr):
                    """DMA natural-x pair pr via SWDGE (Pool) off the HWDGE path."""
                    xb = XL.tile([128, 2, D], f8, tag="xb", name=f"xb{pr}")
                    # SP HWDGE: queues naturally behind the critical xt8/w
                    # loads instead of jumping ahead from the idle Pool queue
                    nc.sync.dma_start(
                        out=xb[:],
                        in_=x8_d[:].rearrange("p (r c d) -> p r c d", c=2, d=D)[:, pr])
                    xbs.append(xb)

                def meanpool_mm(pr, xmps):
                    lhs = ones82[:]  # [128, 2, 64]: dual-fp8 ldweights needs wide M
                    for qh in range(4):
                        half, qq = qh // 2, qh % 2
                        first = (pr == 0 and qq == 0)
                        nc.tensor.matmul(
                            xmps[half][:, qq * 256:(qq + 1) * 256],
                            lhsT=lhs,
                            rhs=xbs[pr][:, :, qh * 256:(qh + 1) * 256],
                            start=first, stop=first,
                            skip_group_check=not first,
                            perf_mode=DRm,
                        )

                def qk_unit(w8v, dst8, bcol, i, sc, eng):
                    """Project one [128, SC] chunk of qT or kT (fp8 out)."""
                    pp = PM.tile([128, SC], fp32, tag="pm",
                                 name=f"pp{dst8.tensor.name}_{i}_{sc}")
                    for qq in range(2):
                        # x / weights live on 64 partitions x 16 d-tiles:
                        # dual-fp8 DR caps 2*K*M at the PE array size and the
                        # PE hangs if ldweights base-partition changes inside
                        # an accumulation chain, so every chain stays K=64.
                        for dp in range(DT):
                            first = (qq == 0 and dp == 0)
                            nc.tensor.matmul(
                                pp[:, qq * 256:(qq + 1) * 256],
                                lhsT=w8v[:, 2 * dp:2 * dp + 2, i * 128:(i + 1) * 128],
                                rhs=xt8v[:, 2 * dp:2 * dp + 2,
                                         sc * SC + qq * 256: sc * SC + (qq + 1) * 256],
                                start=first, stop=first,
                                skip_group_check=not first,
                                perf_mode=DRm,
                            )
                    dst = dst8[:, i * S + sc * SC: i * S + (sc + 1) * SC]
                    if eng == "A":
                        nc.scalar.activation(dst, pp[:], AF.Identity, bias=bcol)
                    else:
                        nc.vector.tensor_scalar(
                            out=dst, in0=pp[:], scalar1=bcol, scalar2=None,
                            op0=ALU.add,
                        )

                def v_unit(g, eng):
                    """Project v for s-tiles 2g, 2g+1 (one psum bank)."""
                    pv = PM.tile([128, 2 * DL], fp32, tag="pm", name=f"pv{g}")
                    for t2 in range(2):
                        t = 2 * g + t2
                        sl = pv[:, t2 * DL:(t2 + 1) * DL]
                        nc.tensor.matmul(
                            sl, lhsT=ones8r[:].rearrange("o (c m) -> o c m", c=2),
                            rhs=bv8p[:].rearrange("o (c m) -> o c m", c=2),
                            start=(t2 == 0), stop=(t2 == 0),
                            skip_group_check=(t2 == 1), perf_mode=DRm,
                        )
                        for dp in range(DT):
                            nc.tensor.matmul(
                                sl,
                                lhsT=xt8v[:, 2 * dp:2 * dp + 2, t * 128:(t + 1) * 128],
                                rhs=wv8v[:, 2 * dp:2 * dp + 2, :],
                                start=False, stop=False,
                                skip_group_check=True,
                                perf_mode=DRm,
                            )
                    for t2 in range(2):
                        t = 2 * g + t2
                        src = pv[:, t2 * DL:(t2 + 1) * DL].rearrange(
                            "p (h c) -> p h c", c=HD)
                        dst = vview(t)[:, :, 0:HD]
                        if eng == "A":
                            nc.scalar.copy(dst, src)
                        else:
                            nc.vector.tensor_copy(dst, src)

                def score_exp(i, sc, hh, skp, eng):
                    """Scores for sk-tiles (2skp, 2skp+1) x [sc*SC, (sc+1)*SC) of
                    head 2i+hh; one wide exp. psum holds 2*q.k (stride-0 DR)."""
                    r = hh * 64
                    sp = PM.tile([128, 2 * SC], fp32, tag="pm",
                                 name=f"sp{i}_{sc}_{hh}_{skp}")
                    for half in range(2):
                        sk = 2 * skp + half
                        lhsT = kT8[r:r + 64, i * S + sk * 128: i * S + (sk + 1) * 128] \
                            .unsqueeze(1).broadcast_to([64, 2, 128])
                        for qq in range(2):
                            rhs = qT8[r:r + 64,
                                      i * S + sc * SC + qq * 256: i * S + sc * SC + (qq + 1) * 256] \
                                .unsqueeze(1).broadcast_to([64, 2, 256])
                            # qq0 opens the bank's zero region; qq1 assigns
                            # into still-pending bytes (no second group)
                            nc.tensor.matmul(
                                sp[:, half * SC + qq * 256: half * SC + (qq + 1) * 256],
                                lhsT=lhsT, rhs=rhs, start=(qq == 0), stop=(qq == 0),
                                skip_group_check=(qq == 1),
                                perf_mode=DRm,
                            )
                    if eng == "A":
                        et = EX.tile([128, 2 * SC], bf16, tag="et",
                                     name=f"et{i}_{sc}_{hh}_{skp}")
                        nc.scalar.activation(et[:], sp[:], AF.Exp, scale=1.0 / 16.0)
                        return et[:]
                    et = EX.tile([128, 2 * SC], i16, tag="et",
                                 name=f"et{i}_{sc}_{hh}_{skp}")
                    nc.vector.tensor_scalar(
                        out=et[:], in0=sp[:], scalar1=A16, scalar2=B16,
                        op0=ALU.mult, op1=ALU.add,
                    )
                    return et[:].bitcast(bf16)

                def pv_mm(cps, ets, i, hh, sk):
                    h = 2 * i + hh
                    skp, half = sk // 2, sk % 2
                    for u in range(SC // 128):
                        nc.tensor.matmul(
                            cps[u][:, hh * 65:(hh + 1) * 65],
                            lhsT=ets[skp][:, half * SC + u * 128: half * SC + (u + 1) * 128],
                            rhs=v[:, sk * HL * 65 + h * 65: sk * HL * 65 + (h + 1) * 65],
                            start=False, stop=False, skip_group_check=True,
                        )

                def outproj_units(t):
                    def unit():
                        ot = OB.tile([128, D], bf16, tag="ot", name=f"ot{t}")
                        po = PM.tile([128, D], fp32, tag="pm", name=f"po{t}")
                        for n2 in range(2):
                            for i2 in range(2):
                                nc.tensor.matmul(
                                    po[:, n2 * 512:(n2 + 1) * 512],
                                    lhsT=ctxT[:, i2 * S + t * 128: i2 * S + (t + 1) * 128],
                                    rhs=wo[:, i2 * D + n2 * 512: i2 * D + (n2 + 1) * 512],
                                    start=(i2 == 0), stop=(i2 == 1),
                                )
                        if OUT_DRAIN[t % len(OUT_DRAIN)] == "A":
                            nc.scalar.copy(ot[:], po[:])
                        else:
                            nc.vector.tensor_copy(ot[:], po[:])
                        (nc.scalar if KB_NOSWDGE else nc.gpsimd).dma_start(
                            out=out_d[t * 128:(t + 1) * 128, :], in_=ot[:])
                    return [(1000, unit)]

                fillers = []

                def pop_fillers(budget_ns):
                    spent = 0
                    while fillers and spent < budget_ns:
                        ns, unit = fillers.pop(0)
                        unit()
                        spent += ns

                # ---------------- lead-in ----------------
                # xt8 chunk 0 + wk8 first: they gate the whole pipeline
                def dma_xt(sc):
                    dma(xt8v[:, :, sc * SC:(sc + 1) * SC],
                        xt8_d[:].rearrange("p (j s) -> p j s", s=S)[:, :, sc * SC:(sc + 1) * SC])
                dma_xt(0)
                dma(wk8[:], wk_d[:])
                dma(bqk[:], bqk_d[:])
                dma(wq8[:], wq_d[:])
                dma_xt(1)
                dma(ident[:], id_d[:])
                dma_xt(2)
                dma_xt(3)
                dma(wv8[:], wv_d[:])
                dma(bv8p[:], bv8_d[:])
                dma(wes[:], wes_d[:])
                dma(bes[:], bes_d[:])
                dma(gexp[:], gexp_d[:])
                dma(wo[:], wo_d[:])
                # keep the pair transfers behind xt8-sc0/wk8/wq8 in the
                # shared DMA queue: they are not needed until the mean-pool
                with tc.tile_wait_until(0.0035):
                    for pr in range(ST // 2):
                        load_x_pair(pr)

                # minimal critical path: k i=0 sc=0 + q i=0 sc=0 lets chunk-0
                # scores start; the other k i=0 chunks interleave with the
                # first a-stretch (k-sc j emitted just before skp 2j).
                if KB_LEADIN >= 1:
                    qk_unit(wk8v, kT8, bqk[:, 2:3], 0, 0, QK_DRAIN[0])
                if KB_LEADIN >= 2:
                    qk_unit(wq8v, qT8, bqk[:, 0:1], 0, 0, QK_DRAIN[4])

                # deferred to fillers: all of i=1 (needed from chunk 4)
                for sc in range(NSC):
                    fillers.append((1100, (lambda s: lambda: qk_unit(
                        wk8v, kT8, bqk[:, 3:4], 1, s, QK_DRAIN[(8 + s) % 16]))(sc)))
                for sc in range(NSC):
                    fillers.append((1100, (lambda s: lambda: qk_unit(
                        wq8v, qT8, bqk[:, 1:2], 1, s, QK_DRAIN[(12 + s) % 16]))(sc)))

                def gates_block():
                    xcp = PCS.tile([128, DT], fp32, tag="cp", name="xcp")
                    for hf in range(2):
                        nc.vector.tensor_copy(
                            xmrow[:, hf * 512:(hf + 1) * 512], xmp[hf][0:1, :])
                    for j in range(DT):
                        nc.tensor.matmul(
                            xcp[:, j: j + 1],
                            lhsT=xmrow[:, j * 128:(j + 1) * 128],
                            rhs=ones1[:],
                            start=True, stop=True,
                        )
                    nc.vector.tensor_copy(xm_col[:], xcp[:])
                    gpe = PCS.tile([HL, 1], fp32, tag="cp", name="gpe")
                    gps = PCS.tile([HL, 1], fp32, tag="cp", name="gps")
                    for j in range(DT):
                        nc.tensor.matmul(
                            gpe[:], lhsT=wes[:, j * 8: j * 8 + 4],
                            rhs=xm_col[:, j: j + 1],
                            start=(j == 0), stop=(j == DT - 1),
                        )
                    for j in range(DT):
                        nc.tensor.matmul(
                            gps[:], lhsT=wes[:, j * 8 + 4: j * 8 + 8],
                            rhs=xm_col[:, j: j + 1],
                            start=(j == 0), stop=(j == DT - 1),
                        )
                    eth = GS.tile([HL, 1], fp32, tag="eth")
                    saf = GS.tile([HL, 1], fp32, tag="saf")
                    gate = GS.tile([HL, 1], fp32, tag="gate")
                    nc.scalar.activation(eth[:], gpe[:], AF.Sigmoid,
                                         bias=bes[:, 0:1], scale=1.0 / S)
                    nc.scalar.activation(saf[:], gps[:], AF.Sigmoid,
                                         bias=bes[:, 1:2], scale=1.0 / S)
                    nc.vector.tensor_mul(gate[:], eth[:], saf[:])
                    for i in range(2):
                        pgc = PCS.tile([128, 1], fp32, tag="cp", name=f"pgc{i}")
                        nc.tensor.matmul(
                            pgc[:], lhsT=gexp[:, i * 128:(i + 1) * 128], rhs=gate[:],
                            start=True, stop=True,
                        )
                        nc.vector.tensor_copy(gcol[:, i: i + 1], pgc[:])

                # ---------------- chunk loop ----------------
                def alloc_cps(i, sc):
                    # two 1-bank tiles, each holding two u-slots of [128, 130];
                    # a zero rank-1 matmul opens each bank's zero region so the
                    # interleaved PV accumulations need no group bookkeeping
                    pair = [PCS.tile([128, 2, 130], fp32, tag="cp",
                                     name=f"cp{i}_{sc}_{w}") for w in range(2)]
                    for w in range(2):
                        for a2 in range(2):
                            nc.tensor.matmul(
                                pair[w][:, a2, :],
                                lhsT=ones8r[:].rearrange("o (c m) -> o c m", c=2),
                                rhs=z8row[:].rearrange("o (c m) -> o c m", c=2)[:, :, 0:130],
                                start=True, stop=True,
                                skip_group_check=(a2 == 1),
                                perf_mode=DRm,
                            )
                    return [pair[u // 2][:, u % 2, :] for u in range(SC // 128)]

                def pv_mm_u(cps, ets, i, hh, u):
                    h = 2 * i + hh
                    for sk in range(ST):
                        skp, half = sk // 2, sk % 2
                        nc.tensor.matmul(
                            cps[u][:, hh * 65:(hh + 1) * 65],
                            lhsT=ets[skp][:, half * SC + u * 128: half * SC + (u + 1) * 128],
                            rhs=v[:, sk * HL * 65 + h * 65: sk * HL * 65 + (h + 1) * 65],
                            start=(sk == 0), stop=(sk == ST - 1),
                        )

                def make_tail_parts(i, sc, cps, last):
                    """Staggered per-u closures: partN(u) = DVE normalize only;
                    partT(u) = PE transpose (+ drain at odd u), emitted one
                    slot later so the transpose never queues on PE before its
                    normalize has finished on DVE. PV ran in the b-stretch."""
                    state = {"tp": None}

                    def partN(u):
                        def f():
                            t = sc * (SC // 128) + u
                            rec2 = RC.tile([128, 2], fp32, tag="rec",
                                           name=f"rec{i}_{t}")
                            cpv = cps[u].rearrange("p (h c) -> p h c", c=65)
                            nc.vector.reciprocal(rec2[:], cpv[:, :, 64])
                            with nc.allow_low_precision("softmax-normalized bf16 ctx"):
                                nc.vector.tensor_tensor(
                                    out=ctx[:, t * DL + i * 128: t * DL + (i + 1) * 128]
                                        .rearrange("p (h c) -> p h c", c=HD),
                                    in0=cpv[:, :, 0:HD],
                                    in1=rec2[:].unsqueeze(2).broadcast_to([128, 2, HD]),
                                    op=ALU.mult,
                                )
                        return f

                    def partT(u):
                        def f():
                            if u % 2 == 0:
                                state["tp"] = PM.tile([128, 256], bf16, tag="pm",
                                                      name=f"tp{i}_{sc}_{u // 2}")
                            tp = state["tp"]
                            t = sc * (SC // 128) + u
                            nc.tensor.transpose(
                                tp[:, (u % 2) * 128:(u % 2 + 1) * 128],
                                ctx[:, t * DL + i * 128: t * DL + (i + 1) * 128],
                                ident[:],
                            )
                            if last:
                                # per-u drain so the final out-projs pipeline
                                nc.vector.tensor_scalar(
                                    out=ctxT[:, i * S + t * 128: i * S + (t + 1) * 128],
                                    in0=tp[:, (u % 2) * 128:(u % 2 + 1) * 128],
                                    scalar1=gcol[:, i: i + 1],
                                    scalar2=None,
                                    op0=ALU.mult,
                                )
                                for _, unit in outproj_units(t):
                                    unit()
                            elif u % 2 == 1:
                                nc.vector.tensor_scalar(
                                    out=ctxT[:, i * S + (t - 1) * 128: i * S + (t + 1) * 128],
                                    in0=tp[:],
                                    scalar1=gcol[:, i: i + 1],
                                    scalar2=None,
                                    op0=ALU.mult,
                                )
                                if u == SC // 128 - 1 and i == 1:
                                    for t2 in range(sc * 4, sc * 4 + 4):
                                        fillers.extend(outproj_units(t2))
                        return f

                    parts = []
                    nn = [partN(u) for u in range(SC // 128)]
                    tt = [partT(u) for u in range(SC // 128)]
                    parts.append(nn[0])
                    for u in range(1, SC // 128):
                        parts.append(lambda a=nn[u], b=tt[u - 1]: (a(), b()))
                    parts.append(tt[SC // 128 - 1])
                    return parts

                nchunks = [(i, sc) for i in range(2) for sc in range(NSC)][:KB_CHUNKS]
                pending_parts = []
                for n, (i, sc) in enumerate(nchunks):
                    first = (n == 0)
                    if first:
                        # mean-pool runs here: PE is otherwise idle during the
                        # first a-stretch and the PC psum pool is free.
                        xmp[0] = PCS.tile([64, 512], fp32, tag="cp", name="xmp0")
                        xmp[1] = PCS.tile([64, 512], fp32, tag="cp", name="xmp1")
                    ets_a = []
                    for skp in range(ST // 2):
                        if first and skp in (2, 4, 6):
                            # k i=0 chunk j just ahead of the scores needing it
                            qk_unit(wk8v, kT8, bqk[:, 2:3], 0, skp // 2,
                                    QK_DRAIN[skp // 2])
                        ets_a.append(score_exp(i, sc, 0, skp,
                                               EXP_PAT[n % len(EXP_PAT)][skp]))
                        if skp >= 1 and pending_parts:
                            pending_parts.pop(0)()
                        elif skp >= 2 and not first:
                            pop_fillers(FILLER_NS)
                    while pending_parts:
                        pending_parts.pop(0)()
                    if first:
                        # keep these off the critical lead-in: the scheduler
                        # would otherwise hoist them ahead of the k/q units
                        # and head-block PE on the slow x8-pair DMAs.
                        with tc.tile_wait_until(0.012):
                            for pr in range(ST // 2):
                                meanpool_mm(pr, xmp)
                        with tc.tile_wait_until(0.014):
                            gates_block()
                    cps = alloc_cps(i, sc)
                    ets_b = []
                    for skp in range(ST // 2):
                        ets_b.append(score_exp(i, sc, 1, skp,
                                               EXP_PAT[n % len(EXP_PAT)][8 + skp]))
                        if first:
                            v_unit(skp, V_DRAIN[skp % 8])
                        # PV for both heads streams through the b-stretch.
                        # Head b lags one skp so PE never queues behind the
                        # exp that was just issued for this skp.
                        pv_mm(cps, ets_a, i, 0, 2 * skp)
                        pv_mm(cps, ets_a, i, 0, 2 * skp + 1)
                        if skp >= 1:
                            pv_mm(cps, ets_b, i, 1, 2 * (skp - 1))
                            pv_mm(cps, ets_b, i, 1, 2 * (skp - 1) + 1)
                        if skp == 5 and i == 0 and sc < NSC - 1:
                            # q i=0 chunk sc+1 mid-b-stretch, off the boundary
                            qk_unit(wq8v, qT8, bqk[:, 0:1], 0, sc + 1,
                                    QK_DRAIN[4 + sc + 1])
                        if not first:
                            pop_fillers(FILLER_NS)
                    pending_parts = make_tail_parts(
                        i, sc, cps, last=(n == len(nchunks) - 1))
                    # the last head-b PV pair rides into the next a-stretch so
                    # the chunk boundary never waits on the final exp
                    def last_pv(cps=cps, ets_b=ets_b, i=i):
                        pv_mm(cps, ets_b, i, 1, ST - 2)
                        pv_mm(cps, ets_b, i, 1, ST - 1)
                    pending_parts.insert(0, last_pv)
                if KB_CHUNKS == 8:
                    for p in pending_parts:
                        p()
                    pop_fillers(10**9)
                else:
                    pending_parts.clear()
                    fillers.clear()
                    # touch out so the output DMA graph exists
                    ot = OB.tile([128, D], bf16, tag="ot", name="ot_stub")
                    nc.vector.memset(ot[:], 0.0)
                    nc.sync.dma_start(out=out_d[0:128, :], in_=ot[:])

    _split_multi_waits(nc)
    return nc


def _split_multi_waits(nc, skip=("InstEventSemaphore",)):
    """Hoist extra sync waits onto preceding same-engine NoOps.

    Walrus codegen can attach only one sync wait to some instruction
    encodings, so any instruction carrying N>1 waits is rewritten as N-1
    single-wait NoOps followed by the instruction with the last wait.
    """
    import concourse.mybir as mybir

    eng = {
        "EngineType.PE": nc.tensor,
        "EngineType.DVE": nc.vector,
        "EngineType.Activation": nc.scalar,
        "EngineType.Pool": nc.gpsimd,
        "EngineType.SP": nc.sync,
    }

    def fresh_nop(engine_key):
        nop = eng[engine_key].nop(hint="wsplit").ins
        for fn in nc.m.functions:
            for bb in fn.blocks:
                for i, ins in enumerate(bb.instructions):
                    if ins.name == nop.name:
                        del bb.instructions[i]
                        return nop
        raise RuntimeError("fresh nop not found")

    for fn in nc.m.functions:
        for bb in fn.blocks:
            insertions = []
            for idx, ins in enumerate(bb.instructions):
                if type(ins).__name__ in skip:
                    continue
                si = ins.sync_info
                if si is None or len(si.on_wait) <= 1:
                    continue
                waits = list(si.on_wait)
                nops = []
                for w in waits[:-1]:
                    nop = fresh_nop(str(ins.engine))
                    nop.sync_info = mybir.SyncInfo(on_wait=[w], on_update=[])
                    nops.append(nop)
                ins.sync_info = mybir.SyncInfo(
                    on_wait=[waits[-1]], on_update=list(si.on_update)
                )
                insertions.append((idx, nops))
            for idx, nops in reversed(insertions):
                bb.instructions[idx:idx] = nops


def _in_maps(inputs):
    x = np.ascontiguousarray(inputs["x"], np.float32)
    maps = []
    ident = np.eye(128, dtype=np.float32)
    gexp = np.zeros((HL, DL), np.float32)
    for h in range(HL):
        gexp[h, h * HD:(h + 1) * HD] = 1.0
    x8 = x.astype(F8)          # [B, S, D]
    for c in range(NCORES):
        b, g = c // HG, c % HG
        sl = slice(g * DL, (g + 1) * DL)
        hsl = slice(g * HL, (g + 1) * HL)
        # xT fp8 on 64 partitions: [64, jj, s] = x[b][s, jj*64+p]
        xt8 = np.ascontiguousarray(
            x8[b].T.reshape(2 * DT, 64, S).transpose(1, 0, 2).reshape(64, 2 * DT * S))
        # natural-x pairs: [128, pr, c2, col] = x[b][pr*256 + c2*128 + p, col]
        x8n = np.ascontiguousarray(
            x8[b].reshape(ST // 2, 2, 128, D).transpose(2, 0, 1, 3).reshape(128, ST * D))
        def wtile(w):
            return np.ascontiguousarray(
                w[:, sl].reshape(2 * DT, 64, DL).transpose(1, 0, 2).reshape(64, 2 * DT * DL).astype(F8))
        wo = inputs["Wo"][sl, :].reshape(2, 128, D).transpose(1, 0, 2).reshape(128, 2 * D)
        bqk = np.stack([
            inputs["bq"][sl][0:128], inputs["bq"][sl][128:256],
            inputs["bk"][sl][0:128], inputs["bk"][sl][128:256],
        ], axis=1).astype(np.float32)
        bv8p = np.zeros((1, 2 * DL), F8)
        bv8p[0, 0:DL] = inputs["bv"][sl].astype(F8)
        wes = np.concatenate([inputs["We"][:, hsl], inputs["Ws"][:, hsl]], axis=1)
        wes = wes.reshape(DT, 128, 2 * HL).transpose(1, 0, 2).reshape(128, DT * 2 * HL)
        bes = np.stack([inputs["be"][hsl], inputs["bs"][hsl]], axis=1)
        maps.append({
            "xt8": xt8,
            "x8": x8n,
            "wq8": wtile(inputs["Wq"]),
            "wk8": wtile(inputs["Wk"]),
            "wv8": wtile(inputs["Wv"]),
            "wo": np.ascontiguousarray(wo.astype(BF16)),
            "bqk": np.ascontiguousarray(bqk),
            "bv8p": bv8p,
            "wes": np.ascontiguousarray(wes.astype(np.float32)),
            "bes": np.ascontiguousarray(bes.astype(np.float32)),
            "gexp": gexp,
            "ident": ident.astype(BF16),
        })
    return maps


def kernel(**inputs):
    if "nc" not in _CACHE:
        _CACHE["nc"] = _build_nc()
    nc = _CACHE["nc"]
    maps = _in_maps({k: np.asarray(v) for k, v in inputs.items()})

    from concourse.bass_utils import run_bass_kernel_spmd

    trace = bool(int(os.environ.get("KERNEL_TRACE", "0")))
    res = run_bass_kernel_spmd(
        nc, maps, list(range(NCORES)), trace=trace,
        tmpdir=os.environ.get("KERNEL_TRACE_DIR") if trace else None,
    )
    _CACHE["last_result"] = res
    bo = np.asarray(inputs["bo"], np.float32)
    out = np.zeros((B, S, D), np.float32)
    for b in range(B):
        acc = np.zeros((S, D), np.float32)
        for g in range(HG):
            acc += res.results[b * HG + g]["out"].astype(np.float32)
        out[b] = acc + bo
    return out


# revision 44
# speedup vs baseline: 1.0382x; 1.0144x over previous
"""Trainium2 Bass kernel for nn_ASISNativeAttention (B=2,S=2048,D=1024,H=16).

Sharding: 8 cores = 2 batches x 4 head-groups (4 heads each); host splits
inputs per core and sums the 4 partial output projections per batch (+bo).

v2 design (vs the all-bf16 v1 at ~208us modeled): the two elementwise
engines (ACT, DVE) are the roofline -- 131K partition-lines of exp per core
-- so the exp work is split across BOTH of them, and the big matmuls move
to fp8 DoubleRow mode (2 k-tiles per instruction at half the per-row cost)
to keep PE under that roof:

  PE    q/k/v projections in fp8 DoubleRow from a host-transposed fp8 xT
        laid out on 64 partitions x 16 d-tiles (the PE hangs if a dual-fp8
        ldweights base partition changes inside one accumulation chain, and
        2*K*M may not exceed the 128x128 array, so chains stay K=64);
        scores per head via DoubleRow with a stride-0 replicated k-tile
        pair (computes 2*q.k, folded into the exp scale 1/16); PV and
        out-proj in bf16; mean-pool via fp8 DoubleRow ones-matmuls.
  ACT   exact exp (scale=1/16, psum->bf16) for ~60% of score tiles; its
        share of the qk/v/out-proj drains; the two gate sigmoids.
  DVE   Schraudolph exp for the rest: one tensor_scalar
        (psum*A16+B16 -> int16, round-to-nearest) whose int16 bits ARE the
        bf16 probs (max rel err ~3.5%, invisible after softmax: the PV
        ones-column sums the same stored values for the denominator);
        ctx normalize (strided reciprocal + stride-0-broadcast
        tensor_tensor); gated ctxT drain in 2x mode; remaining drains.
  Pool  x8-pair and output DMA via SWDGE, keeping HWDGE/SP free.

PSUM (8 banks): scores double-buffer 2x2 banks in a 3-slot pool shared
with transient out-proj/transpose/projection tiles; PV accumulators 2
banks (zero-opened by a rank-1 matmul so the interleaved per-head
accumulations need no group bookkeeping); per-2KB-bank accumulation
groups follow the zero-region rules (one pending group per region).

Measured (test.py): modeled exec 163.5us (v1 baseline: 208.0us); hardware
rel err vs reference 1.33e-2 (gate 2e-2; fp8 exposure is limited to
x / Wq,Wk,Wv / stored qT,kT -- probs, v, ctx, Wo stay bf16).

kernel.py is self-contained: numpy/ml_dtypes + the concourse tree at
/opt/trn_rl_repo.
"""

import os
import sys
import numpy as np
import ml_dtypes

BF16 = ml_dtypes.bfloat16
F8 = ml_dtypes.float8_e4m3

sys.path.insert(0, "/opt/trn_rl_repo")

B, S, D, H = 2, 2048, 1024, 16
HD = 64          # head dim
NCORES = 8
HG = 4           # head groups = cores per batch
HL = H // HG     # heads per core (4)
DL = D // HG     # local width (256)
ST = S // 128    # 16 s-tiles
DT = D // 128    # 8 d-tiles
SC = 512         # sq chunk width for scores
NSC = S // SC    # 4 chunks

# Schraudolph exp constants: int16 = rint(psum * A16 + B16); psum holds
# 2*q.k so the effective exp argument is psum/16 = q.k/8.
_C_SCH = 360000.0
A16 = float(2.0**7 / np.log(2.0) / 16.0)
B16 = float(127 * 2**7 - _C_SCH / 65536.0)

# --- engine-assignment knobs (tuned against TimelineSim) ---
# exp engine per chunk (16 chars: a-stretch skp 0-7 then b-stretch skp 0-7):
# 'A' = ACT exact exp, 'D' = DVE Schraudolph
EXP_PAT = [
    "AADAADAD" "AADAADAD",   # 10A chunks
    "AADAADAD" "ADAADADD",   # 9A chunks
] * 3 + ["AADAADAD" "AADAADAD", "AADAADAD" "ADAADDDA"]
if os.environ.get("KB_ALLACT"):
    EXP_PAT = ["A" * 16] * 8
KB_NOSWDGE = bool(os.environ.get("KB_NOSWDGE"))
KB_CHUNKS = int(os.environ.get("KB_CHUNKS", "8"))
KB_LEADIN = int(os.environ.get("KB_LEADIN", "99"))
# qk projection drain engines, one char per unit (k-i0 x4, q-i0 x4, then 8 i1)
QK_DRAIN = "ADADADADADADADAD"
# v drain engines, one per psum group (8)
V_DRAIN = "ADADADAD"
# out-proj drain engines, one per po half (32)
OUT_DRAIN = "AD" * 16
FILLER_NS = 1500   # filler budget popped per b-stretch skp

_CACHE = {}


def _build_nc():
    import concourse.bass as bass
    import concourse.mybir as mybir
    from concourse.tile import TileContext

    fp32 = mybir.dt.float32
    bf16 = mybir.dt.bfloat16
    f8 = mybir.dt.float8e4
    i16 = mybir.dt.int16
    AF = mybir.ActivationFunctionType
    ALU = mybir.AluOpType
    DRm = mybir.MatmulPerfMode.DoubleRow

    nc = bass.Bass()

    xt8_d = nc.declare_dram_parameter("xt8", [64, 2 * DT * S], f8, isOutput=False)
    x8_d = nc.declare_dram_parameter("x8", [128, ST * D], f8, isOutput=False)
    wq_d = nc.declare_dram_parameter("wq8", [64, 2 * DT * DL], f8, isOutput=False)
    wk_d = nc.declare_dram_parameter("wk8", [64, 2 * DT * DL], f8, isOutput=False)
    wv_d = nc.declare_dram_parameter("wv8", [64, 2 * DT * DL], f8, isOutput=False)
    wo_d = nc.declare_dram_parameter("wo", [128, 2 * D], bf16, isOutput=False)
    bqk_d = nc.declare_dram_parameter("bqk", [128, 4], fp32, isOutput=False)
    bv8_d = nc.declare_dram_parameter("bv8p", [1, 2 * DL], f8, isOutput=False)
    wes_d = nc.declare_dram_parameter("wes", [128, DT * 2 * HL], fp32, isOutput=False)
    bes_d = nc.declare_dram_parameter("bes", [HL, 2], fp32, isOutput=False)
    gexp_d = nc.declare_dram_parameter("gexp", [HL, DL], fp32, isOutput=False)
    id_d = nc.declare_dram_parameter("ident", [128, 128], bf16, isOutput=False)
    out_d = nc.declare_dram_parameter("out", [S, D], bf16, isOutput=True)

    with TileContext(nc) as tc:
        with tc.tile_pool(name="persist", bufs=1) as P:
            xt8 = P.tile([64, 2 * DT * S], f8, tag="xt8")
            wq8 = P.tile([64, 2 * DT * DL], f8, tag="wq8")
            wk8 = P.tile([64, 2 * DT * DL], f8, tag="wk8")
            wv8 = P.tile([64, 2 * DT * DL], f8, tag="wv8")
            wo = P.tile([128, 2 * D], bf16, tag="wo")
            qT8 = P.tile([128, 2 * S], f8, tag="qT8")
            kT8 = P.tile([128, 2 * S], f8, tag="kT8")
            v = P.tile([128, ST * HL * 65], bf16, tag="v")
            ctx = P.tile([128, ST * DL], bf16, tag="ctx")
            ctxT = P.tile([128, 2 * S], bf16, tag="ctxT")
            bqk = P.tile([128, 4], fp32, tag="bqk")
            bv8p = P.tile([1, 2 * DL], f8, tag="bv8p")
            ones82 = P.tile([128, 2, 64], f8, tag="ones82")
            ones8r = P.tile([1, 2 * 128], f8, tag="ones8r")
            z8row = P.tile([1, 2 * 260], f8, tag="z8row")
            wes = P.tile([128, DT * 2 * HL], fp32, tag="wes")
            bes = P.tile([HL, 2], fp32, tag="bes")
            gexp = P.tile([HL, DL], fp32, tag="gexp")
            ident = P.tile([128, 128], bf16, tag="ident")
            ones1 = P.tile([1, 1], fp32, tag="ones1")
            xmrow = P.tile([1, D], fp32, tag="xmrow")
            xm_col = P.tile([128, DT], fp32, tag="xm_col")
            gcol = P.tile([128, 2], fp32, tag="gcol")

            dma = nc.sync.dma_start

            def vview(t):
                return v[:].rearrange("p (t h c) -> p t h c", h=HL, c=65)[:, t]

            nc.vector.memset(ones82[:], 1.0)
            nc.vector.memset(ones8r[:], 1.0)
            nc.vector.memset(z8row[:], 0.0)
            nc.vector.memset(ones1[:], 1.0)
            # constant softmax-denominator columns of v
            nc.vector.memset(
                v[:].rearrange("p (t h c) -> p t h c", h=HL, c=65)[:, :, :, 64:65], 1.0
            )

            xt8v = xt8[:].rearrange("p (j s) -> p j s", s=S)      # [64, 16, S]
            wq8v = wq8[:].rearrange("p (j m) -> p j m", m=DL)     # [64, 16, DL]
            wk8v = wk8[:].rearrange("p (j m) -> p j m", m=DL)
            wv8v = wv8[:].rearrange("p (j m) -> p j m", m=DL)

            with (
                tc.tile_pool(name="x8l", bufs=8) as XL,
                tc.tile_pool(name="pm", bufs=3, space="PSUM") as PM,
                tc.tile_pool(name="pcps", bufs=2, space="PSUM") as PCS,
                tc.tile_pool(name="ets", bufs=26) as EX,
                tc.tile_pool(name="rc", bufs=8) as RC,
                tc.tile_pool(name="ob", bufs=2) as OB,
                tc.tile_pool(name="gs", bufs=1) as GS,
            ):
                xmp = [None, None]

                xbs = []

                def load_x_pair(pr):
                    """DMA natural-x pair pr via SWDGE (Pool) off the HWDGE path."""
                    xb = XL.tile([128, 2, D], f8, tag="xb", name=f"xb{pr}")
                    # SP HWDGE: queues naturally behind the critical xt8/w
                    # loads instead of jumping ahead from the idle Pool queue
                    nc.sync.dma_start(
                        out=xb[:],
                        in_=x8_d[:].rearrange("p (r c d) -> p r c d", c=2, d=D)[:, pr])
                    xbs.append(xb)

                def meanpool_mm(pr, xmps):
                    lhs = ones82[:]  # [128, 2, 64]: dual-fp8 ldweights needs wide M
                    for qh in range(4):
                        half, qq = qh // 2, qh % 2
                        first = (pr == 0 and qq == 0)
                        nc.tensor.matmul(
                            xmps[half][:, qq * 256:(qq + 1) * 256],
                            lhsT=lhs,
                            rhs=xbs[pr][:, :, qh * 256:(qh + 1) * 256],
                            start=first, stop=first,
                            skip_group_check=not first,
                            perf_mode=DRm,
                        )

                def qk_unit(w8v, dst8, bcol, i, sc, eng):
                    """Project one [128, SC] chunk of qT or kT (fp8 out)."""
                    pp = PM.tile([128, SC], fp32, tag="pm",
                                 name=f"pp{dst8.tensor.name}_{i}_{sc}")
                    for qq in range(2):
                        # x / weights live on 64 partitions x 16 d-tiles:
                        # dual-fp8 DR caps 2*K*M at the PE array size and the
                        # PE hangs if ldweights base-partition changes inside
                        # an accumulation chain, so every chain stays K=64.
                        for dp in range(DT):
                            first = (qq == 0 and dp == 0)
                            nc.tensor.matmul(
                                pp[:, qq * 256:(qq + 1) * 256],
                                lhsT=w8v[:, 2 * dp:2 * dp + 2, i * 128:(i + 1) * 128],
                                rhs=xt8v[:, 2 * dp:2 * dp + 2,
                                         sc * SC + qq * 256: sc * SC + (qq + 1) * 256],
                                start=first, stop=first,
                                skip_group_check=not first,
                                perf_mode=DRm,
                            )
                    dst = dst8[:, i * S + sc * SC: i * S + (sc + 1) * SC]
                    if eng == "A":
                        nc.scalar.activation(dst, pp[:], AF.Identity, bias=bcol)
                    else:
                        nc.vector.tensor_scalar(
                            out=dst, in0=pp[:], scalar1=bcol, scalar2=None,
                            op0=ALU.add,
                        )

                def v_unit(g, eng):
                    """Project v for s-tiles 2g, 2g+1 (one psum bank)."""
                    pv = PM.tile([128, 2 * DL], fp32, tag="pm", name=f"pv{g}")
                    for t2 in range(2):
                        t = 2 * g + t2
                        sl = pv[:, t2 * DL:(t2 + 1) * DL]
                        nc.tensor.matmul(
                            sl, lhsT=ones8r[:].rearrange("o (c m) -> o c m", c=2),
                            rhs=bv8p[:].rearrange("o (c m) -> o c m", c=2),
                            start=(t2 == 0), stop=(t2 == 0),
                            skip_group_check=(t2 == 1), perf_mode=DRm,
                        )
                        for dp in range(DT):
                            nc.tensor.matmul(
                                sl,
                                lhsT=xt8v[:, 2 * dp:2 * dp + 2, t * 128:(t + 1) * 128],
                                rhs=wv8v[:, 2 * dp:2 * dp + 2, :],
                                start=False, stop=False,
                                skip_group_check=True,
                                perf_mode=DRm,
                            )
                    for t2 in range(2):
                        t = 2 * g + t2
                        src = pv[:, t2 * DL:(t2 + 1) * DL].rearrange(
                            "p (h c) -> p h c", c=HD)
                        dst = vview(t)[:, :, 0:HD]
                        if eng == "A":
                            nc.scalar.copy(dst, src)
                        else:
                            nc.vector.tensor_copy(dst, src)

                def score_exp(i, sc, hh, skp, eng):
                    """Scores for sk-tiles (2skp, 2skp+1) x [sc*SC, (sc+1)*SC) of
                    head 2i+hh; one wide exp. psum holds 2*q.k (stride-0 DR)."""
                    r = hh * 64
                    sp = PM.tile([128, 2 * SC], fp32, tag="pm",
                                 name=f"sp{i}_{sc}_{hh}_{skp}")
                    for half in range(2):
                        sk = 2 * skp + half
                        lhsT = kT8[r:r + 64, i * S + sk * 128: i * S + (sk + 1) * 128] \
                            .unsqueeze(1).broadcast_to([64, 2, 128])
                        for qq in range(2):
                            rhs = qT8[r:r + 64,
                                      i * S + sc * SC + qq * 256: i * S + sc * SC + (qq + 1) * 256] \
                                .unsqueeze(1).broadcast_to([64, 2, 256])
                            # qq0 opens the bank's zero region; qq1 assigns
                            # into still-pending bytes (no second group)
                            nc.tensor.matmul(
                                sp[:, half * SC + qq * 256: half * SC + (qq + 1) * 256],
                                lhsT=lhsT, rhs=rhs, start=(qq == 0), stop=(qq == 0),
                                skip_group_check=(qq == 1),
                                perf_mode=DRm,
                            )
                    if eng == "A":
                        et = EX.tile([128, 2 * SC], bf16, tag="et",
                                     name=f"et{i}_{sc}_{hh}_{skp}")
                        nc.scalar.activation(et[:], sp[:], AF.Exp, scale=1.0 / 16.0)
                        return et[:]
                    et = EX.tile([128, 2 * SC], i16, tag="et",
                                 name=f"et{i}_{sc}_{hh}_{skp}")
                    nc.vector.tensor_scalar(
                        out=et[:], in0=sp[:], scalar1=A16, scalar2=B16,
                        op0=ALU.mult, op1=ALU.add,
                    )
                    return et[:].bitcast(bf16)

                def pv_mm(cps, ets, i, hh, sk):
                    h = 2 * i + hh
                    skp, half = sk // 2, sk % 2
                    for u in range(SC // 128):
                        nc.tensor.matmul(
                            cps[u][:, hh * 65:(hh + 1) * 65],
                            lhsT=ets[skp][:, half * SC + u * 128: half * SC + (u + 1) * 128],
                            rhs=v[:, sk * HL * 65 + h * 65: sk * HL * 65 + (h + 1) * 65],
                            start=False, stop=False, skip_group_check=True,
                        )

                def outproj_units(t):
                    def unit():
                        ot = OB.tile([128, D], bf16, tag="ot", name=f"ot{t}")
                        po = PM.tile([128, D], fp32, tag="pm", name=f"po{t}")
                        for n2 in range(2):
                            for i2 in range(2):
                                nc.tensor.matmul(
                                    po[:, n2 * 512:(n2 + 1) * 512],
                                    lhsT=ctxT[:, i2 * S + t * 128: i2 * S + (t + 1) * 128],
                                    rhs=wo[:, i2 * D + n2 * 512: i2 * D + (n2 + 1) * 512],
                                    start=(i2 == 0), stop=(i2 == 1),
                                )
                        if OUT_DRAIN[t % len(OUT_DRAIN)] == "A":
                            nc.scalar.copy(ot[:], po[:])
                        else:
                            nc.vector.tensor_copy(ot[:], po[:])
                        (nc.scalar if KB_NOSWDGE else nc.gpsimd).dma_start(
                            out=out_d[t * 128:(t + 1) * 128, :], in_=ot[:])
                    return [(1000, unit)]

                fillers = []

                def pop_fillers(budget_ns):
                    spent = 0
                    while fillers and spent < budget_ns:
                        ns, unit = fillers.pop(0)
                        unit()
                        spent += ns

                # ---------------- lead-in ----------------
                # xt8 chunk 0 + wk8 first: they gate the whole pipeline
                def dma_xt(sc):
                    dma(xt8v[:, :, sc * SC:(sc + 1) * SC],
                        xt8_d[:].rearrange("p (j s) -> p j s", s=S)[:, :, sc * SC:(sc + 1) * SC])
                dma_xt(0)
                dma(wk8[:], wk_d[:])
                dma(bqk[:], bqk_d[:])
                dma(wq8[:], wq_d[:])
                dma_xt(1)
                dma(ident[:], id_d[:])
                dma_xt(2)
                dma_xt(3)
                dma(wv8[:], wv_d[:])
                dma(bv8p[:], bv8_d[:])
                dma(wes[:], wes_d[:])
                dma(bes[:], bes_d[:])
                dma(gexp[:], gexp_d[:])
                dma(wo[:], wo_d[:])
                # keep the pair transfers behind xt8-sc0/wk8/wq8 in the
                # shared DMA queue: they are not needed until the mean-pool
                with tc.tile_wait_until(0.0035):
                    for pr in range(ST // 2):
                        load_x_pair(pr)

                # minimal critical path: k i=0 sc=0 + q i=0 sc=0 lets chunk-0
                # scores start; the other k i=0 chunks interleave with the
                # first a-stretch (k-sc j emitted just before skp 2j).
                if KB_LEADIN >= 1:
                    qk_unit(wk8v, kT8, bqk[:, 2:3], 0, 0, QK_DRAIN[0])
                if KB_LEADIN >= 2:
                    qk_unit(wq8v, qT8, bqk[:, 0:1], 0, 0, QK_DRAIN[4])

                # deferred to fillers: all of i=1 (needed from chunk 4)
                for sc in range(NSC):
                    fillers.append((1100, (lambda s: lambda: qk_unit(
                        wk8v, kT8, bqk[:, 3:4], 1, s, QK_DRAIN[(8 + s) % 16]))(sc)))
                for sc in range(NSC):
                    fillers.append((1100, (lambda s: lambda: qk_unit(
                        wq8v, qT8, bqk[:, 1:2], 1, s, QK_DRAIN[(12 + s) % 16]))(sc)))

                def gates_block():
                    xcp = PCS.tile([128, DT], fp32, tag="cp", name="xcp")
                    for hf in range(2):
                        nc.vector.tensor_copy(
                            xmrow[:, hf * 512:(hf + 1) * 512], xmp[hf][0:1, :])
                    for j in range(DT):
                        nc.tensor.matmul(
                            xcp[:, j: j + 1],
                            lhsT=xmrow[:, j * 128:(j + 1) * 128],
                            rhs=ones1[:],
                            start=True, stop=True,
                        )
                    nc.vector.tensor_copy(xm_col[:], xcp[:])
                    gpe = PCS.tile([HL, 1], fp32, tag="cp", name="gpe")
                    gps = PCS.tile([HL, 1], fp32, tag="cp", name="gps")
                    for j in range(DT):
                        nc.tensor.matmul(
                            gpe[:], lhsT=wes[:, j * 8: j * 8 + 4],
                            rhs=xm_col[:, j: j + 1],
                            start=(j == 0), stop=(j == DT - 1),
                        )
                    for j in range(DT):
                        nc.tensor.matmul(
                            gps[:], lhsT=wes[:, j * 8 + 4: j * 8 + 8],
                            rhs=xm_col[:, j: j + 1],
                            start=(j == 0), stop=(j == DT - 1),
                        )
                    eth = GS.tile([HL, 1], fp32, tag="eth")
                    saf = GS.tile([HL, 1], fp32, tag="saf")
                    gate = GS.tile([HL, 1], fp32, tag="gate")
                    nc.scalar.activation(eth[:], gpe[:], AF.Sigmoid,
                                         bias=bes[:, 0:1], scale=1.0 / S)
                    nc.scalar.activation(saf[:], gps[:], AF.Sigmoid,
                                         bias=bes[:, 1:2], scale=1.0 / S)
                    nc.vector.tensor_mul(gate[:], eth[:], saf[:])
                    for i in range(2):
                        pgc = PCS.tile([128, 1], fp32, tag="cp", name=f"pgc{i}")
                        nc.tensor.matmul(
                            pgc[:], lhsT=gexp[:, i * 128:(i + 1) * 128], rhs=gate[:],
                            start=True, stop=True,
                        )
                        nc.vector.tensor_copy(gcol[:, i: i + 1], pgc[:])

                # ---------------- chunk loop ----------------
                def alloc_cps(i, sc):
                    # two 1-bank tiles, each holding two u-slots of [128, 130];
                    # a zero rank-1 matmul opens each bank's zero region so the
                    # interleaved PV accumulations need no group bookkeeping
                    pair = [PCS.tile([128, 2, 130], fp32, tag="cp",
                                     name=f"cp{i}_{sc}_{w}") for w in range(2)]
                    for w in range(2):
                        for a2 in range(2):
                            nc.tensor.matmul(
                                pair[w][:, a2, :],
                                lhsT=ones8r[:].rearrange("o (c m) -> o c m", c=2),
                                rhs=z8row[:].rearrange("o (c m) -> o c m", c=2)[:, :, 0:130],
                                start=True, stop=True,
                                skip_group_check=(a2 == 1),
                                perf_mode=DRm,
                            )
                    return [pair[u // 2][:, u % 2, :] for u in range(SC // 128)]

                def pv_mm_u(cps, ets, i, hh, u):
                    h = 2 * i + hh
                    for sk in range(ST):
                        skp, half = sk // 2, sk % 2
                        nc.tensor.matmul(
                            cps[u][:, hh * 65:(hh + 1) * 65],
                            lhsT=ets[skp][:, half * SC + u * 128: half * SC + (u + 1) * 128],
                            rhs=v[:, sk * HL * 65 + h * 65: sk * HL * 65 + (h + 1) * 65],
                            start=(sk == 0), stop=(sk == ST - 1),
                        )

                def make_tail_parts(i, sc, cps, last):
                    """Staggered per-u closures: partN(u) = DVE normalize only;
                    partT(u) = PE transpose (+ drain at odd u), emitted one
                    slot later so the transpose never queues on PE before its
                    normalize has finished on DVE. PV ran in the b-stretch."""
                    state = {"tp": None}

                    def partN(u):
                        def f():
                            t = sc * (SC // 128) + u
                            rec2 = RC.tile([128, 2], fp32, tag="rec",
                                           name=f"rec{i}_{t}")
                            cpv = cps[u].rearrange("p (h c) -> p h c", c=65)
                            nc.vector.reciprocal(rec2[:], cpv[:, :, 64])
                            with nc.allow_low_precision("softmax-normalized bf16 ctx"):
                                nc.vector.tensor_tensor(
                                    out=ctx[:, t * DL + i * 128: t * DL + (i + 1) * 128]
                                        .rearrange("p (h c) -> p h c", c=HD),
                                    in0=cpv[:, :, 0:HD],
                                    in1=rec2[:].unsqueeze(2).broadcast_to([128, 2, HD]),
                                    op=ALU.mult,
                                )
                        return f

                    def partT(u):
                        def f():
                            if u % 2 == 0:
                                # lives in the cps pool's idle window during the
                                # a-stretch, keeping all 3 PM slots for scores
                                state["tp"] = PCS.tile([128, 256], bf16, tag="cp",
                                                       name=f"tp{i}_{sc}_{u // 2}")
                            tp = state["tp"]
                            t = sc * (SC // 128) + u
                            nc.tensor.transpose(
                                tp[:, (u % 2) * 128:(u % 2 + 1) * 128],
                                ctx[:, t * DL + i * 128: t * DL + (i + 1) * 128],
                                ident[:],
                            )
                            if last:
                                # per-u drain so the final out-projs pipeline
                                nc.vector.tensor_scalar(
                                    out=ctxT[:, i * S + t * 128: i * S + (t + 1) * 128],
                                    in0=tp[:, (u % 2) * 128:(u % 2 + 1) * 128],
                                    scalar1=gcol[:, i: i + 1],
                                    scalar2=None,
                                    op0=ALU.mult,
                                )
                                for _, unit in outproj_units(t):
                                    unit()
                            elif u % 2 == 1:
                                nc.vector.tensor_scalar(
                                    out=ctxT[:, i * S + (t - 1) * 128: i * S + (t + 1) * 128],
                                    in0=tp[:],
                                    scalar1=gcol[:, i: i + 1],
                                    scalar2=None,
                                    op0=ALU.mult,
                                )
                                if u == SC // 128 - 1 and i == 1:
                                    for t2 in range(sc * 4, sc * 4 + 4):
                                        fillers.extend(outproj_units(t2))
                        return f

                    parts = []
                    nn = [partN(u) for u in range(SC // 128)]
                    tt = [partT(u) for u in range(SC // 128)]
                    parts.append(nn[0])
                    for u in range(1, SC // 128):
                        parts.append(lambda a=nn[u], b=tt[u - 1]: (a(), b()))
                    parts.append(tt[SC // 128 - 1])
                    return parts

                nchunks = [(i, sc) for i in range(2) for sc in range(NSC)][:KB_CHUNKS]
                pending_parts = []
                for n, (i, sc) in enumerate(nchunks):
                    first = (n == 0)
                    if first:
                        # mean-pool runs here: PE is otherwise idle during the
                        # first a-stretch and the PC psum pool is free.
                        xmp[0] = PCS.tile([64, 512], fp32, tag="cp", name="xmp0")
                        xmp[1] = PCS.tile([64, 512], fp32, tag="cp", name="xmp1")
                    ets_a = []
                    for skp in range(ST // 2):
                        if first and skp in (2, 4, 6):
                            # k i=0 chunk j just ahead of the scores needing it
                            qk_unit(wk8v, kT8, bqk[:, 2:3], 0, skp // 2,
                                    QK_DRAIN[skp // 2])
                        ets_a.append(score_exp(i, sc, 0, skp,
                                               EXP_PAT[n % len(EXP_PAT)][skp]))
                        if skp >= 1 and pending_parts:
                            pending_parts.pop(0)()
                        elif skp >= 2 and not first:
                            pop_fillers(FILLER_NS)
                    while pending_parts:
                        pending_parts.pop(0)()
                    if first:
                        # keep these off the critical lead-in: the scheduler
                        # would otherwise hoist them ahead of the k/q units
                        # and head-block PE on the slow x8-pair DMAs.
                        with tc.tile_wait_until(0.012):
                            for pr in range(ST // 2):
                                meanpool_mm(pr, xmp)
                        with tc.tile_wait_until(0.014):
                            gates_block()
                    cps = alloc_cps(i, sc)
                    ets_b = []
                    for skp in range(ST // 2):
                        ets_b.append(score_exp(i, sc, 1, skp,
                                               EXP_PAT[n % len(EXP_PAT)][8 + skp]))
                        if first:
                            v_unit(skp, V_DRAIN[skp % 8])
                        # PV for both heads streams through the b-stretch.
                        # Head b lags one skp so PE never queues behind the
                        # exp that was just issued for this skp.
                        pv_mm(cps, ets_a, i, 0, 2 * skp)
                        pv_mm(cps, ets_a, i, 0, 2 * skp + 1)
                        if skp >= 1:
                            pv_mm(cps, ets_b, i, 1, 2 * (skp - 1))
                            pv_mm(cps, ets_b, i, 1, 2 * (skp - 1) + 1)
                        if skp == 5 and i == 0 and sc < NSC - 1:
                            # q i=0 chunk sc+1 mid-b-stretch, off the boundary
                            qk_unit(wq8v, qT8, bqk[:, 0:1], 0, sc + 1,
                                    QK_DRAIN[4 + sc + 1])
                        if not first:
                            pop_fillers(FILLER_NS)
                    pending_parts = make_tail_parts(
                        i, sc, cps, last=(n == len(nchunks) - 1))
                    # the last head-b PV pair rides into the next a-stretch so
                    # the chunk boundary never waits on the final exp
                    def last_pv(cps=cps, ets_b=ets_b, i=i):
                        pv_mm(cps, ets_b, i, 1, ST - 2)
                        pv_mm(cps, ets_b, i, 1, ST - 1)
                    pending_parts.insert(0, last_pv)
                if KB_CHUNKS == 8:
                    for p in pending_parts:
                        p()
                    pop_fillers(10**9)
                else:
                    pending_parts.clear()
                    fillers.clear()
                    # touch out so the output DMA graph exists
                    ot = OB.tile([128, D], bf16, tag="ot", name="ot_stub")
                    nc.vector.memset(ot[:], 0.0)
                    nc.sync.dma_start(out=out_d[0:128, :], in_=ot[:])

    _split_multi_waits(nc)
    return nc


def _split_multi_waits(nc, skip=("InstEventSemaphore",)):
    """Hoist extra sync waits onto preceding same-engine NoOps.

    Walrus codegen can attach only one sync wait to some instruction
    encodings, so any instruction carrying N>1 waits is rewritten as N-1
    single-wait NoOps followed by the instruction with the last wait.
    """
    import concourse.mybir as mybir

    eng = {
        "EngineType.PE": nc.tensor,
        "EngineType.DVE": nc.vector,
        "EngineType.Activation": nc.scalar,
        "EngineType.Pool": nc.gpsimd,
        "EngineType.SP": nc.sync,
    }

    def fresh_nop(engine_key):
        nop = eng[engine_key].nop(hint="wsplit").ins
        for fn in nc.m.functions:
            for bb in fn.blocks:
                for i, ins in enumerate(bb.instructions):
                    if ins.name == nop.name:
                        del bb.instructions[i]
                        return nop
        raise RuntimeError("fresh nop not found")

    for fn in nc.m.functions:
        for bb in fn.blocks:
            insertions = []
            for idx, ins in enumerate(bb.instructions):
                if type(ins).__name__ in skip:
                    continue
                si = ins.sync_info
                if si is None or len(si.on_wait) <= 1:
                    continue
                waits = list(si.on_wait)
                nops = []
                for w in waits[:-1]:
                    nop = fresh_nop(str(ins.engine))
                    nop.sync_info = mybir.SyncInfo(on_wait=[w], on_update=[])
                    nops.append(nop)
                ins.sync_info = mybir.SyncInfo(
                    on_wait=[waits[-1]], on_update=list(si.on_update)
                )
                insertions.append((idx, nops))
            for idx, nops in reversed(insertions):
                bb.instructions[idx:idx] = nops


def _in_maps(inputs):
    x = np.ascontiguousarray(inputs["x"], np.float32)
    maps = []
    ident = np.eye(128, dtype=np.float32)
    gexp = np.zeros((HL, DL), np.float32)
    for h in range(HL):
        gexp[h, h * HD:(h + 1) * HD] = 1.0
    x8 = x.astype(F8)          # [B, S, D]
    for c in range(NCORES):
        b, g = c // HG, c % HG
        sl = slice(g * DL, (g + 1) * DL)
        hsl = slice(g * HL, (g + 1) * HL)
        # xT fp8 on 64 partitions: [64, jj, s] = x[b][s, jj*64+p]
        xt8 = np.ascontiguousarray(
            x8[b].T.reshape(2 * DT, 64, S).transpose(1, 0, 2).reshape(64, 2 * DT * S))
        # natural-x pairs: [128, pr, c2, col] = x[b][pr*256 + c2*128 + p, col]
        x8n = np.ascontiguousarray(
            x8[b].reshape(ST // 2, 2, 128, D).transpose(2, 0, 1, 3).reshape(128, ST * D))
        def wtile(w):
            return np.ascontiguousarray(
                w[:, sl].reshape(2 * DT, 64, DL).transpose(1, 0, 2).reshape(64, 2 * DT * DL).astype(F8))
        wo = inputs["Wo"][sl, :].reshape(2, 128, D).transpose(1, 0, 2).reshape(128, 2 * D)
        bqk = np.stack([
            inputs["bq"][sl][0:128], inputs["bq"][sl][128:256],
            inputs["bk"][sl][0:128], inputs["bk"][sl][128:256],
        ], axis=1).astype(np.float32)
        bv8p = np.zeros((1, 2 * DL), F8)
        bv8p[0, 0:DL] = inputs["bv"][sl].astype(F8)
        wes = np.concatenate([inputs["We"][:, hsl], inputs["Ws"][:, hsl]], axis=1)
        wes = wes.reshape(DT, 128, 2 * HL).transpose(1, 0, 2).reshape(128, DT * 2 * HL)
        bes = np.stack([inputs["be"][hsl], inputs["bs"][hsl]], axis=1)
        maps.append({
            "xt8": xt8,
            "x8": x8n,
            "wq8": wtile(inputs["Wq"]),
            "wk8": wtile(inputs["Wk"]),
            "wv8": wtile(inputs["Wv"]),
            "wo": np.ascontiguousarray(wo.astype(BF16)),
            "bqk": np.ascontiguousarray(bqk),
            "bv8p": bv8p,
            "wes": np.ascontiguousarray(wes.astype(np.float32)),
            "bes": np.ascontiguousarray(bes.astype(np.float32)),
            "gexp": gexp,
            "ident": ident.astype(BF16),
        })
    return maps


def kernel(**inputs):
    if "nc" not in _CACHE:
        _CACHE["nc"] = _build_nc()
    nc = _CACHE["nc"]
    maps = _in_maps({k: np.asarray(v) for k, v in inputs.items()})

    from concourse.bass_utils import run_bass_kernel_spmd

    trace = bool(int(os.environ.get("KERNEL_TRACE", "0")))
    res = run_bass_kernel_spmd(
        nc, maps, list(range(NCORES)), trace=trace,
        tmpdir=os.environ.get("KERNEL_TRACE_DIR") if trace else None,
    )
    _CACHE["last_result"] = res
    bo = np.asarray(inputs["bo"], np.float32)
    out = np.zeros((B, S, D), np.float32)
    for b in range(B):
        acc = np.zeros((S, D), np.float32)
        for g in range(HG):
            acc += res.results[b * HG + g]["out"].astype(np.float32)
        out[b] = acc + bo
    return out


# revision 45
# speedup vs baseline: 1.0555x; 1.0167x over previous
"""Trainium2 Bass kernel for nn_ASISNativeAttention (B=2,S=2048,D=1024,H=16).

Sharding: 8 cores = 2 batches x 4 head-groups (4 heads each); host splits
inputs per core and sums the 4 partial output projections per batch (+bo).

v2 design (vs the all-bf16 v1 at ~208us modeled): the two elementwise
engines (ACT, DVE) are the roofline -- 131K partition-lines of exp per core
-- so the exp work is split across BOTH of them, and the big matmuls move
to fp8 DoubleRow mode (2 k-tiles per instruction at half the per-row cost)
to keep PE under that roof:

  PE    q/k/v projections in fp8 DoubleRow from a host-transposed fp8 xT
        laid out on 64 partitions x 16 d-tiles (the PE hangs if a dual-fp8
        ldweights base partition changes inside one accumulation chain, and
        2*K*M may not exceed the 128x128 array, so chains stay K=64);
        scores per head via DoubleRow with a stride-0 replicated k-tile
        pair (computes 2*q.k, folded into the exp scale 1/16); PV and
        out-proj in bf16; mean-pool via fp8 DoubleRow ones-matmuls.
  ACT   exact exp (scale=1/16, psum->bf16) for ~60% of score tiles; its
        share of the qk/v/out-proj drains; the two gate sigmoids.
  DVE   Schraudolph exp for the rest: one tensor_scalar
        (psum*A16+B16 -> int16, round-to-nearest) whose int16 bits ARE the
        bf16 probs (max rel err ~3.5%, invisible after softmax: the PV
        ones-column sums the same stored values for the denominator);
        ctx normalize (strided reciprocal + stride-0-broadcast
        tensor_tensor); gated ctxT drain in 2x mode; remaining drains.
  Pool  x8-pair and output DMA via SWDGE, keeping HWDGE/SP free.

PSUM (8 banks): scores double-buffer 2x2 banks in a 3-slot pool shared
with transient out-proj/transpose/projection tiles; PV accumulators 2
banks (zero-opened by a rank-1 matmul so the interleaved per-head
accumulations need no group bookkeeping); per-2KB-bank accumulation
groups follow the zero-region rules (one pending group per region).

Measured (test.py): modeled exec 163.5us (v1 baseline: 208.0us); hardware
rel err vs reference 1.33e-2 (gate 2e-2; fp8 exposure is limited to
x / Wq,Wk,Wv / stored qT,kT -- probs, v, ctx, Wo stay bf16).

kernel.py is self-contained: numpy/ml_dtypes + the concourse tree at
/opt/trn_rl_repo.
"""

import os
import sys
import numpy as np
import ml_dtypes

BF16 = ml_dtypes.bfloat16
F8 = ml_dtypes.float8_e4m3

sys.path.insert(0, "/opt/trn_rl_repo")

B, S, D, H = 2, 2048, 1024, 16
HD = 64          # head dim
NCORES = 8
HG = 4           # head groups = cores per batch
HL = H // HG     # heads per core (4)
DL = D // HG     # local width (256)
ST = S // 128    # 16 s-tiles
DT = D // 128    # 8 d-tiles
SC = 512         # sq chunk width for scores
NSC = S // SC    # 4 chunks

# Schraudolph exp constants: int16 = rint(psum * A16 + B16); psum holds
# 2*q.k so the effective exp argument is psum/16 = q.k/8.
_C_SCH = 360000.0
A16 = float(2.0**7 / np.log(2.0) / 16.0)
B16 = float(127 * 2**7 - _C_SCH / 65536.0)

# --- engine-assignment knobs (tuned against TimelineSim) ---
# exp engine per chunk (16 chars: a-stretch skp 0-7 then b-stretch skp 0-7):
# 'A' = ACT exact exp, 'D' = DVE Schraudolph
EXP_PAT = [
    "ADAADAAD" "AADAADAD",   # 10A chunks (early D: DVE lane starts sooner)
    "AADAADAD" "ADAADADD",   # 9A chunks
] * 3 + ["AADAADAD" "AADAADAD", "AADAADAD" "ADAADDDA"]
if os.environ.get("KB_ALLACT"):
    EXP_PAT = ["A" * 16] * 8
KB_NOSWDGE = bool(os.environ.get("KB_NOSWDGE"))
KB_CHUNKS = int(os.environ.get("KB_CHUNKS", "8"))
KB_LEADIN = int(os.environ.get("KB_LEADIN", "99"))
# qk projection drain engines, one char per unit (k-i0 x4, q-i0 x4, then 8 i1)
QK_DRAIN = "ADADADADADADADAD"
# v drain engines, one per psum group (8)
V_DRAIN = "ADADADAD"
# out-proj drain engines, one per po half (32)
OUT_DRAIN = "AD" * 16
FILLER_NS = 1500   # filler budget popped per b-stretch skp

_CACHE = {}


def _build_nc():
    import concourse.bass as bass
    import concourse.mybir as mybir
    from concourse.tile import TileContext

    fp32 = mybir.dt.float32
    bf16 = mybir.dt.bfloat16
    f8 = mybir.dt.float8e4
    i16 = mybir.dt.int16
    AF = mybir.ActivationFunctionType
    ALU = mybir.AluOpType
    DRm = mybir.MatmulPerfMode.DoubleRow

    nc = bass.Bass()

    xt8_d = nc.declare_dram_parameter("xt8", [64, 2 * DT * S], f8, isOutput=False)
    x8_d = nc.declare_dram_parameter("x8", [128, ST * D], f8, isOutput=False)
    wq_d = nc.declare_dram_parameter("wq8", [64, 2 * DT * DL], f8, isOutput=False)
    wk_d = nc.declare_dram_parameter("wk8", [64, 2 * DT * DL], f8, isOutput=False)
    wv_d = nc.declare_dram_parameter("wv8", [64, 2 * DT * DL], f8, isOutput=False)
    wo_d = nc.declare_dram_parameter("wo", [128, 2 * D], bf16, isOutput=False)
    bqk_d = nc.declare_dram_parameter("bqk", [128, 4], fp32, isOutput=False)
    bv8_d = nc.declare_dram_parameter("bv8p", [1, 2 * DL], f8, isOutput=False)
    wes_d = nc.declare_dram_parameter("wes", [128, DT * 2 * HL], fp32, isOutput=False)
    bes_d = nc.declare_dram_parameter("bes", [HL, 2], fp32, isOutput=False)
    gexp_d = nc.declare_dram_parameter("gexp", [HL, DL], fp32, isOutput=False)
    id_d = nc.declare_dram_parameter("ident", [128, 128], bf16, isOutput=False)
    out_d = nc.declare_dram_parameter("out", [S, D], bf16, isOutput=True)

    with TileContext(nc) as tc:
        with tc.tile_pool(name="persist", bufs=1) as P:
            xt8 = P.tile([64, 2 * DT * S], f8, tag="xt8")
            wq8 = P.tile([64, 2 * DT * DL], f8, tag="wq8")
            wk8 = P.tile([64, 2 * DT * DL], f8, tag="wk8")
            wv8 = P.tile([64, 2 * DT * DL], f8, tag="wv8")
            wo = P.tile([128, 2 * D], bf16, tag="wo")
            qT8 = P.tile([128, 2 * S], f8, tag="qT8")
            kT8 = P.tile([128, 2 * S], f8, tag="kT8")
            v = P.tile([128, ST * HL * 65], bf16, tag="v")
            ctx = P.tile([128, ST * DL], bf16, tag="ctx")
            ctxT = P.tile([128, 2 * S], bf16, tag="ctxT")
            bqk = P.tile([128, 4], fp32, tag="bqk")
            bv8p = P.tile([1, 2 * DL], f8, tag="bv8p")
            ones82 = P.tile([128, 2, 64], f8, tag="ones82")
            ones8r = P.tile([1, 2 * 128], f8, tag="ones8r")
            z8row = P.tile([1, 2 * 260], f8, tag="z8row")
            wes = P.tile([128, DT * 2 * HL], fp32, tag="wes")
            bes = P.tile([HL, 2], fp32, tag="bes")
            gexp = P.tile([HL, DL], fp32, tag="gexp")
            ident = P.tile([128, 128], bf16, tag="ident")
            ones1 = P.tile([1, 1], fp32, tag="ones1")
            xmrow = P.tile([1, D], fp32, tag="xmrow")
            xm_col = P.tile([128, DT], fp32, tag="xm_col")
            gcol = P.tile([128, 2], fp32, tag="gcol")

            dma = nc.sync.dma_start

            def vview(t):
                return v[:].rearrange("p (t h c) -> p t h c", h=HL, c=65)[:, t]

            nc.vector.memset(ones82[:], 1.0)
            nc.vector.memset(ones8r[:], 1.0)
            nc.vector.memset(z8row[:], 0.0)
            nc.vector.memset(ones1[:], 1.0)
            # constant softmax-denominator columns of v
            nc.vector.memset(
                v[:].rearrange("p (t h c) -> p t h c", h=HL, c=65)[:, :, :, 64:65], 1.0
            )

            xt8v = xt8[:].rearrange("p (j s) -> p j s", s=S)      # [64, 16, S]
            wq8v = wq8[:].rearrange("p (j m) -> p j m", m=DL)     # [64, 16, DL]
            wk8v = wk8[:].rearrange("p (j m) -> p j m", m=DL)
            wv8v = wv8[:].rearrange("p (j m) -> p j m", m=DL)

            with (
                tc.tile_pool(name="x8l", bufs=8) as XL,
                tc.tile_pool(name="pm", bufs=3, space="PSUM") as PM,
                tc.tile_pool(name="pcps", bufs=2, space="PSUM") as PCS,
                tc.tile_pool(name="ets", bufs=26) as EX,
                tc.tile_pool(name="rc", bufs=8) as RC,
                tc.tile_pool(name="ob", bufs=2) as OB,
                tc.tile_pool(name="gs", bufs=1) as GS,
            ):
                xmp = [None, None]

                xbs = []

                def load_x_pair(pr):
                    """DMA natural-x pair pr via SWDGE (Pool) off the HWDGE path."""
                    xb = XL.tile([128, 2, D], f8, tag="xb", name=f"xb{pr}")
                    # SP HWDGE: queues naturally behind the critical xt8/w
                    # loads instead of jumping ahead from the idle Pool queue
                    nc.sync.dma_start(
                        out=xb[:],
                        in_=x8_d[:].rearrange("p (r c d) -> p r c d", c=2, d=D)[:, pr])
                    xbs.append(xb)

                def meanpool_mm(pr, xmps):
                    lhs = ones82[:]  # [128, 2, 64]: dual-fp8 ldweights needs wide M
                    for qh in range(4):
                        half, qq = qh // 2, qh % 2
                        first = (pr == 0 and qq == 0)
                        nc.tensor.matmul(
                            xmps[half][:, qq * 256:(qq + 1) * 256],
                            lhsT=lhs,
                            rhs=xbs[pr][:, :, qh * 256:(qh + 1) * 256],
                            start=first, stop=first,
                            skip_group_check=not first,
                            perf_mode=DRm,
                        )

                def qk_unit(w8v, dst8, bcol, i, sc, eng):
                    """Project one [128, SC] chunk of qT or kT (fp8 out)."""
                    pp = PM.tile([128, SC], fp32, tag="pm",
                                 name=f"pp{dst8.tensor.name}_{i}_{sc}")
                    for qq in range(2):
                        # x / weights live on 64 partitions x 16 d-tiles:
                        # dual-fp8 DR caps 2*K*M at the PE array size and the
                        # PE hangs if ldweights base-partition changes inside
                        # an accumulation chain, so every chain stays K=64.
                        for dp in range(DT):
                            first = (qq == 0 and dp == 0)
                            nc.tensor.matmul(
                                pp[:, qq * 256:(qq + 1) * 256],
                                lhsT=w8v[:, 2 * dp:2 * dp + 2, i * 128:(i + 1) * 128],
                                rhs=xt8v[:, 2 * dp:2 * dp + 2,
                                         sc * SC + qq * 256: sc * SC + (qq + 1) * 256],
                                start=first, stop=first,
                                skip_group_check=not first,
                                perf_mode=DRm,
                            )
                    dst = dst8[:, i * S + sc * SC: i * S + (sc + 1) * SC]
                    if eng == "A":
                        nc.scalar.activation(dst, pp[:], AF.Identity, bias=bcol)
                    else:
                        nc.vector.tensor_scalar(
                            out=dst, in0=pp[:], scalar1=bcol, scalar2=None,
                            op0=ALU.add,
                        )

                def v_unit(g, eng):
                    """Project v for s-tiles 2g, 2g+1 (one psum bank)."""
                    pv = PM.tile([128, 2 * DL], fp32, tag="pm", name=f"pv{g}")
                    for t2 in range(2):
                        t = 2 * g + t2
                        sl = pv[:, t2 * DL:(t2 + 1) * DL]
                        nc.tensor.matmul(
                            sl, lhsT=ones8r[:].rearrange("o (c m) -> o c m", c=2),
                            rhs=bv8p[:].rearrange("o (c m) -> o c m", c=2),
                            start=(t2 == 0), stop=(t2 == 0),
                            skip_group_check=(t2 == 1), perf_mode=DRm,
                        )
                        for dp in range(DT):
                            nc.tensor.matmul(
                                sl,
                                lhsT=xt8v[:, 2 * dp:2 * dp + 2, t * 128:(t + 1) * 128],
                                rhs=wv8v[:, 2 * dp:2 * dp + 2, :],
                                start=False, stop=False,
                                skip_group_check=True,
                                perf_mode=DRm,
                            )
                    for t2 in range(2):
                        t = 2 * g + t2
                        src = pv[:, t2 * DL:(t2 + 1) * DL].rearrange(
                            "p (h c) -> p h c", c=HD)
                        dst = vview(t)[:, :, 0:HD]
                        if eng == "A":
                            nc.scalar.copy(dst, src)
                        else:
                            nc.vector.tensor_copy(dst, src)

                def score_exp(i, sc, hh, skp, eng):
                    """Scores for sk-tiles (2skp, 2skp+1) x [sc*SC, (sc+1)*SC) of
                    head 2i+hh; one wide exp. psum holds 2*q.k (stride-0 DR)."""
                    r = hh * 64
                    sp = PM.tile([128, 2 * SC], fp32, tag="pm",
                                 name=f"sp{i}_{sc}_{hh}_{skp}")
                    for half in range(2):
                        sk = 2 * skp + half
                        lhsT = kT8[r:r + 64, i * S + sk * 128: i * S + (sk + 1) * 128] \
                            .unsqueeze(1).broadcast_to([64, 2, 128])
                        for qq in range(2):
                            rhs = qT8[r:r + 64,
                                      i * S + sc * SC + qq * 256: i * S + sc * SC + (qq + 1) * 256] \
                                .unsqueeze(1).broadcast_to([64, 2, 256])
                            # qq0 opens the bank's zero region; qq1 assigns
                            # into still-pending bytes (no second group)
                            nc.tensor.matmul(
                                sp[:, half * SC + qq * 256: half * SC + (qq + 1) * 256],
                                lhsT=lhsT, rhs=rhs, start=(qq == 0), stop=(qq == 0),
                                skip_group_check=(qq == 1),
                                perf_mode=DRm,
                            )
                    if eng == "A":
                        et = EX.tile([128, 2 * SC], bf16, tag="et",
                                     name=f"et{i}_{sc}_{hh}_{skp}")
                        nc.scalar.activation(et[:], sp[:], AF.Exp, scale=1.0 / 16.0)
                        return et[:]
                    et = EX.tile([128, 2 * SC], i16, tag="et",
                                 name=f"et{i}_{sc}_{hh}_{skp}")
                    nc.vector.tensor_scalar(
                        out=et[:], in0=sp[:], scalar1=A16, scalar2=B16,
                        op0=ALU.mult, op1=ALU.add,
                    )
                    return et[:].bitcast(bf16)

                def pv_mm(cps, ets, i, hh, sk):
                    h = 2 * i + hh
                    skp, half = sk // 2, sk % 2
                    for u in range(SC // 128):
                        nc.tensor.matmul(
                            cps[u][:, hh * 65:(hh + 1) * 65],
                            lhsT=ets[skp][:, half * SC + u * 128: half * SC + (u + 1) * 128],
                            rhs=v[:, sk * HL * 65 + h * 65: sk * HL * 65 + (h + 1) * 65],
                            start=False, stop=False, skip_group_check=True,
                        )

                def outproj_units(t):
                    def unit():
                        ot = OB.tile([128, D], bf16, tag="ot", name=f"ot{t}")
                        po = PM.tile([128, D], fp32, tag="pm", name=f"po{t}")
                        for n2 in range(2):
                            for i2 in range(2):
                                nc.tensor.matmul(
                                    po[:, n2 * 512:(n2 + 1) * 512],
                                    lhsT=ctxT[:, i2 * S + t * 128: i2 * S + (t + 1) * 128],
                                    rhs=wo[:, i2 * D + n2 * 512: i2 * D + (n2 + 1) * 512],
                                    start=(i2 == 0), stop=(i2 == 1),
                                )
                        if OUT_DRAIN[t % len(OUT_DRAIN)] == "A":
                            nc.scalar.copy(ot[:], po[:])
                        else:
                            nc.vector.tensor_copy(ot[:], po[:])
                        (nc.scalar if KB_NOSWDGE else nc.gpsimd).dma_start(
                            out=out_d[t * 128:(t + 1) * 128, :], in_=ot[:])
                    return [(1000, unit)]

                fillers = []

                def pop_fillers(budget_ns):
                    spent = 0
                    while fillers and spent < budget_ns:
                        ns, unit = fillers.pop(0)
                        unit()
                        spent += ns

                # ---------------- lead-in ----------------
                # warm the PE p-state while DMAs are in flight: dummy fp8
                # matmuls on memset data keep the array busy from ~0.6us so
                # the first real projections run at full clock, not the
                # 3x-slower cold state.
                wu = PM.tile([128, 128], fp32, tag="pm", name="warmup")
                o82f = ones82[:].rearrange("p c m -> p (c m)")
                for wi in range(24):
                    nc.tensor.matmul(
                        wu[:], lhsT=o82f, rhs=o82f,
                        start=True, stop=True, skip_group_check=True,
                    )
                # xt8 chunk 0 + wk8 first: they gate the whole pipeline
                def dma_xt(sc):
                    dma(xt8v[:, :, sc * SC:(sc + 1) * SC],
                        xt8_d[:].rearrange("p (j s) -> p j s", s=S)[:, :, sc * SC:(sc + 1) * SC])
                dma_xt(0)
                dma(wk8[:], wk_d[:])
                dma(bqk[:], bqk_d[:])
                dma(wq8[:], wq_d[:])
                dma_xt(1)
                dma(ident[:], id_d[:])
                dma_xt(2)
                dma_xt(3)
                dma(wv8[:], wv_d[:])
                dma(bv8p[:], bv8_d[:])
                dma(wes[:], wes_d[:])
                dma(bes[:], bes_d[:])
                dma(gexp[:], gexp_d[:])
                dma(wo[:], wo_d[:])
                # keep the pair transfers behind xt8-sc0/wk8/wq8 in the
                # shared DMA queue: they are not needed until the mean-pool
                with tc.tile_wait_until(0.0035):
                    for pr in range(ST // 2):
                        load_x_pair(pr)

                # minimal critical path: k i=0 sc=0 + q i=0 sc=0 lets chunk-0
                # scores start; the other k i=0 chunks interleave with the
                # first a-stretch (k-sc j emitted just before skp 2j).
                if KB_LEADIN >= 1:
                    qk_unit(wk8v, kT8, bqk[:, 2:3], 0, 0, QK_DRAIN[0])
                if KB_LEADIN >= 2:
                    qk_unit(wq8v, qT8, bqk[:, 0:1], 0, 0, QK_DRAIN[4])

                # deferred to fillers: all of i=1 (needed from chunk 4)
                for sc in range(NSC):
                    fillers.append((1100, (lambda s: lambda: qk_unit(
                        wk8v, kT8, bqk[:, 3:4], 1, s, QK_DRAIN[(8 + s) % 16]))(sc)))
                for sc in range(NSC):
                    fillers.append((1100, (lambda s: lambda: qk_unit(
                        wq8v, qT8, bqk[:, 1:2], 1, s, QK_DRAIN[(12 + s) % 16]))(sc)))

                def gates_block():
                    xcp = PCS.tile([128, DT], fp32, tag="cp", name="xcp")
                    for hf in range(2):
                        nc.vector.tensor_copy(
                            xmrow[:, hf * 512:(hf + 1) * 512], xmp[hf][0:1, :])
                    for j in range(DT):
                        nc.tensor.matmul(
                            xcp[:, j: j + 1],
                            lhsT=xmrow[:, j * 128:(j + 1) * 128],
                            rhs=ones1[:],
                            start=True, stop=True,
                        )
                    nc.vector.tensor_copy(xm_col[:], xcp[:])
                    gpe = PCS.tile([HL, 1], fp32, tag="cp", name="gpe")
                    gps = PCS.tile([HL, 1], fp32, tag="cp", name="gps")
                    for j in range(DT):
                        nc.tensor.matmul(
                            gpe[:], lhsT=wes[:, j * 8: j * 8 + 4],
                            rhs=xm_col[:, j: j + 1],
                            start=(j == 0), stop=(j == DT - 1),
                        )
                    for j in range(DT):
                        nc.tensor.matmul(
                            gps[:], lhsT=wes[:, j * 8 + 4: j * 8 + 8],
                            rhs=xm_col[:, j: j + 1],
                            start=(j == 0), stop=(j == DT - 1),
                        )
                    eth = GS.tile([HL, 1], fp32, tag="eth")
                    saf = GS.tile([HL, 1], fp32, tag="saf")
                    gate = GS.tile([HL, 1], fp32, tag="gate")
                    nc.scalar.activation(eth[:], gpe[:], AF.Sigmoid,
                                         bias=bes[:, 0:1], scale=1.0 / S)
                    nc.scalar.activation(saf[:], gps[:], AF.Sigmoid,
                                         bias=bes[:, 1:2], scale=1.0 / S)
                    nc.vector.tensor_mul(gate[:], eth[:], saf[:])
                    for i in range(2):
                        pgc = PCS.tile([128, 1], fp32, tag="cp", name=f"pgc{i}")
                        nc.tensor.matmul(
                            pgc[:], lhsT=gexp[:, i * 128:(i + 1) * 128], rhs=gate[:],
                            start=True, stop=True,
                        )
                        nc.vector.tensor_copy(gcol[:, i: i + 1], pgc[:])

                # ---------------- chunk loop ----------------
                def alloc_cps(i, sc):
                    # two 1-bank tiles, each holding two u-slots of [128, 130];
                    # a zero rank-1 matmul opens each bank's zero region so the
                    # interleaved PV accumulations need no group bookkeeping
                    pair = [PCS.tile([128, 2, 130], fp32, tag="cp",
                                     name=f"cp{i}_{sc}_{w}") for w in range(2)]
                    for w in range(2):
                        for a2 in range(2):
                            nc.tensor.matmul(
                                pair[w][:, a2, :],
                                lhsT=ones8r[:].rearrange("o (c m) -> o c m", c=2),
                                rhs=z8row[:].rearrange("o (c m) -> o c m", c=2)[:, :, 0:130],
                                start=True, stop=True,
                                skip_group_check=(a2 == 1),
                                perf_mode=DRm,
                            )
                    return [pair[u // 2][:, u % 2, :] for u in range(SC // 128)]

                def pv_mm_u(cps, ets, i, hh, u):
                    h = 2 * i + hh
                    for sk in range(ST):
                        skp, half = sk // 2, sk % 2
                        nc.tensor.matmul(
                            cps[u][:, hh * 65:(hh + 1) * 65],
                            lhsT=ets[skp][:, half * SC + u * 128: half * SC + (u + 1) * 128],
                            rhs=v[:, sk * HL * 65 + h * 65: sk * HL * 65 + (h + 1) * 65],
                            start=(sk == 0), stop=(sk == ST - 1),
                        )

                def make_tail_parts(i, sc, cps, last):
                    """Staggered per-u closures: partN(u) = DVE normalize only;
                    partT(u) = PE transpose (+ drain at odd u), emitted one
                    slot later so the transpose never queues on PE before its
                    normalize has finished on DVE. PV ran in the b-stretch."""
                    state = {"tp": None}

                    def partN(u):
                        def f():
                            t = sc * (SC // 128) + u
                            rec2 = RC.tile([128, 2], fp32, tag="rec",
                                           name=f"rec{i}_{t}")
                            cpv = cps[u].rearrange("p (h c) -> p h c", c=65)
                            nc.vector.reciprocal(rec2[:], cpv[:, :, 64])
                            with nc.allow_low_precision("softmax-normalized bf16 ctx"):
                                nc.vector.tensor_tensor(
                                    out=ctx[:, t * DL + i * 128: t * DL + (i + 1) * 128]
                                        .rearrange("p (h c) -> p h c", c=HD),
                                    in0=cpv[:, :, 0:HD],
                                    in1=rec2[:].unsqueeze(2).broadcast_to([128, 2, HD]),
                                    op=ALU.mult,
                                )
                        return f

                    def partT(u):
                        def f():
                            if u % 2 == 0:
                                # lives in the cps pool's idle window during the
                                # a-stretch, keeping all 3 PM slots for scores
                                state["tp"] = PCS.tile([128, 256], bf16, tag="cp",
                                                       name=f"tp{i}_{sc}_{u // 2}")
                            tp = state["tp"]
                            t = sc * (SC // 128) + u
                            nc.tensor.transpose(
                                tp[:, (u % 2) * 128:(u % 2 + 1) * 128],
                                ctx[:, t * DL + i * 128: t * DL + (i + 1) * 128],
                                ident[:],
                            )
                            if last:
                                # per-u drain so the final out-projs pipeline
                                nc.vector.tensor_scalar(
                                    out=ctxT[:, i * S + t * 128: i * S + (t + 1) * 128],
                                    in0=tp[:, (u % 2) * 128:(u % 2 + 1) * 128],
                                    scalar1=gcol[:, i: i + 1],
                                    scalar2=None,
                                    op0=ALU.mult,
                                )
                                for _, unit in outproj_units(t):
                                    unit()
                            elif u % 2 == 1:
                                nc.vector.tensor_scalar(
                                    out=ctxT[:, i * S + (t - 1) * 128: i * S + (t + 1) * 128],
                                    in0=tp[:],
                                    scalar1=gcol[:, i: i + 1],
                                    scalar2=None,
                                    op0=ALU.mult,
                                )
                                if u == SC // 128 - 1 and i == 1:
                                    for t2 in range(sc * 4, sc * 4 + 4):
                                        fillers.extend(outproj_units(t2))
                        return f

                    parts = []
                    nn = [partN(u) for u in range(SC // 128)]
                    tt = [partT(u) for u in range(SC // 128)]
                    parts.append(nn[0])
                    for u in range(1, SC // 128):
                        parts.append(lambda a=nn[u], b=tt[u - 1]: (a(), b()))
                    parts.append(tt[SC // 128 - 1])
                    return parts

                nchunks = [(i, sc) for i in range(2) for sc in range(NSC)][:KB_CHUNKS]
                pending_parts = []
                for n, (i, sc) in enumerate(nchunks):
                    first = (n == 0)
                    if first:
                        # mean-pool runs here: PE is otherwise idle during the
                        # first a-stretch and the PC psum pool is free.
                        xmp[0] = PCS.tile([64, 512], fp32, tag="cp", name="xmp0")
                        xmp[1] = PCS.tile([64, 512], fp32, tag="cp", name="xmp1")
                    ets_a = []
                    for skp in range(ST // 2):
                        if first and skp in (2, 4, 6):
                            # k i=0 chunk j just ahead of the scores needing it
                            qk_unit(wk8v, kT8, bqk[:, 2:3], 0, skp // 2,
                                    QK_DRAIN[skp // 2])
                        ets_a.append(score_exp(i, sc, 0, skp,
                                               EXP_PAT[n % len(EXP_PAT)][skp]))
                        if skp >= 1 and pending_parts:
                            pending_parts.pop(0)()
                        elif skp >= 2 and not first:
                            pop_fillers(FILLER_NS)
                    while pending_parts:
                        pending_parts.pop(0)()
                    if first:
                        # keep these off the critical lead-in: the scheduler
                        # would otherwise hoist them ahead of the k/q units
                        # and head-block PE on the slow x8-pair DMAs.
                        with tc.tile_wait_until(0.012):
                            for pr in range(ST // 2):
                                meanpool_mm(pr, xmp)
                        with tc.tile_wait_until(0.014):
                            gates_block()
                    cps = alloc_cps(i, sc)
                    ets_b = []
                    for skp in range(ST // 2):
                        ets_b.append(score_exp(i, sc, 1, skp,
                                               EXP_PAT[n % len(EXP_PAT)][8 + skp]))
                        if first:
                            v_unit(skp, V_DRAIN[skp % 8])
                        # PV for both heads streams through the b-stretch.
                        # Head b lags one skp so PE never queues behind the
                        # exp that was just issued for this skp.
                        pv_mm(cps, ets_a, i, 0, 2 * skp)
                        pv_mm(cps, ets_a, i, 0, 2 * skp + 1)
                        if skp >= 1:
                            pv_mm(cps, ets_b, i, 1, 2 * (skp - 1))
                            pv_mm(cps, ets_b, i, 1, 2 * (skp - 1) + 1)
                        if skp == 5 and i == 0 and sc < NSC - 1:
                            # q i=0 chunk sc+1 mid-b-stretch, off the boundary
                            qk_unit(wq8v, qT8, bqk[:, 0:1], 0, sc + 1,
                                    QK_DRAIN[4 + sc + 1])
                        if not first:
                            pop_fillers(FILLER_NS)
                    pending_parts = make_tail_parts(
                        i, sc, cps, last=(n == len(nchunks) - 1))
                    # the last head-b PV pair rides into the next a-stretch so
                    # the chunk boundary never waits on the final exp
                    def last_pv(cps=cps, ets_b=ets_b, i=i):
                        pv_mm(cps, ets_b, i, 1, ST - 2)
                        pv_mm(cps, ets_b, i, 1, ST - 1)
                    pending_parts.insert(0, last_pv)
                if KB_CHUNKS == 8:
                    for p in pending_parts:
                        p()
                    pop_fillers(10**9)
                else:
                    pending_parts.clear()
                    fillers.clear()
                    # touch out so the output DMA graph exists
                    ot = OB.tile([128, D], bf16, tag="ot", name="ot_stub")
                    nc.vector.memset(ot[:], 0.0)
                    nc.sync.dma_start(out=out_d[0:128, :], in_=ot[:])

    _split_multi_waits(nc)
    return nc


def _split_multi_waits(nc, skip=("InstEventSemaphore",)):
    """Hoist extra sync waits onto preceding same-engine NoOps.

    Walrus codegen can attach only one sync wait to some instruction
    encodings, so any instruction carrying N>1 waits is rewritten as N-1
    single-wait NoOps followed by the instruction with the last wait.
    """
    import concourse.mybir as mybir

    eng = {
        "EngineType.PE": nc.tensor,
        "EngineType.DVE": nc.vector,
        "EngineType.Activation": nc.scalar,
        "EngineType.Pool": nc.gpsimd,
        "EngineType.SP": nc.sync,
    }

    def fresh_nop(engine_key):
        nop = eng[engine_key].nop(hint="wsplit").ins
        for fn in nc.m.functions:
            for bb in fn.blocks:
                for i, ins in enumerate(bb.instructions):
                    if ins.name == nop.name:
                        del bb.instructions[i]
                        return nop
        raise RuntimeError("fresh nop not found")

    for fn in nc.m.functions:
        for bb in fn.blocks:
            insertions = []
            for idx, ins in enumerate(bb.instructions):
                if type(ins).__name__ in skip:
                    continue
                si = ins.sync_info
                if si is None or len(si.on_wait) <= 1:
                    continue
                waits = list(si.on_wait)
                nops = []
                for w in waits[:-1]:
                    nop = fresh_nop(str(ins.engine))
                    nop.sync_info = mybir.SyncInfo(on_wait=[w], on_update=[])
                    nops.append(nop)
                ins.sync_info = mybir.SyncInfo(
                    on_wait=[waits[-1]], on_update=list(si.on_update)
                )
                insertions.append((idx, nops))
            for idx, nops in reversed(insertions):
                bb.instructions[idx:idx] = nops


def _in_maps(inputs):
    x = np.ascontiguousarray(inputs["x"], np.float32)
    maps = []
    ident = np.eye(128, dtype=np.float32)
    gexp = np.zeros((HL, DL), np.float32)
    for h in range(HL):
        gexp[h, h * HD:(h + 1) * HD] = 1.0
    x8 = x.astype(F8)          # [B, S, D]
    for c in range(NCORES):
        b, g = c // HG, c % HG
        sl = slice(g * DL, (g + 1) * DL)
        hsl = slice(g * HL, (g + 1) * HL)
        # xT fp8 on 64 partitions: [64, jj, s] = x[b][s, jj*64+p]
        xt8 = np.ascontiguousarray(
            x8[b].T.reshape(2 * DT, 64, S).transpose(1, 0, 2).reshape(64, 2 * DT * S))
        # natural-x pairs: [128, pr, c2, col] = x[b][pr*256 + c2*128 + p, col]
        x8n = np.ascontiguousarray(
            x8[b].reshape(ST // 2, 2, 128, D).transpose(2, 0, 1, 3).reshape(128, ST * D))
        def wtile(w):
            return np.ascontiguousarray(
                w[:, sl].reshape(2 * DT, 64, DL).transpose(1, 0, 2).reshape(64, 2 * DT * DL).astype(F8))
        wo = inputs["Wo"][sl, :].reshape(2, 128, D).transpose(1, 0, 2).reshape(128, 2 * D)
        bqk = np.stack([
            inputs["bq"][sl][0:128], inputs["bq"][sl][128:256],
            inputs["bk"][sl][0:128], inputs["bk"][sl][128:256],
        ], axis=1).astype(np.float32)
        bv8p = np.zeros((1, 2 * DL), F8)
        bv8p[0, 0:DL] = inputs["bv"][sl].astype(F8)
        wes = np.concatenate([inputs["We"][:, hsl], inputs["Ws"][:, hsl]], axis=1)
        wes = wes.reshape(DT, 128, 2 * HL).transpose(1, 0, 2).reshape(128, DT * 2 * HL)
        bes = np.stack([inputs["be"][hsl], inputs["bs"][hsl]], axis=1)
        maps.append({
            "xt8": xt8,
            "x8": x8n,
            "wq8": wtile(inputs["Wq"]),
            "wk8": wtile(inputs["Wk"]),
            "wv8": wtile(inputs["Wv"]),
            "wo": np.ascontiguousarray(wo.astype(BF16)),
            "bqk": np.ascontiguousarray(bqk),
            "bv8p": bv8p,
            "wes": np.ascontiguousarray(wes.astype(np.float32)),
            "bes": np.ascontiguousarray(bes.astype(np.float32)),
            "gexp": gexp,
            "ident": ident.astype(BF16),
        })
    return maps


def kernel(**inputs):
    if "nc" not in _CACHE:
        _CACHE["nc"] = _build_nc()
    nc = _CACHE["nc"]
    maps = _in_maps({k: np.asarray(v) for k, v in inputs.items()})

    from concourse.bass_utils import run_bass_kernel_spmd

    trace = bool(int(os.environ.get("KERNEL_TRACE", "0")))
    res = run_bass_kernel_spmd(
        nc, maps, list(range(NCORES)), trace=trace,
        tmpdir=os.environ.get("KERNEL_TRACE_DIR") if trace else None,
    )
    _CACHE["last_result"] = res
    bo = np.asarray(inputs["bo"], np.float32)
    out = np.zeros((B, S, D), np.float32)
    for b in range(B):
        acc = np.zeros((S, D), np.float32)
        for g in range(HG):
            acc += res.results[b * HG + g]["out"].astype(np.float32)
        out[b] = acc + bo
    return out


# revision 47
# speedup vs baseline: 1.0562x; 1.0007x over previous
"""Trainium2 Bass kernel for nn_ASISNativeAttention (B=2,S=2048,D=1024,H=16).

Sharding: 8 cores = 2 batches x 4 head-groups (4 heads each); host splits
inputs per core and sums the 4 partial output projections per batch (+bo).

v2 design (vs the all-bf16 v1 at ~208us modeled): the two elementwise
engines (ACT, DVE) are the roofline -- 131K partition-lines of exp per core
-- so the exp work is split across BOTH of them, and the big matmuls move
to fp8 DoubleRow mode (2 k-tiles per instruction at half the per-row cost)
to keep PE under that roof:

  PE    q/k/v projections in fp8 DoubleRow from a host-transposed fp8 xT
        laid out on 64 partitions x 16 d-tiles (the PE hangs if a dual-fp8
        ldweights base partition changes inside one accumulation chain, and
        2*K*M may not exceed the 128x128 array, so chains stay K=64);
        scores per head via DoubleRow with a stride-0 replicated k-tile
        pair (computes 2*q.k, folded into the exp scale 1/16); PV and
        out-proj in bf16; mean-pool via fp8 DoubleRow ones-matmuls.
  ACT   exact exp (scale=1/16, psum->bf16) for ~60% of score tiles; its
        share of the qk/v/out-proj drains; the two gate sigmoids.
  DVE   Schraudolph exp for the rest: one tensor_scalar
        (psum*A16+B16 -> int16, round-to-nearest) whose int16 bits ARE the
        bf16 probs (max rel err ~3.5%, invisible after softmax: the PV
        ones-column sums the same stored values for the denominator);
        ctx normalize (strided reciprocal + stride-0-broadcast
        tensor_tensor); gated ctxT drain in 2x mode; remaining drains.
  Pool  x8-pair and output DMA via SWDGE, keeping HWDGE/SP free.

PSUM (8 banks): scores double-buffer 2x2 banks in a 3-slot pool shared
with transient out-proj/transpose/projection tiles; PV accumulators 2
banks (zero-opened by a rank-1 matmul so the interleaved per-head
accumulations need no group bookkeeping); per-2KB-bank accumulation
groups follow the zero-region rules (one pending group per region).

Measured (test.py): modeled exec 160.7us (v1 baseline: 208.0us); hardware
rel err vs reference 1.33e-2 (gate 2e-2; fp8 exposure is limited to
x / Wq,Wk,Wv / stored qT,kT -- probs, v, ctx, Wo stay bf16).

kernel.py is self-contained: numpy/ml_dtypes + the concourse tree at
/opt/trn_rl_repo.
"""

import os
import sys
import numpy as np
import ml_dtypes

BF16 = ml_dtypes.bfloat16
F8 = ml_dtypes.float8_e4m3

sys.path.insert(0, "/opt/trn_rl_repo")

B, S, D, H = 2, 2048, 1024, 16
HD = 64          # head dim
NCORES = 8
HG = 4           # head groups = cores per batch
HL = H // HG     # heads per core (4)
DL = D // HG     # local width (256)
ST = S // 128    # 16 s-tiles
DT = D // 128    # 8 d-tiles
SC = 512         # sq chunk width for scores
NSC = S // SC    # 4 chunks

# Schraudolph exp constants: int16 = rint(psum * A16 + B16); psum holds
# 2*q.k so the effective exp argument is psum/16 = q.k/8.
_C_SCH = 360000.0
A16 = float(2.0**7 / np.log(2.0) / 16.0)
B16 = float(127 * 2**7 - _C_SCH / 65536.0)

# --- engine-assignment knobs (tuned against TimelineSim) ---
# exp engine per chunk (16 chars: a-stretch skp 0-7 then b-stretch skp 0-7):
# 'A' = ACT exact exp, 'D' = DVE Schraudolph
EXP_PAT = [
    "ADAADAAD" "AADAADAD",   # 10A chunks (early D: DVE lane starts sooner)
    "AADAADAD" "ADAADADD",   # 9A chunks
] * 3 + ["AADAADAD" "AADAADAD", "AADAADAD" "ADAADDDA"]
if os.environ.get("KB_ALLACT"):
    EXP_PAT = ["A" * 16] * 8
KB_NOSWDGE = bool(os.environ.get("KB_NOSWDGE"))
KB_CHUNKS = int(os.environ.get("KB_CHUNKS", "8"))
KB_LEADIN = int(os.environ.get("KB_LEADIN", "99"))
# qk projection drain engines, one char per unit (k-i0 x4, q-i0 x4, then 8 i1)
QK_DRAIN = "ADADADADADADADAD"
# v drain engines, one per psum group (8)
V_DRAIN = "ADADADAD"
# out-proj drain engines, one per po half (32)
OUT_DRAIN = "AD" * 16
FILLER_NS = 1500   # filler budget popped per b-stretch skp

_CACHE = {}


def _build_nc():
    import concourse.bass as bass
    import concourse.mybir as mybir
    from concourse.tile import TileContext

    fp32 = mybir.dt.float32
    bf16 = mybir.dt.bfloat16
    f8 = mybir.dt.float8e4
    i16 = mybir.dt.int16
    AF = mybir.ActivationFunctionType
    ALU = mybir.AluOpType
    DRm = mybir.MatmulPerfMode.DoubleRow

    nc = bass.Bass()

    xt8_d = nc.declare_dram_parameter("xt8", [64, 2 * DT * S], f8, isOutput=False)
    x8_d = nc.declare_dram_parameter("x8", [128, ST * D], f8, isOutput=False)
    wq_d = nc.declare_dram_parameter("wq8", [64, 2 * DT * DL], f8, isOutput=False)
    wk_d = nc.declare_dram_parameter("wk8", [64, 2 * DT * DL], f8, isOutput=False)
    wv_d = nc.declare_dram_parameter("wv8", [64, 2 * DT * DL], f8, isOutput=False)
    wo_d = nc.declare_dram_parameter("wo", [128, 2 * D], bf16, isOutput=False)
    bqk_d = nc.declare_dram_parameter("bqk", [128, 4], fp32, isOutput=False)
    bv8_d = nc.declare_dram_parameter("bv8p", [1, 2 * DL], f8, isOutput=False)
    wes_d = nc.declare_dram_parameter("wes", [128, DT * 2 * HL], fp32, isOutput=False)
    bes_d = nc.declare_dram_parameter("bes", [HL, 2], fp32, isOutput=False)
    gexp_d = nc.declare_dram_parameter("gexp", [HL, DL], fp32, isOutput=False)
    id_d = nc.declare_dram_parameter("ident", [128, 128], bf16, isOutput=False)
    out_d = nc.declare_dram_parameter("out", [S, D], bf16, isOutput=True)

    with TileContext(nc) as tc:
        with tc.tile_pool(name="persist", bufs=1) as P:
            xt8 = P.tile([64, 2 * DT * S], f8, tag="xt8")
            wq8 = P.tile([64, 2 * DT * DL], f8, tag="wq8")
            wk8 = P.tile([64, 2 * DT * DL], f8, tag="wk8")
            wv8 = P.tile([64, 2 * DT * DL], f8, tag="wv8")
            wo = P.tile([128, 2 * D], bf16, tag="wo")
            qT8 = P.tile([128, 2 * S], f8, tag="qT8")
            kT8 = P.tile([128, 2 * S], f8, tag="kT8")
            v = P.tile([128, ST * HL * 65], bf16, tag="v")
            ctx = P.tile([128, ST * DL], bf16, tag="ctx")
            ctxT = P.tile([128, 2 * S], bf16, tag="ctxT")
            bqk = P.tile([128, 4], fp32, tag="bqk")
            bv8p = P.tile([1, 2 * DL], f8, tag="bv8p")
            ones82 = P.tile([128, 2, 64], f8, tag="ones82")
            ones8r = P.tile([1, 2 * 128], f8, tag="ones8r")
            z8row = P.tile([1, 2 * 260], f8, tag="z8row")
            wes = P.tile([128, DT * 2 * HL], fp32, tag="wes")
            bes = P.tile([HL, 2], fp32, tag="bes")
            gexp = P.tile([HL, DL], fp32, tag="gexp")
            ident = P.tile([128, 128], bf16, tag="ident")
            ones1 = P.tile([1, 1], fp32, tag="ones1")
            xmrow = P.tile([1, D], fp32, tag="xmrow")
            xm_col = P.tile([128, DT], fp32, tag="xm_col")
            gcol = P.tile([128, 2], fp32, tag="gcol")

            dma = nc.sync.dma_start

            def vview(t):
                return v[:].rearrange("p (t h c) -> p t h c", h=HL, c=65)[:, t]

            nc.vector.memset(ones82[:], 1.0)
            nc.vector.memset(ones8r[:], 1.0)
            nc.vector.memset(z8row[:], 0.0)
            nc.vector.memset(ones1[:], 1.0)
            # constant softmax-denominator columns of v
            nc.vector.memset(
                v[:].rearrange("p (t h c) -> p t h c", h=HL, c=65)[:, :, :, 64:65], 1.0
            )

            xt8v = xt8[:].rearrange("p (j s) -> p j s", s=S)      # [64, 16, S]
            wq8v = wq8[:].rearrange("p (j m) -> p j m", m=DL)     # [64, 16, DL]
            wk8v = wk8[:].rearrange("p (j m) -> p j m", m=DL)
            wv8v = wv8[:].rearrange("p (j m) -> p j m", m=DL)

            with (
                tc.tile_pool(name="x8l", bufs=8) as XL,
                tc.tile_pool(name="pm", bufs=3, space="PSUM") as PM,
                tc.tile_pool(name="pcps", bufs=2, space="PSUM") as PCS,
                tc.tile_pool(name="ets", bufs=26) as EX,
                tc.tile_pool(name="rc", bufs=8) as RC,
                tc.tile_pool(name="ob", bufs=2) as OB,
                tc.tile_pool(name="gs", bufs=1) as GS,
            ):
                xmp = [None, None]

                xbs = []

                def load_x_pair(pr):
                    """DMA natural-x pair pr via SWDGE (Pool) off the HWDGE path."""
                    xb = XL.tile([128, 2, D], f8, tag="xb", name=f"xb{pr}")
                    # SP HWDGE: queues naturally behind the critical xt8/w
                    # loads instead of jumping ahead from the idle Pool queue
                    nc.sync.dma_start(
                        out=xb[:],
                        in_=x8_d[:].rearrange("p (r c d) -> p r c d", c=2, d=D)[:, pr])
                    xbs.append(xb)

                def meanpool_mm(pr, xmps):
                    lhs = ones82[:]  # [128, 2, 64]: dual-fp8 ldweights needs wide M
                    for qh in range(4):
                        half, qq = qh // 2, qh % 2
                        first = (pr == 0 and qq == 0)
                        nc.tensor.matmul(
                            xmps[half][:, qq * 256:(qq + 1) * 256],
                            lhsT=lhs,
                            rhs=xbs[pr][:, :, qh * 256:(qh + 1) * 256],
                            start=first, stop=first,
                            skip_group_check=not first,
                            perf_mode=DRm,
                        )

                def qk_unit(w8v, dst8, bcol, i, sc, eng):
                    """Project one [128, SC] chunk of qT or kT (fp8 out)."""
                    pp = PM.tile([128, SC], fp32, tag="pm",
                                 name=f"pp{dst8.tensor.name}_{i}_{sc}")
                    for qq in range(2):
                        # x / weights live on 64 partitions x 16 d-tiles:
                        # dual-fp8 DR caps 2*K*M at the PE array size and the
                        # PE hangs if ldweights base-partition changes inside
                        # an accumulation chain, so every chain stays K=64.
                        for dp in range(DT):
                            first = (qq == 0 and dp == 0)
                            nc.tensor.matmul(
                                pp[:, qq * 256:(qq + 1) * 256],
                                lhsT=w8v[:, 2 * dp:2 * dp + 2, i * 128:(i + 1) * 128],
                                rhs=xt8v[:, 2 * dp:2 * dp + 2,
                                         sc * SC + qq * 256: sc * SC + (qq + 1) * 256],
                                start=first, stop=first,
                                skip_group_check=not first,
                                perf_mode=DRm,
                            )
                    dst = dst8[:, i * S + sc * SC: i * S + (sc + 1) * SC]
                    if eng == "A":
                        nc.scalar.activation(dst, pp[:], AF.Identity, bias=bcol)
                    else:
                        nc.vector.tensor_scalar(
                            out=dst, in0=pp[:], scalar1=bcol, scalar2=None,
                            op0=ALU.add,
                        )

                def v_unit(g, eng):
                    """Project v for s-tiles 2g, 2g+1 (one psum bank)."""
                    pv = PM.tile([128, 2 * DL], fp32, tag="pm", name=f"pv{g}")
                    for t2 in range(2):
                        t = 2 * g + t2
                        sl = pv[:, t2 * DL:(t2 + 1) * DL]
                        nc.tensor.matmul(
                            sl, lhsT=ones8r[:].rearrange("o (c m) -> o c m", c=2),
                            rhs=bv8p[:].rearrange("o (c m) -> o c m", c=2),
                            start=(t2 == 0), stop=(t2 == 0),
                            skip_group_check=(t2 == 1), perf_mode=DRm,
                        )
                        for dp in range(DT):
                            nc.tensor.matmul(
                                sl,
                                lhsT=xt8v[:, 2 * dp:2 * dp + 2, t * 128:(t + 1) * 128],
                                rhs=wv8v[:, 2 * dp:2 * dp + 2, :],
                                start=False, stop=False,
                                skip_group_check=True,
                                perf_mode=DRm,
                            )
                    for t2 in range(2):
                        t = 2 * g + t2
                        src = pv[:, t2 * DL:(t2 + 1) * DL].rearrange(
                            "p (h c) -> p h c", c=HD)
                        dst = vview(t)[:, :, 0:HD]
                        if eng == "A":
                            nc.scalar.copy(dst, src)
                        else:
                            nc.vector.tensor_copy(dst, src)

                def score_exp(i, sc, hh, skp, eng):
                    """Scores for sk-tiles (2skp, 2skp+1) x [sc*SC, (sc+1)*SC) of
                    head 2i+hh; one wide exp. psum holds 2*q.k (stride-0 DR)."""
                    r = hh * 64
                    sp = PM.tile([128, 2 * SC], fp32, tag="pm",
                                 name=f"sp{i}_{sc}_{hh}_{skp}")
                    for half in range(2):
                        sk = 2 * skp + half
                        lhsT = kT8[r:r + 64, i * S + sk * 128: i * S + (sk + 1) * 128] \
                            .unsqueeze(1).broadcast_to([64, 2, 128])
                        for qq in range(2):
                            rhs = qT8[r:r + 64,
                                      i * S + sc * SC + qq * 256: i * S + sc * SC + (qq + 1) * 256] \
                                .unsqueeze(1).broadcast_to([64, 2, 256])
                            # qq0 opens the bank's zero region; qq1 assigns
                            # into still-pending bytes (no second group)
                            nc.tensor.matmul(
                                sp[:, half * SC + qq * 256: half * SC + (qq + 1) * 256],
                                lhsT=lhsT, rhs=rhs, start=(qq == 0), stop=(qq == 0),
                                skip_group_check=(qq == 1),
                                perf_mode=DRm,
                            )
                    if eng == "A":
                        et = EX.tile([128, 2 * SC], bf16, tag="et",
                                     name=f"et{i}_{sc}_{hh}_{skp}")
                        nc.scalar.activation(et[:], sp[:], AF.Exp, scale=1.0 / 16.0)
                        return et[:]
                    et = EX.tile([128, 2 * SC], i16, tag="et",
                                 name=f"et{i}_{sc}_{hh}_{skp}")
                    nc.vector.tensor_scalar(
                        out=et[:], in0=sp[:], scalar1=A16, scalar2=B16,
                        op0=ALU.mult, op1=ALU.add,
                    )
                    return et[:].bitcast(bf16)

                def pv_mm(cps, ets, i, hh, sk):
                    h = 2 * i + hh
                    skp, half = sk // 2, sk % 2
                    for u in range(SC // 128):
                        nc.tensor.matmul(
                            cps[u][:, hh * 65:(hh + 1) * 65],
                            lhsT=ets[skp][:, half * SC + u * 128: half * SC + (u + 1) * 128],
                            rhs=v[:, sk * HL * 65 + h * 65: sk * HL * 65 + (h + 1) * 65],
                            start=False, stop=False, skip_group_check=True,
                        )

                def outproj_units(t):
                    def unit():
                        ot = OB.tile([128, D], bf16, tag="ot", name=f"ot{t}")
                        po = PM.tile([128, D], fp32, tag="pm", name=f"po{t}")
                        for n2 in range(2):
                            for i2 in range(2):
                                nc.tensor.matmul(
                                    po[:, n2 * 512:(n2 + 1) * 512],
                                    lhsT=ctxT[:, i2 * S + t * 128: i2 * S + (t + 1) * 128],
                                    rhs=wo[:, i2 * D + n2 * 512: i2 * D + (n2 + 1) * 512],
                                    start=(i2 == 0), stop=(i2 == 1),
                                )
                        if OUT_DRAIN[t % len(OUT_DRAIN)] == "A":
                            nc.scalar.copy(ot[:], po[:])
                        else:
                            nc.vector.tensor_copy(ot[:], po[:])
                        (nc.scalar if KB_NOSWDGE else nc.gpsimd).dma_start(
                            out=out_d[t * 128:(t + 1) * 128, :], in_=ot[:])
                    return [(1000, unit)]

                fillers = []

                def pop_fillers(budget_ns):
                    spent = 0
                    while fillers and spent < budget_ns:
                        ns, unit = fillers.pop(0)
                        unit()
                        spent += ns

                # ---------------- lead-in ----------------
                # warm the PE p-state while DMAs are in flight: dummy fp8
                # matmuls on memset data keep the array busy from ~0.6us so
                # the first real projections run at full clock, not the
                # 3x-slower cold state.
                wu = PM.tile([128, 128], fp32, tag="pm", name="warmup")
                o82f = ones82[:].rearrange("p c m -> p (c m)")
                for wi in range(40):
                    nc.tensor.matmul(
                        wu[:], lhsT=o82f, rhs=o82f,
                        start=True, stop=True, skip_group_check=True,
                    )
                # xt8 chunk 0 + wk8 first: they gate the whole pipeline
                def dma_xt(sc):
                    dma(xt8v[:, :, sc * SC:(sc + 1) * SC],
                        xt8_d[:].rearrange("p (j s) -> p j s", s=S)[:, :, sc * SC:(sc + 1) * SC])
                dma_xt(0)
                dma(wk8[:], wk_d[:])
                dma(bqk[:], bqk_d[:])
                dma(wq8[:], wq_d[:])
                dma_xt(1)
                dma(ident[:], id_d[:])
                dma_xt(2)
                dma_xt(3)
                dma(wv8[:], wv_d[:])
                dma(bv8p[:], bv8_d[:])
                dma(wes[:], wes_d[:])
                dma(bes[:], bes_d[:])
                dma(gexp[:], gexp_d[:])
                dma(wo[:], wo_d[:])
                # keep the pair transfers behind xt8-sc0/wk8/wq8 in the
                # shared DMA queue: they are not needed until the mean-pool
                with tc.tile_wait_until(0.0035):
                    for pr in range(ST // 2):
                        load_x_pair(pr)

                # minimal critical path: k i=0 sc=0 + q i=0 sc=0 lets chunk-0
                # scores start; the other k i=0 chunks interleave with the
                # first a-stretch (k-sc j emitted just before skp 2j).
                if KB_LEADIN >= 1:
                    qk_unit(wk8v, kT8, bqk[:, 2:3], 0, 0, QK_DRAIN[0])
                if KB_LEADIN >= 2:
                    qk_unit(wq8v, qT8, bqk[:, 0:1], 0, 0, QK_DRAIN[4])

                # deferred to fillers: all of i=1 (needed from chunk 4)
                for sc in range(NSC):
                    fillers.append((1100, (lambda s: lambda: qk_unit(
                        wk8v, kT8, bqk[:, 3:4], 1, s, QK_DRAIN[(8 + s) % 16]))(sc)))
                for sc in range(NSC):
                    fillers.append((1100, (lambda s: lambda: qk_unit(
                        wq8v, qT8, bqk[:, 1:2], 1, s, QK_DRAIN[(12 + s) % 16]))(sc)))

                def gates_block():
                    xcp = PCS.tile([128, DT], fp32, tag="cp", name="xcp")
                    for hf in range(2):
                        nc.vector.tensor_copy(
                            xmrow[:, hf * 512:(hf + 1) * 512], xmp[hf][0:1, :])
                    for j in range(DT):
                        nc.tensor.matmul(
                            xcp[:, j: j + 1],
                            lhsT=xmrow[:, j * 128:(j + 1) * 128],
                            rhs=ones1[:],
                            start=True, stop=True,
                        )
                    nc.vector.tensor_copy(xm_col[:], xcp[:])
                    gpe = PCS.tile([HL, 1], fp32, tag="cp", name="gpe")
                    gps = PCS.tile([HL, 1], fp32, tag="cp", name="gps")
                    for j in range(DT):
                        nc.tensor.matmul(
                            gpe[:], lhsT=wes[:, j * 8: j * 8 + 4],
                            rhs=xm_col[:, j: j + 1],
                            start=(j == 0), stop=(j == DT - 1),
                        )
                    for j in range(DT):
                        nc.tensor.matmul(
                            gps[:], lhsT=wes[:, j * 8 + 4: j * 8 + 8],
                            rhs=xm_col[:, j: j + 1],
                            start=(j == 0), stop=(j == DT - 1),
                        )
                    eth = GS.tile([HL, 1], fp32, tag="eth")
                    saf = GS.tile([HL, 1], fp32, tag="saf")
                    gate = GS.tile([HL, 1], fp32, tag="gate")
                    nc.scalar.activation(eth[:], gpe[:], AF.Sigmoid,
                                         bias=bes[:, 0:1], scale=1.0 / S)
                    nc.scalar.activation(saf[:], gps[:], AF.Sigmoid,
                                         bias=bes[:, 1:2], scale=1.0 / S)
                    nc.vector.tensor_mul(gate[:], eth[:], saf[:])
                    for i in range(2):
                        pgc = PCS.tile([128, 1], fp32, tag="cp", name=f"pgc{i}")
                        nc.tensor.matmul(
                            pgc[:], lhsT=gexp[:, i * 128:(i + 1) * 128], rhs=gate[:],
                            start=True, stop=True,
                        )
                        nc.vector.tensor_copy(gcol[:, i: i + 1], pgc[:])

                # ---------------- chunk loop ----------------
                def alloc_cps(i, sc):
                    # two 1-bank tiles, each holding two u-slots of [128, 130];
                    # a zero rank-1 matmul opens each bank's zero region so the
                    # interleaved PV accumulations need no group bookkeeping
                    pair = [PCS.tile([128, 2, 130], fp32, tag="cp",
                                     name=f"cp{i}_{sc}_{w}") for w in range(2)]
                    for w in range(2):
                        for a2 in range(2):
                            nc.tensor.matmul(
                                pair[w][:, a2, :],
                                lhsT=ones8r[:].rearrange("o (c m) -> o c m", c=2),
                                rhs=z8row[:].rearrange("o (c m) -> o c m", c=2)[:, :, 0:130],
                                start=True, stop=True,
                                skip_group_check=(a2 == 1),
                                perf_mode=DRm,
                            )
                    return [pair[u // 2][:, u % 2, :] for u in range(SC // 128)]

                def pv_mm_u(cps, ets, i, hh, u):
                    h = 2 * i + hh
                    for sk in range(ST):
                        skp, half = sk // 2, sk % 2
                        nc.tensor.matmul(
                            cps[u][:, hh * 65:(hh + 1) * 65],
                            lhsT=ets[skp][:, half * SC + u * 128: half * SC + (u + 1) * 128],
                            rhs=v[:, sk * HL * 65 + h * 65: sk * HL * 65 + (h + 1) * 65],
                            start=(sk == 0), stop=(sk == ST - 1),
                        )

                def make_tail_parts(i, sc, cps, last):
                    """Staggered per-u closures: partN(u) = DVE normalize only;
                    partT(u) = PE transpose (+ drain at odd u), emitted one
                    slot later so the transpose never queues on PE before its
                    normalize has finished on DVE. PV ran in the b-stretch."""
                    state = {"tp": None}

                    def partN(u):
                        def f():
                            t = sc * (SC // 128) + u
                            rec2 = RC.tile([128, 2], fp32, tag="rec",
                                           name=f"rec{i}_{t}")
                            cpv = cps[u].rearrange("p (h c) -> p h c", c=65)
                            nc.vector.reciprocal(rec2[:], cpv[:, :, 64])
                            with nc.allow_low_precision("softmax-normalized bf16 ctx"):
                                nc.vector.tensor_tensor(
                                    out=ctx[:, t * DL + i * 128: t * DL + (i + 1) * 128]
                                        .rearrange("p (h c) -> p h c", c=HD),
                                    in0=cpv[:, :, 0:HD],
                                    in1=rec2[:].unsqueeze(2).broadcast_to([128, 2, HD]),
                                    op=ALU.mult,
                                )
                        return f

                    def partT(u):
                        def f():
                            if u % 2 == 0:
                                # lives in the cps pool's idle window during the
                                # a-stretch, keeping all 3 PM slots for scores
                                state["tp"] = PCS.tile([128, 256], bf16, tag="cp",
                                                       name=f"tp{i}_{sc}_{u // 2}")
                            tp = state["tp"]
                            t = sc * (SC // 128) + u
                            nc.tensor.transpose(
                                tp[:, (u % 2) * 128:(u % 2 + 1) * 128],
                                ctx[:, t * DL + i * 128: t * DL + (i + 1) * 128],
                                ident[:],
                            )
                            if last:
                                # per-u drain so the final out-projs pipeline
                                nc.vector.tensor_scalar(
                                    out=ctxT[:, i * S + t * 128: i * S + (t + 1) * 128],
                                    in0=tp[:, (u % 2) * 128:(u % 2 + 1) * 128],
                                    scalar1=gcol[:, i: i + 1],
                                    scalar2=None,
                                    op0=ALU.mult,
                                )
                                for _, unit in outproj_units(t):
                                    unit()
                            elif u % 2 == 1:
                                nc.vector.tensor_scalar(
                                    out=ctxT[:, i * S + (t - 1) * 128: i * S + (t + 1) * 128],
                                    in0=tp[:],
                                    scalar1=gcol[:, i: i + 1],
                                    scalar2=None,
                                    op0=ALU.mult,
                                )
                                if u == SC // 128 - 1 and i == 1:
                                    for t2 in range(sc * 4, sc * 4 + 4):
                                        fillers.extend(outproj_units(t2))
                        return f

                    parts = []
                    nn = [partN(u) for u in range(SC // 128)]
                    tt = [partT(u) for u in range(SC // 128)]
                    parts.append(nn[0])
                    for u in range(1, SC // 128):
                        parts.append(lambda a=nn[u], b=tt[u - 1]: (a(), b()))
                    parts.append(tt[SC // 128 - 1])
                    return parts

                nchunks = [(i, sc) for i in range(2) for sc in range(NSC)][:KB_CHUNKS]
                pending_parts = []
                for n, (i, sc) in enumerate(nchunks):
                    first = (n == 0)
                    if first:
                        # mean-pool runs here: PE is otherwise idle during the
                        # first a-stretch and the PC psum pool is free.
                        xmp[0] = PCS.tile([64, 512], fp32, tag="cp", name="xmp0")
                        xmp[1] = PCS.tile([64, 512], fp32, tag="cp", name="xmp1")
                    ets_a = []
                    for skp in range(ST // 2):
                        if first and skp in (2, 4, 6):
                            # k i=0 chunk j just ahead of the scores needing it
                            qk_unit(wk8v, kT8, bqk[:, 2:3], 0, skp // 2,
                                    QK_DRAIN[skp // 2])
                        ets_a.append(score_exp(i, sc, 0, skp,
                                               EXP_PAT[n % len(EXP_PAT)][skp]))
                        if skp >= 1 and pending_parts:
                            pending_parts.pop(0)()
                        elif skp >= 2 and not first:
                            pop_fillers(FILLER_NS)
                    while pending_parts:
                        pending_parts.pop(0)()
                    if first:
                        # keep these off the critical lead-in: the scheduler
                        # would otherwise hoist them ahead of the k/q units
                        # and head-block PE on the slow x8-pair DMAs.
                        with tc.tile_wait_until(0.012):
                            for pr in range(ST // 2):
                                meanpool_mm(pr, xmp)
                        with tc.tile_wait_until(0.014):
                            gates_block()
                    cps = alloc_cps(i, sc)
                    ets_b = []
                    for skp in range(ST // 2):
                        ets_b.append(score_exp(i, sc, 1, skp,
                                               EXP_PAT[n % len(EXP_PAT)][8 + skp]))
                        if first:
                            v_unit(skp, V_DRAIN[skp % 8])
                        # PV for both heads streams through the b-stretch.
                        # Head b lags one skp so PE never queues behind the
                        # exp that was just issued for this skp.
                        pv_mm(cps, ets_a, i, 0, 2 * skp)
                        pv_mm(cps, ets_a, i, 0, 2 * skp + 1)
                        if skp >= 1:
                            pv_mm(cps, ets_b, i, 1, 2 * (skp - 1))
                            pv_mm(cps, ets_b, i, 1, 2 * (skp - 1) + 1)
                        if skp == 5 and i == 0 and sc < NSC - 1:
                            # q i=0 chunk sc+1 mid-b-stretch, off the boundary
                            qk_unit(wq8v, qT8, bqk[:, 0:1], 0, sc + 1,
                                    QK_DRAIN[4 + sc + 1])
                        if not first:
                            pop_fillers(FILLER_NS)
                    pending_parts = make_tail_parts(
                        i, sc, cps, last=(n == len(nchunks) - 1))
                    # the last head-b PV pair rides into the next a-stretch so
                    # the chunk boundary never waits on the final exp
                    def last_pv(cps=cps, ets_b=ets_b, i=i):
                        pv_mm(cps, ets_b, i, 1, ST - 2)
                        pv_mm(cps, ets_b, i, 1, ST - 1)
                    pending_parts.insert(0, last_pv)
                if KB_CHUNKS == 8:
                    for p in pending_parts:
                        p()
                    pop_fillers(10**9)
                else:
                    pending_parts.clear()
                    fillers.clear()
                    # touch out so the output DMA graph exists
                    ot = OB.tile([128, D], bf16, tag="ot", name="ot_stub")
                    nc.vector.memset(ot[:], 0.0)
                    nc.sync.dma_start(out=out_d[0:128, :], in_=ot[:])

    _split_multi_waits(nc)
    return nc


def _split_multi_waits(nc, skip=("InstEventSemaphore",)):
    """Hoist extra sync waits onto preceding same-engine NoOps.

    Walrus codegen can attach only one sync wait to some instruction
    encodings, so any instruction carrying N>1 waits is rewritten as N-1
    single-wait NoOps followed by the instruction with the last wait.
    """
    import concourse.mybir as mybir

    eng = {
        "EngineType.PE": nc.tensor,
        "EngineType.DVE": nc.vector,
        "EngineType.Activation": nc.scalar,
        "EngineType.Pool": nc.gpsimd,
        "EngineType.SP": nc.sync,
    }

    def fresh_nop(engine_key):
        nop = eng[engine_key].nop(hint="wsplit").ins
        for fn in nc.m.functions:
            for bb in fn.blocks:
                for i, ins in enumerate(bb.instructions):
                    if ins.name == nop.name:
                        del bb.instructions[i]
                        return nop
        raise RuntimeError("fresh nop not found")

    for fn in nc.m.functions:
        for bb in fn.blocks:
            insertions = []
            for idx, ins in enumerate(bb.instructions):
                if type(ins).__name__ in skip:
                    continue
                si = ins.sync_info
                if si is None or len(si.on_wait) <= 1:
                    continue
                waits = list(si.on_wait)
                nops = []
                for w in waits[:-1]:
                    nop = fresh_nop(str(ins.engine))
                    nop.sync_info = mybir.SyncInfo(on_wait=[w], on_update=[])
                    nops.append(nop)
                ins.sync_info = mybir.SyncInfo(
                    on_wait=[waits[-1]], on_update=list(si.on_update)
                )
                insertions.append((idx, nops))
            for idx, nops in reversed(insertions):
                bb.instructions[idx:idx] = nops


def _in_maps(inputs):
    x = np.ascontiguousarray(inputs["x"], np.float32)
    maps = []
    ident = np.eye(128, dtype=np.float32)
    gexp = np.zeros((HL, DL), np.float32)
    for h in range(HL):
        gexp[h, h * HD:(h + 1) * HD] = 1.0
    x8 = x.astype(F8)          # [B, S, D]
    for c in range(NCORES):
        b, g = c // HG, c % HG
        sl = slice(g * DL, (g + 1) * DL)
        hsl = slice(g * HL, (g + 1) * HL)
        # xT fp8 on 64 partitions: [64, jj, s] = x[b][s, jj*64+p]
        xt8 = np.ascontiguousarray(
            x8[b].T.reshape(2 * DT, 64, S).transpose(1, 0, 2).reshape(64, 2 * DT * S))
        # natural-x pairs: [128, pr, c2, col] = x[b][pr*256 + c2*128 + p, col]
        x8n = np.ascontiguousarray(
            x8[b].reshape(ST // 2, 2, 128, D).transpose(2, 0, 1, 3).reshape(128, ST * D))
        def wtile(w):
            return np.ascontiguousarray(
                w[:, sl].reshape(2 * DT, 64, DL).transpose(1, 0, 2).reshape(64, 2 * DT * DL).astype(F8))
        wo = inputs["Wo"][sl, :].reshape(2, 128, D).transpose(1, 0, 2).reshape(128, 2 * D)
        bqk = np.stack([
            inputs["bq"][sl][0:128], inputs["bq"][sl][128:256],
            inputs["bk"][sl][0:128], inputs["bk"][sl][128:256],
        ], axis=1).astype(np.float32)
        bv8p = np.zeros((1, 2 * DL), F8)
        bv8p[0, 0:DL] = inputs["bv"][sl].astype(F8)
        wes = np.concatenate([inputs["We"][:, hsl], inputs["Ws"][:, hsl]], axis=1)
        wes = wes.reshape(DT, 128, 2 * HL).transpose(1, 0, 2).reshape(128, DT * 2 * HL)
        bes = np.stack([inputs["be"][hsl], inputs["bs"][hsl]], axis=1)
        maps.append({
            "xt8": xt8,
            "x8": x8n,
            "wq8": wtile(inputs["Wq"]),
            "wk8": wtile(inputs["Wk"]),
            "wv8": wtile(inputs["Wv"]),
            "wo": np.ascontiguousarray(wo.astype(BF16)),
            "bqk": np.ascontiguousarray(bqk),
            "bv8p": bv8p,
            "wes": np.ascontiguousarray(wes.astype(np.float32)),
            "bes": np.ascontiguousarray(bes.astype(np.float32)),
            "gexp": gexp,
            "ident": ident.astype(BF16),
        })
    return maps


def kernel(**inputs):
    if "nc" not in _CACHE:
        _CACHE["nc"] = _build_nc()
    nc = _CACHE["nc"]
    maps = _in_maps({k: np.asarray(v) for k, v in inputs.items()})

    from concourse.bass_utils import run_bass_kernel_spmd

    trace = bool(int(os.environ.get("KERNEL_TRACE", "0")))
    res = run_bass_kernel_spmd(
        nc, maps, list(range(NCORES)), trace=trace,
        tmpdir=os.environ.get("KERNEL_TRACE_DIR") if trace else None,
    )
    _CACHE["last_result"] = res
    bo = np.asarray(inputs["bo"], np.float32)
    out = np.zeros((B, S, D), np.float32)
    for b in range(B):
        acc = np.zeros((S, D), np.float32)
        for g in range(HG):
            acc += res.results[b * HG + g]["out"].astype(np.float32)
        out[b] = acc + bo
    return out


# revision 51
# speedup vs baseline: 1.0636x; 1.0070x over previous
"""Trainium2 Bass kernel for nn_ASISNativeAttention (B=2,S=2048,D=1024,H=16).

Sharding: 8 cores = 2 batches x 4 head-groups (4 heads each); host splits
inputs per core and sums the 4 partial output projections per batch (+bo).

v2 design (vs the all-bf16 v1 at ~208us modeled): the two elementwise
engines (ACT, DVE) are the roofline -- 131K partition-lines of exp per core
-- so the exp work is split across BOTH of them, and the big matmuls move
to fp8 DoubleRow mode (2 k-tiles per instruction at half the per-row cost)
to keep PE under that roof:

  PE    q/k/v projections in fp8 DoubleRow from a host-transposed fp8 xT
        laid out on 64 partitions x 16 d-tiles (the PE hangs if a dual-fp8
        ldweights base partition changes inside one accumulation chain, and
        2*K*M may not exceed the 128x128 array, so chains stay K=64);
        scores per head via DoubleRow with a stride-0 replicated k-tile
        pair (computes 2*q.k, folded into the exp scale 1/16); PV and
        out-proj in bf16; mean-pool via fp8 DoubleRow ones-matmuls.
  ACT   exact exp (scale=1/16, psum->bf16) for ~60% of score tiles; its
        share of the qk/v/out-proj drains; the two gate sigmoids.
  DVE   Schraudolph exp for the rest: one tensor_scalar
        (psum*A16+B16 -> int16, round-to-nearest) whose int16 bits ARE the
        bf16 probs (max rel err ~3.5%, invisible after softmax: the PV
        ones-column sums the same stored values for the denominator);
        ctx normalize (strided reciprocal + stride-0-broadcast
        tensor_tensor); gated ctxT drain in 2x mode; remaining drains.
  Pool  x8-pair and output DMA via SWDGE, keeping HWDGE/SP free.

PSUM (8 banks): scores double-buffer 2x2 banks in a 3-slot pool shared
with transient out-proj/transpose/projection tiles; PV accumulators 2
banks (zero-opened by a rank-1 matmul so the interleaved per-head
accumulations need no group bookkeeping); per-2KB-bank accumulation
groups follow the zero-region rules (one pending group per region).

Measured (test.py): modeled exec 159.6us (v1 baseline: 208.0us); hardware
rel err vs reference 1.33e-2 (gate 2e-2; fp8 exposure is limited to
x / Wq,Wk,Wv / stored qT,kT -- probs, v, ctx, Wo stay bf16).

kernel.py is self-contained: numpy/ml_dtypes + the concourse tree at
/opt/trn_rl_repo.
"""

import os
import sys
import numpy as np
import ml_dtypes

BF16 = ml_dtypes.bfloat16
F8 = ml_dtypes.float8_e4m3

sys.path.insert(0, "/opt/trn_rl_repo")

B, S, D, H = 2, 2048, 1024, 16
HD = 64          # head dim
NCORES = 8
HG = 4           # head groups = cores per batch
HL = H // HG     # heads per core (4)
DL = D // HG     # local width (256)
ST = S // 128    # 16 s-tiles
DT = D // 128    # 8 d-tiles
SC = 512         # sq chunk width for scores
NSC = S // SC    # 4 chunks

# Schraudolph exp constants: int16 = rint(psum * A16 + B16); psum holds
# 2*q.k so the effective exp argument is psum/16 = q.k/8.
_C_SCH = 360000.0
A16 = float(2.0**7 / np.log(2.0) / 16.0)
B16 = float(127 * 2**7 - _C_SCH / 65536.0)

# --- engine-assignment knobs (tuned against TimelineSim) ---
# exp engine per chunk (16 chars: a-stretch skp 0-7 then b-stretch skp 0-7):
# 'A' = ACT exact exp, 'D' = DVE Schraudolph
EXP_PAT = [
    "ADAADAAD" "AADAADAD",   # 10A chunks (early D: DVE lane starts sooner)
    "AADAADAD" "ADAADADD",   # 9A chunks
] * 3 + ["AADAADAD" "AADAADAD", "AADAADAD" "ADAADDDA"]
if os.environ.get("KB_ALLACT"):
    EXP_PAT = ["A" * 16] * 8
KB_NOSWDGE = bool(os.environ.get("KB_NOSWDGE"))
KB_CHUNKS = int(os.environ.get("KB_CHUNKS", "8"))
KB_LEADIN = int(os.environ.get("KB_LEADIN", "99"))
# qk projection drain engines, one char per unit (k-i0 x4, q-i0 x4, then 8 i1)
QK_DRAIN = "ADADADADADADADAD"
# v drain engines, one per psum group (8)
V_DRAIN = "ADADADAD"
# out-proj drain engines, one per po half (32)
OUT_DRAIN = "AD" * 16
FILLER_NS = 1500   # filler budget popped per b-stretch skp

_CACHE = {}


def _build_nc():
    import concourse.bass as bass
    import concourse.mybir as mybir
    from concourse.tile import TileContext

    fp32 = mybir.dt.float32
    bf16 = mybir.dt.bfloat16
    f8 = mybir.dt.float8e4
    i16 = mybir.dt.int16
    AF = mybir.ActivationFunctionType
    ALU = mybir.AluOpType
    DRm = mybir.MatmulPerfMode.DoubleRow

    nc = bass.Bass()

    xt8_d = nc.declare_dram_parameter("xt8", [64, 2 * DT * S], f8, isOutput=False)
    x8_d = nc.declare_dram_parameter("x8", [128, ST * D], f8, isOutput=False)
    wq_d = nc.declare_dram_parameter("wq8", [64, 2 * DT * DL], f8, isOutput=False)
    wk_d = nc.declare_dram_parameter("wk8", [64, 2 * DT * DL], f8, isOutput=False)
    wv_d = nc.declare_dram_parameter("wv8", [64, 2 * DT * DL], f8, isOutput=False)
    wo_d = nc.declare_dram_parameter("wo", [128, 2 * D], bf16, isOutput=False)
    bqk_d = nc.declare_dram_parameter("bqk", [128, 4], fp32, isOutput=False)
    bv8_d = nc.declare_dram_parameter("bv8p", [1, 2 * DL], f8, isOutput=False)
    wes_d = nc.declare_dram_parameter("wes", [128, DT * 2 * HL], fp32, isOutput=False)
    bes_d = nc.declare_dram_parameter("bes", [HL, 2], fp32, isOutput=False)
    gexp_d = nc.declare_dram_parameter("gexp", [HL, DL], fp32, isOutput=False)
    id_d = nc.declare_dram_parameter("ident", [128, 128], bf16, isOutput=False)
    out_d = nc.declare_dram_parameter("out", [S, D], bf16, isOutput=True)

    with TileContext(nc) as tc:
        with tc.tile_pool(name="persist", bufs=1) as P:
            xt8 = P.tile([64, 2 * DT * S], f8, tag="xt8")
            wq8 = P.tile([64, 2 * DT * DL], f8, tag="wq8")
            wk8 = P.tile([64, 2 * DT * DL], f8, tag="wk8")
            wv8 = P.tile([64, 2 * DT * DL], f8, tag="wv8")
            wo = P.tile([128, 2 * D], bf16, tag="wo")
            qT8 = P.tile([128, 2 * S], f8, tag="qT8")
            kT8 = P.tile([128, 2 * S], f8, tag="kT8")
            v = P.tile([128, ST * HL * 65], bf16, tag="v")
            ctx = P.tile([128, ST * DL], bf16, tag="ctx")
            ctxT = P.tile([128, 2 * S], bf16, tag="ctxT")
            bqk = P.tile([128, 4], fp32, tag="bqk")
            bv8p = P.tile([1, 2 * DL], f8, tag="bv8p")
            ones82 = P.tile([128, 2, 64], f8, tag="ones82")
            ones8r = P.tile([1, 2 * 128], f8, tag="ones8r")
            z8row = P.tile([1, 2 * 260], f8, tag="z8row")
            wes = P.tile([128, DT * 2 * HL], fp32, tag="wes")
            bes = P.tile([HL, 2], fp32, tag="bes")
            gexp = P.tile([HL, DL], fp32, tag="gexp")
            ident = P.tile([128, 128], bf16, tag="ident")
            ones1 = P.tile([1, 1], fp32, tag="ones1")
            xmrow = P.tile([1, D], fp32, tag="xmrow")
            xm_col = P.tile([128, DT], fp32, tag="xm_col")
            gcol = P.tile([128, 2], fp32, tag="gcol")

            dma = nc.sync.dma_start

            def vview(t):
                return v[:].rearrange("p (t h c) -> p t h c", h=HL, c=65)[:, t]

            nc.vector.memset(ones82[:], 1.0)
            nc.vector.memset(ones8r[:], 1.0)
            nc.vector.memset(z8row[:], 0.0)
            nc.vector.memset(ones1[:], 1.0)
            # constant softmax-denominator columns of v
            nc.vector.memset(
                v[:].rearrange("p (t h c) -> p t h c", h=HL, c=65)[:, :, :, 64:65], 1.0
            )

            xt8v = xt8[:].rearrange("p (j s) -> p j s", s=S)      # [64, 16, S]
            wq8v = wq8[:].rearrange("p (j m) -> p j m", m=DL)     # [64, 16, DL]
            wk8v = wk8[:].rearrange("p (j m) -> p j m", m=DL)
            wv8v = wv8[:].rearrange("p (j m) -> p j m", m=DL)

            with (
                tc.tile_pool(name="x8l", bufs=8) as XL,
                tc.tile_pool(name="pm", bufs=3, space="PSUM") as PM,
                tc.tile_pool(name="pcps", bufs=2, space="PSUM") as PCS,
                tc.tile_pool(name="ets", bufs=26) as EX,
                tc.tile_pool(name="rc", bufs=8) as RC,
                tc.tile_pool(name="ob", bufs=2) as OB,
                tc.tile_pool(name="gs", bufs=1) as GS,
            ):
                xmp = [None, None]

                xbs = []

                def load_x_pair(pr):
                    """DMA natural-x pair pr via SWDGE (Pool) off the HWDGE path."""
                    xb = XL.tile([128, 2, D], f8, tag="xb", name=f"xb{pr}")
                    # SP HWDGE: queues naturally behind the critical xt8/w
                    # loads instead of jumping ahead from the idle Pool queue
                    nc.sync.dma_start(
                        out=xb[:],
                        in_=x8_d[:].rearrange("p (r c d) -> p r c d", c=2, d=D)[:, pr])
                    xbs.append(xb)

                def meanpool_mm(pr, xmps):
                    lhs = ones82[:]  # [128, 2, 64]: dual-fp8 ldweights needs wide M
                    for qh in range(4):
                        half, qq = qh // 2, qh % 2
                        first = (pr == 0 and qq == 0)
                        nc.tensor.matmul(
                            xmps[half][:, qq * 256:(qq + 1) * 256],
                            lhsT=lhs,
                            rhs=xbs[pr][:, :, qh * 256:(qh + 1) * 256],
                            start=first, stop=first,
                            skip_group_check=not first,
                            perf_mode=DRm,
                        )

                def qk_unit(w8v, dst8, bcol, i, sc, eng):
                    """Project one [128, SC] chunk of qT or kT (fp8 out)."""
                    pp = PM.tile([128, SC], fp32, tag="pm",
                                 name=f"pp{dst8.tensor.name}_{i}_{sc}")
                    for qq in range(2):
                        # x / weights live on 64 partitions x 16 d-tiles:
                        # dual-fp8 DR caps 2*K*M at the PE array size and the
                        # PE hangs if ldweights base-partition changes inside
                        # an accumulation chain, so every chain stays K=64.
                        for dp in range(DT):
                            first = (qq == 0 and dp == 0)
                            nc.tensor.matmul(
                                pp[:, qq * 256:(qq + 1) * 256],
                                lhsT=w8v[:, 2 * dp:2 * dp + 2, i * 128:(i + 1) * 128],
                                rhs=xt8v[:, 2 * dp:2 * dp + 2,
                                         sc * SC + qq * 256: sc * SC + (qq + 1) * 256],
                                start=first, stop=first,
                                skip_group_check=not first,
                                perf_mode=DRm,
                            )
                    dst = dst8[:, i * S + sc * SC: i * S + (sc + 1) * SC]
                    if eng == "A":
                        nc.scalar.activation(dst, pp[:], AF.Identity, bias=bcol)
                    else:
                        nc.vector.tensor_scalar(
                            out=dst, in0=pp[:], scalar1=bcol, scalar2=None,
                            op0=ALU.add,
                        )

                def v_unit(g, eng):
                    """Project v for s-tiles 2g, 2g+1 (one psum bank)."""
                    pv = PM.tile([128, 2 * DL], fp32, tag="pm", name=f"pv{g}")
                    for t2 in range(2):
                        t = 2 * g + t2
                        sl = pv[:, t2 * DL:(t2 + 1) * DL]
                        nc.tensor.matmul(
                            sl, lhsT=ones8r[:].rearrange("o (c m) -> o c m", c=2),
                            rhs=bv8p[:].rearrange("o (c m) -> o c m", c=2),
                            start=(t2 == 0), stop=(t2 == 0),
                            skip_group_check=(t2 == 1), perf_mode=DRm,
                        )
                        for dp in range(DT):
                            nc.tensor.matmul(
                                sl,
                                lhsT=xt8v[:, 2 * dp:2 * dp + 2, t * 128:(t + 1) * 128],
                                rhs=wv8v[:, 2 * dp:2 * dp + 2, :],
                                start=False, stop=False,
                                skip_group_check=True,
                                perf_mode=DRm,
                            )
                    for t2 in range(2):
                        t = 2 * g + t2
                        src = pv[:, t2 * DL:(t2 + 1) * DL].rearrange(
                            "p (h c) -> p h c", c=HD)
                        dst = vview(t)[:, :, 0:HD]
                        if eng == "A":
                            nc.scalar.copy(dst, src)
                        else:
                            nc.vector.tensor_copy(dst, src)

                def score_exp(i, sc, hh, skp, eng):
                    """Scores for sk-tiles (2skp, 2skp+1) x [sc*SC, (sc+1)*SC) of
                    head 2i+hh; one wide exp. psum holds 2*q.k (stride-0 DR)."""
                    r = hh * 64
                    sp = PM.tile([128, 2 * SC], fp32, tag="pm",
                                 name=f"sp{i}_{sc}_{hh}_{skp}")
                    for half in range(2):
                        sk = 2 * skp + half
                        lhsT = kT8[r:r + 64, i * S + sk * 128: i * S + (sk + 1) * 128] \
                            .unsqueeze(1).broadcast_to([64, 2, 128])
                        for qq in range(2):
                            rhs = qT8[r:r + 64,
                                      i * S + sc * SC + qq * 256: i * S + sc * SC + (qq + 1) * 256] \
                                .unsqueeze(1).broadcast_to([64, 2, 256])
                            # qq0 opens the bank's zero region; qq1 assigns
                            # into still-pending bytes (no second group)
                            nc.tensor.matmul(
                                sp[:, half * SC + qq * 256: half * SC + (qq + 1) * 256],
                                lhsT=lhsT, rhs=rhs, start=(qq == 0), stop=(qq == 0),
                                skip_group_check=(qq == 1),
                                perf_mode=DRm,
                            )
                    if eng == "A":
                        et = EX.tile([128, 2 * SC], bf16, tag="et",
                                     name=f"et{i}_{sc}_{hh}_{skp}")
                        nc.scalar.activation(et[:], sp[:], AF.Exp, scale=1.0 / 16.0)
                        return et[:]
                    et = EX.tile([128, 2 * SC], i16, tag="et",
                                 name=f"et{i}_{sc}_{hh}_{skp}")
                    nc.vector.tensor_scalar(
                        out=et[:], in0=sp[:], scalar1=A16, scalar2=B16,
                        op0=ALU.mult, op1=ALU.add,
                    )
                    return et[:].bitcast(bf16)

                def pv_mm(cps, ets, i, hh, sk):
                    h = 2 * i + hh
                    skp, half = sk // 2, sk % 2
                    for u in range(SC // 128):
                        nc.tensor.matmul(
                            cps[u][:, hh * 65:(hh + 1) * 65],
                            lhsT=ets[skp][:, half * SC + u * 128: half * SC + (u + 1) * 128],
                            rhs=v[:, sk * HL * 65 + h * 65: sk * HL * 65 + (h + 1) * 65],
                            start=False, stop=False, skip_group_check=True,
                        )

                def outproj_units(t):
                    def unit():
                        ot = OB.tile([128, D], bf16, tag="ot", name=f"ot{t}")
                        po = PM.tile([128, D], fp32, tag="pm", name=f"po{t}")
                        for n2 in range(2):
                            for i2 in range(2):
                                nc.tensor.matmul(
                                    po[:, n2 * 512:(n2 + 1) * 512],
                                    lhsT=ctxT[:, i2 * S + t * 128: i2 * S + (t + 1) * 128],
                                    rhs=wo[:, i2 * D + n2 * 512: i2 * D + (n2 + 1) * 512],
                                    start=(i2 == 0), stop=(i2 == 1),
                                )
                        if OUT_DRAIN[t % len(OUT_DRAIN)] == "A":
                            nc.scalar.copy(ot[:], po[:])
                        else:
                            nc.vector.tensor_copy(ot[:], po[:])
                        (nc.scalar if KB_NOSWDGE else nc.gpsimd).dma_start(
                            out=out_d[t * 128:(t + 1) * 128, :], in_=ot[:])
                    return [(1000, unit)]

                fillers = []

                def pop_fillers(budget_ns):
                    spent = 0
                    while fillers and spent < budget_ns:
                        ns, unit = fillers.pop(0)
                        unit()
                        spent += ns

                # ---------------- lead-in ----------------
                # warm the PE p-state while DMAs are in flight: dummy fp8
                # matmuls on memset data keep the array busy from ~0.6us so
                # the first real projections run at full clock, not the
                # 3x-slower cold state.
                wu = PM.tile([128, 128], fp32, tag="pm", name="warmup")
                o82f = ones82[:].rearrange("p c m -> p (c m)")
                for wi in range(40):
                    nc.tensor.matmul(
                        wu[:], lhsT=o82f, rhs=o82f,
                        start=True, stop=True, skip_group_check=True,
                    )
                # xt8 chunk 0 + wk8 first: they gate the whole pipeline
                def dma_xt(sc):
                    dma(xt8v[:, :, sc * SC:(sc + 1) * SC],
                        xt8_d[:].rearrange("p (j s) -> p j s", s=S)[:, :, sc * SC:(sc + 1) * SC])
                dma_xt(0)
                dma(wk8[:], wk_d[:])
                dma(bqk[:], bqk_d[:])
                dma(wq8[:], wq_d[:])
                dma_xt(1)
                dma(ident[:], id_d[:])
                dma_xt(2)
                dma_xt(3)
                dma(wv8[:], wv_d[:])
                dma(bv8p[:], bv8_d[:])
                dma(wes[:], wes_d[:])
                dma(bes[:], bes_d[:])
                dma(gexp[:], gexp_d[:])
                dma(wo[:], wo_d[:])
                # keep the pair transfers behind xt8-sc0/wk8/wq8 in the
                # shared DMA queue: they are not needed until the mean-pool
                with tc.tile_wait_until(0.0035):
                    for pr in range(ST // 2):
                        load_x_pair(pr)

                # minimal critical path: k i=0 sc=0 + q i=0 sc=0 lets chunk-0
                # scores start; the other k i=0 chunks interleave with the
                # first a-stretch (k-sc j emitted just before skp 2j).
                if KB_LEADIN >= 1:
                    qk_unit(wk8v, kT8, bqk[:, 2:3], 0, 0, QK_DRAIN[0])
                if KB_LEADIN >= 2:
                    qk_unit(wq8v, qT8, bqk[:, 0:1], 0, 0, QK_DRAIN[4])

                # deferred to fillers: all of i=1 (needed from chunk 4)
                for sc in range(NSC):
                    fillers.append((1100, (lambda s: lambda: qk_unit(
                        wk8v, kT8, bqk[:, 3:4], 1, s, QK_DRAIN[(8 + s) % 16]))(sc)))
                for sc in range(NSC):
                    fillers.append((1100, (lambda s: lambda: qk_unit(
                        wq8v, qT8, bqk[:, 1:2], 1, s, QK_DRAIN[(12 + s) % 16]))(sc)))

                def gates_block():
                    xcp = PCS.tile([128, DT], fp32, tag="cp", name="xcp")
                    for hf in range(2):
                        nc.vector.tensor_copy(
                            xmrow[:, hf * 512:(hf + 1) * 512], xmp[hf][0:1, :])
                    for j in range(DT):
                        nc.tensor.matmul(
                            xcp[:, j: j + 1],
                            lhsT=xmrow[:, j * 128:(j + 1) * 128],
                            rhs=ones1[:],
                            start=True, stop=True,
                        )
                    nc.vector.tensor_copy(xm_col[:], xcp[:])
                    gpe = PCS.tile([HL, 1], fp32, tag="cp", name="gpe")
                    gps = PCS.tile([HL, 1], fp32, tag="cp", name="gps")
                    for j in range(DT):
                        nc.tensor.matmul(
                            gpe[:], lhsT=wes[:, j * 8: j * 8 + 4],
                            rhs=xm_col[:, j: j + 1],
                            start=(j == 0), stop=(j == DT - 1),
                        )
                    for j in range(DT):
                        nc.tensor.matmul(
                            gps[:], lhsT=wes[:, j * 8 + 4: j * 8 + 8],
                            rhs=xm_col[:, j: j + 1],
                            start=(j == 0), stop=(j == DT - 1),
                        )
                    eth = GS.tile([HL, 1], fp32, tag="eth")
                    saf = GS.tile([HL, 1], fp32, tag="saf")
                    gate = GS.tile([HL, 1], fp32, tag="gate")
                    nc.scalar.activation(eth[:], gpe[:], AF.Sigmoid,
                                         bias=bes[:, 0:1], scale=1.0 / S)
                    nc.scalar.activation(saf[:], gps[:], AF.Sigmoid,
                                         bias=bes[:, 1:2], scale=1.0 / S)
                    nc.vector.tensor_mul(gate[:], eth[:], saf[:])
                    for i in range(2):
                        pgc = PCS.tile([128, 1], fp32, tag="cp", name=f"pgc{i}")
                        nc.tensor.matmul(
                            pgc[:], lhsT=gexp[:, i * 128:(i + 1) * 128], rhs=gate[:],
                            start=True, stop=True,
                        )
                        nc.vector.tensor_copy(gcol[:, i: i + 1], pgc[:])

                # ---------------- chunk loop ----------------
                def alloc_cps(i, sc):
                    # two 1-bank tiles, each holding two u-slots of [128, 130];
                    # a zero rank-1 matmul opens each bank's zero region so the
                    # interleaved PV accumulations need no group bookkeeping
                    pair = [PCS.tile([128, 2, 130], fp32, tag="cp",
                                     name=f"cp{i}_{sc}_{w}") for w in range(2)]
                    for w in range(2):
                        for a2 in range(2):
                            nc.tensor.matmul(
                                pair[w][:, a2, :],
                                lhsT=ones8r[:].rearrange("o (c m) -> o c m", c=2),
                                rhs=z8row[:].rearrange("o (c m) -> o c m", c=2)[:, :, 0:130],
                                start=True, stop=True,
                                skip_group_check=(a2 == 1),
                                perf_mode=DRm,
                            )
                    return [pair[u // 2][:, u % 2, :] for u in range(SC // 128)]

                def pv_mm_u(cps, ets, i, hh, u):
                    h = 2 * i + hh
                    for sk in range(ST):
                        skp, half = sk // 2, sk % 2
                        nc.tensor.matmul(
                            cps[u][:, hh * 65:(hh + 1) * 65],
                            lhsT=ets[skp][:, half * SC + u * 128: half * SC + (u + 1) * 128],
                            rhs=v[:, sk * HL * 65 + h * 65: sk * HL * 65 + (h + 1) * 65],
                            start=(sk == 0), stop=(sk == ST - 1),
                        )

                def make_tail_parts(i, sc, cps, last):
                    """Staggered per-u closures: partN(u) = DVE normalize only;
                    partT(u) = PE transpose (+ drain at odd u), emitted one
                    slot later so the transpose never queues on PE before its
                    normalize has finished on DVE. PV ran in the b-stretch."""
                    state = {"tp": None}

                    def partN(u):
                        def f():
                            t = sc * (SC // 128) + u
                            rec2 = RC.tile([128, 2], fp32, tag="rec",
                                           name=f"rec{i}_{t}")
                            cpv = cps[u].rearrange("p (h c) -> p h c", c=65)
                            nc.vector.reciprocal(rec2[:], cpv[:, :, 64])
                            with nc.allow_low_precision("softmax-normalized bf16 ctx"):
                                nc.vector.tensor_tensor(
                                    out=ctx[:, t * DL + i * 128: t * DL + (i + 1) * 128]
                                        .rearrange("p (h c) -> p h c", c=HD),
                                    in0=cpv[:, :, 0:HD],
                                    in1=rec2[:].unsqueeze(2).broadcast_to([128, 2, HD]),
                                    op=ALU.mult,
                                )
                        return f

                    def partT(u):
                        def f():
                            if u % 2 == 0:
                                # lives in the cps pool's idle window during the
                                # a-stretch, keeping all 3 PM slots for scores
                                state["tp"] = PCS.tile([128, 256], bf16, tag="cp",
                                                       name=f"tp{i}_{sc}_{u // 2}")
                            tp = state["tp"]
                            t = sc * (SC // 128) + u
                            nc.tensor.transpose(
                                tp[:, (u % 2) * 128:(u % 2 + 1) * 128],
                                ctx[:, t * DL + i * 128: t * DL + (i + 1) * 128],
                                ident[:],
                            )
                            if last:
                                # per-u drain so the final out-projs pipeline
                                nc.vector.tensor_scalar(
                                    out=ctxT[:, i * S + t * 128: i * S + (t + 1) * 128],
                                    in0=tp[:, (u % 2) * 128:(u % 2 + 1) * 128],
                                    scalar1=gcol[:, i: i + 1],
                                    scalar2=None,
                                    op0=ALU.mult,
                                )
                                for _, unit in outproj_units(t):
                                    unit()
                            elif u % 2 == 1:
                                nc.vector.tensor_scalar(
                                    out=ctxT[:, i * S + (t - 1) * 128: i * S + (t + 1) * 128],
                                    in0=tp[:],
                                    scalar1=gcol[:, i: i + 1],
                                    scalar2=None,
                                    op0=ALU.mult,
                                )
                                if u == SC // 128 - 1 and i == 1:
                                    for t2 in range(sc * 4, sc * 4 + 4):
                                        fillers.extend(outproj_units(t2))
                        return f

                    parts = []
                    nn = [partN(u) for u in range(SC // 128)]
                    tt = [partT(u) for u in range(SC // 128)]
                    parts.append(nn[0])
                    for u in range(1, SC // 128):
                        parts.append(lambda a=nn[u], b=tt[u - 1]: (a(), b()))
                    parts.append(tt[SC // 128 - 1])
                    return parts

                nchunks = [(i, sc) for i in range(2) for sc in range(NSC)][:KB_CHUNKS]
                pending_parts = []
                for n, (i, sc) in enumerate(nchunks):
                    first = (n == 0)
                    if first:
                        # mean-pool runs here: PE is otherwise idle during the
                        # first a-stretch and the PC psum pool is free.
                        xmp[0] = PCS.tile([64, 512], fp32, tag="cp", name="xmp0")
                        xmp[1] = PCS.tile([64, 512], fp32, tag="cp", name="xmp1")
                    ets_a = []
                    for skp in range(ST // 2):
                        if first and skp in (2, 4, 6):
                            # k i=0 chunk j just ahead of the scores needing it
                            qk_unit(wk8v, kT8, bqk[:, 2:3], 0, skp // 2,
                                    QK_DRAIN[skp // 2])
                        ets_a.append(score_exp(i, sc, 0, skp,
                                               EXP_PAT[n % len(EXP_PAT)][skp]))
                        if skp >= 1 and pending_parts:
                            pending_parts.pop(0)()
                            if skp >= 3 and not first:
                                pop_fillers(1000)
                        elif skp >= 2 and not first:
                            pop_fillers(FILLER_NS)
                    while pending_parts:
                        pending_parts.pop(0)()
                    if first:
                        # keep these off the critical lead-in: the scheduler
                        # would otherwise hoist them ahead of the k/q units
                        # and head-block PE on the slow x8-pair DMAs.
                        with tc.tile_wait_until(0.012):
                            for pr in range(ST // 2):
                                meanpool_mm(pr, xmp)
                        with tc.tile_wait_until(0.014):
                            gates_block()
                    cps = alloc_cps(i, sc)
                    ets_b = []
                    for skp in range(ST // 2):
                        ets_b.append(score_exp(i, sc, 1, skp,
                                               EXP_PAT[n % len(EXP_PAT)][8 + skp]))
                        if first:
                            v_unit(skp, V_DRAIN[skp % 8])
                        # PV for both heads streams through the b-stretch.
                        # Head b lags one skp so PE never queues behind the
                        # exp that was just issued for this skp.
                        pv_mm(cps, ets_a, i, 0, 2 * skp)
                        pv_mm(cps, ets_a, i, 0, 2 * skp + 1)
                        if skp >= 1:
                            pv_mm(cps, ets_b, i, 1, 2 * (skp - 1))
                            pv_mm(cps, ets_b, i, 1, 2 * (skp - 1) + 1)
                        if skp == 5 and i == 0 and sc < NSC - 1:
                            # q i=0 chunk sc+1 mid-b-stretch, off the boundary
                            qk_unit(wq8v, qT8, bqk[:, 0:1], 0, sc + 1,
                                    QK_DRAIN[4 + sc + 1])
                        if not first:
                            pop_fillers(FILLER_NS)
                    pending_parts = make_tail_parts(
                        i, sc, cps, last=(n == len(nchunks) - 1))
                    # the last head-b PV pair rides into the next a-stretch so
                    # the chunk boundary never waits on the final exp
                    def last_pv(cps=cps, ets_b=ets_b, i=i):
                        pv_mm(cps, ets_b, i, 1, ST - 2)
                        pv_mm(cps, ets_b, i, 1, ST - 1)
                    pending_parts.insert(0, last_pv)
                if KB_CHUNKS == 8:
                    for p in pending_parts:
                        p()
                    pop_fillers(10**9)
                else:
                    pending_parts.clear()
                    fillers.clear()
                    # touch out so the output DMA graph exists
                    ot = OB.tile([128, D], bf16, tag="ot", name="ot_stub")
                    nc.vector.memset(ot[:], 0.0)
                    nc.sync.dma_start(out=out_d[0:128, :], in_=ot[:])

    _split_multi_waits(nc)
    return nc


def _split_multi_waits(nc, skip=("InstEventSemaphore",)):
    """Hoist extra sync waits onto preceding same-engine NoOps.

    Walrus codegen can attach only one sync wait to some instruction
    encodings, so any instruction carrying N>1 waits is rewritten as N-1
    single-wait NoOps followed by the instruction with the last wait.
    """
    import concourse.mybir as mybir

    eng = {
        "EngineType.PE": nc.tensor,
        "EngineType.DVE": nc.vector,
        "EngineType.Activation": nc.scalar,
        "EngineType.Pool": nc.gpsimd,
        "EngineType.SP": nc.sync,
    }

    def fresh_nop(engine_key):
        nop = eng[engine_key].nop(hint="wsplit").ins
        for fn in nc.m.functions:
            for bb in fn.blocks:
                for i, ins in enumerate(bb.instructions):
                    if ins.name == nop.name:
                        del bb.instructions[i]
                        return nop
        raise RuntimeError("fresh nop not found")

    for fn in nc.m.functions:
        for bb in fn.blocks:
            insertions = []
            for idx, ins in enumerate(bb.instructions):
                if type(ins).__name__ in skip:
                    continue
                si = ins.sync_info
                if si is None or len(si.on_wait) <= 1:
                    continue
                waits = list(si.on_wait)
                nops = []
                for w in waits[:-1]:
                    nop = fresh_nop(str(ins.engine))
                    nop.sync_info = mybir.SyncInfo(on_wait=[w], on_update=[])
                    nops.append(nop)
                ins.sync_info = mybir.SyncInfo(
                    on_wait=[waits[-1]], on_update=list(si.on_update)
                )
                insertions.append((idx, nops))
            for idx, nops in reversed(insertions):
                bb.instructions[idx:idx] = nops


def _in_maps(inputs):
    x = np.ascontiguousarray(inputs["x"], np.float32)
    maps = []
    ident = np.eye(128, dtype=np.float32)
    gexp = np.zeros((HL, DL), np.float32)
    for h in range(HL):
        gexp[h, h * HD:(h + 1) * HD] = 1.0
    x8 = x.astype(F8)          # [B, S, D]
    for c in range(NCORES):
        b, g = c // HG, c % HG
        sl = slice(g * DL, (g + 1) * DL)
        hsl = slice(g * HL, (g + 1) * HL)
        # xT fp8 on 64 partitions: [64, jj, s] = x[b][s, jj*64+p]
        xt8 = np.ascontiguousarray(
            x8[b].T.reshape(2 * DT, 64, S).transpose(1, 0, 2).reshape(64, 2 * DT * S))
        # natural-x pairs: [128, pr, c2, col] = x[b][pr*256 + c2*128 + p, col]
        x8n = np.ascontiguousarray(
            x8[b].reshape(ST // 2, 2, 128, D).transpose(2, 0, 1, 3).reshape(128, ST * D))
        def wtile(w):
            return np.ascontiguousarray(
                w[:, sl].reshape(2 * DT, 64, DL).transpose(1, 0, 2).reshape(64, 2 * DT * DL).astype(F8))
        wo = inputs["Wo"][sl, :].reshape(2, 128, D).transpose(1, 0, 2).reshape(128, 2 * D)
        bqk = np.stack([
            inputs["bq"][sl][0:128], inputs["bq"][sl][128:256],
            inputs["bk"][sl][0:128], inputs["bk"][sl][128:256],
        ], axis=1).astype(np.float32)
        bv8p = np.zeros((1, 2 * DL), F8)
        bv8p[0, 0:DL] = inputs["bv"][sl].astype(F8)
        wes = np.concatenate([inputs["We"][:, hsl], inputs["Ws"][:, hsl]], axis=1)
        wes = wes.reshape(DT, 128, 2 * HL).transpose(1, 0, 2).reshape(128, DT * 2 * HL)
        bes = np.stack([inputs["be"][hsl], inputs["bs"][hsl]], axis=1)
        maps.append({
            "xt8": xt8,
            "x8": x8n,
            "wq8": wtile(inputs["Wq"]),
            "wk8": wtile(inputs["Wk"]),
            "wv8": wtile(inputs["Wv"]),
            "wo": np.ascontiguousarray(wo.astype(BF16)),
            "bqk": np.ascontiguousarray(bqk),
            "bv8p": bv8p,
            "wes": np.ascontiguousarray(wes.astype(np.float32)),
            "bes": np.ascontiguousarray(bes.astype(np.float32)),
            "gexp": gexp,
            "ident": ident.astype(BF16),
        })
    return maps


def kernel(**inputs):
    if "nc" not in _CACHE:
        _CACHE["nc"] = _build_nc()
    nc = _CACHE["nc"]
    maps = _in_maps({k: np.asarray(v) for k, v in inputs.items()})

    from concourse.bass_utils import run_bass_kernel_spmd

    trace = bool(int(os.environ.get("KERNEL_TRACE", "0")))
    res = run_bass_kernel_spmd(
        nc, maps, list(range(NCORES)), trace=trace,
        tmpdir=os.environ.get("KERNEL_TRACE_DIR") if trace else None,
    )
    _CACHE["last_result"] = res
    bo = np.asarray(inputs["bo"], np.float32)
    out = np.zeros((B, S, D), np.float32)
    for b in range(B):
        acc = np.zeros((S, D), np.float32)
        for g in range(HG):
            acc += res.results[b * HG + g]["out"].astype(np.float32)
        out[b] = acc + bo
    return out


# revision 52
# speedup vs baseline: 1.0823x; 1.0176x over previous
"""Trainium2 Bass kernel for nn_ASISNativeAttention (B=2,S=2048,D=1024,H=16).

Sharding: 8 cores = 2 batches x 4 head-groups (4 heads each); host splits
inputs per core and sums the 4 partial output projections per batch (+bo).

v2 design (vs the all-bf16 v1 at ~208us modeled): the two elementwise
engines (ACT, DVE) are the roofline -- 131K partition-lines of exp per core
-- so the exp work is split across BOTH of them, and the big matmuls move
to fp8 DoubleRow mode (2 k-tiles per instruction at half the per-row cost)
to keep PE under that roof:

  PE    q/k/v projections in fp8 DoubleRow from a host-transposed fp8 xT
        laid out on 64 partitions x 16 d-tiles (the PE hangs if a dual-fp8
        ldweights base partition changes inside one accumulation chain, and
        2*K*M may not exceed the 128x128 array, so chains stay K=64);
        scores per head via DoubleRow with a stride-0 replicated k-tile
        pair (computes 2*q.k, folded into the exp scale 1/16); PV and
        out-proj in bf16; mean-pool via fp8 DoubleRow ones-matmuls.
  ACT   exact exp (scale=1/16, psum->bf16) for ~60% of score tiles; its
        share of the qk/v/out-proj drains; the two gate sigmoids.
  DVE   Schraudolph exp for the rest: one tensor_scalar
        (psum*A16+B16 -> int16, round-to-nearest) whose int16 bits ARE the
        bf16 probs (max rel err ~3.5%, invisible after softmax: the PV
        ones-column sums the same stored values for the denominator);
        ctx normalize (strided reciprocal + stride-0-broadcast
        tensor_tensor); gated ctxT drain in 2x mode; remaining drains.
  Pool  x8-pair and output DMA via SWDGE, keeping HWDGE/SP free.

PSUM (8 banks): scores double-buffer 2x2 banks in a 3-slot pool shared
with transient out-proj/transpose/projection tiles; PV accumulators 2
banks (zero-opened by a rank-1 matmul so the interleaved per-head
accumulations need no group bookkeeping); per-2KB-bank accumulation
groups follow the zero-region rules (one pending group per region).

Measured (test.py): modeled exec 159.6us (v1 baseline: 208.0us); hardware
rel err vs reference 1.33e-2 (gate 2e-2; fp8 exposure is limited to
x / Wq,Wk,Wv / stored qT,kT -- probs, v, ctx, Wo stay bf16).

kernel.py is self-contained: numpy/ml_dtypes + the concourse tree at
/opt/trn_rl_repo.
"""

import os
import sys
import numpy as np
import ml_dtypes

BF16 = ml_dtypes.bfloat16
F8 = ml_dtypes.float8_e4m3

sys.path.insert(0, "/opt/trn_rl_repo")

B, S, D, H = 2, 2048, 1024, 16
HD = 64          # head dim
NCORES = 8
HG = 4           # head groups = cores per batch
HL = H // HG     # heads per core (4)
DL = D // HG     # local width (256)
ST = S // 128    # 16 s-tiles
DT = D // 128    # 8 d-tiles
SC = 512         # sq chunk width for scores
NSC = S // SC    # 4 chunks

# Schraudolph exp constants: int16 = rint(psum * A16 + B16); psum holds
# 2*q.k so the effective exp argument is psum/16 = q.k/8.
_C_SCH = 360000.0
A16 = float(2.0**7 / np.log(2.0) / 16.0)
B16 = float(127 * 2**7 - _C_SCH / 65536.0)

# --- engine-assignment knobs (tuned against TimelineSim) ---
# exp engine per chunk (16 chars: a-stretch skp 0-7 then b-stretch skp 0-7):
# 'A' = ACT exact exp, 'D' = DVE Schraudolph
EXP_PAT = [
    "ADAADAAD" "AADAADAD",   # 10A chunks (early D: DVE lane starts sooner)
    "AADAADAD" "ADAADADD",   # 9A chunks
] * 3 + ["AADAADAD" "AADAADAD", "AADAADAD" "ADAADDDA"]
if os.environ.get("KB_ALLACT"):
    EXP_PAT = ["A" * 16] * 8
KB_NOSWDGE = bool(os.environ.get("KB_NOSWDGE"))
KB_CHUNKS = int(os.environ.get("KB_CHUNKS", "8"))
KB_LEADIN = int(os.environ.get("KB_LEADIN", "99"))
# qk projection drain engines, one char per unit (k-i0 x4, q-i0 x4, then 8 i1)
QK_DRAIN = "ADADADADADADADAD"
# v drain engines, one per psum group (8)
V_DRAIN = "ADADADAD"
# out-proj drain engines, one per po half (32)
OUT_DRAIN = "AD" * 16
FILLER_NS = 1500   # filler budget popped per b-stretch skp

_CACHE = {}


def _build_nc():
    import concourse.bass as bass
    import concourse.mybir as mybir
    from concourse.tile import TileContext

    fp32 = mybir.dt.float32
    bf16 = mybir.dt.bfloat16
    f8 = mybir.dt.float8e4
    i16 = mybir.dt.int16
    AF = mybir.ActivationFunctionType
    ALU = mybir.AluOpType
    DRm = mybir.MatmulPerfMode.DoubleRow

    nc = bass.Bass()

    xt8_d = nc.declare_dram_parameter("xt8", [64, 2 * DT * S], f8, isOutput=False)
    x8_d = nc.declare_dram_parameter("x8", [128, ST * D], f8, isOutput=False)
    wq_d = nc.declare_dram_parameter("wq8", [64, 2 * DT * DL], f8, isOutput=False)
    wk_d = nc.declare_dram_parameter("wk8", [64, 2 * DT * DL], f8, isOutput=False)
    wv_d = nc.declare_dram_parameter("wv8", [64, 2 * DT * DL], f8, isOutput=False)
    wo_d = nc.declare_dram_parameter("wo", [128, 2 * D], bf16, isOutput=False)
    bqk_d = nc.declare_dram_parameter("bqk", [128, 4], fp32, isOutput=False)
    bv8_d = nc.declare_dram_parameter("bv8p", [1, 2 * DL], f8, isOutput=False)
    wes_d = nc.declare_dram_parameter("wes", [128, DT * 2 * HL], fp32, isOutput=False)
    bes_d = nc.declare_dram_parameter("bes", [HL, 2], fp32, isOutput=False)
    gexp_d = nc.declare_dram_parameter("gexp", [HL, DL], fp32, isOutput=False)
    id_d = nc.declare_dram_parameter("ident", [128, 128], bf16, isOutput=False)
    out_d = nc.declare_dram_parameter("out", [S, D], bf16, isOutput=True)

    with TileContext(nc) as tc:
        with tc.tile_pool(name="persist", bufs=1) as P:
            xt8 = P.tile([64, 2 * DT * S], f8, tag="xt8")
            wq8 = P.tile([64, 2 * DT * DL], f8, tag="wq8")
            wk8 = P.tile([64, 2 * DT * DL], f8, tag="wk8")
            wv8 = P.tile([64, 2 * DT * DL], f8, tag="wv8")
            wo = P.tile([128, 2 * D], bf16, tag="wo")
            qT8 = P.tile([128, 2 * S], f8, tag="qT8")
            kT8 = P.tile([128, 2 * S], f8, tag="kT8")
            v = P.tile([128, ST * HL * 65], bf16, tag="v")
            ctx = P.tile([128, ST * DL], bf16, tag="ctx")
            ctxT = P.tile([128, 2 * S], bf16, tag="ctxT")
            bqk = P.tile([128, 4], fp32, tag="bqk")
            bv8p = P.tile([1, 2 * DL], f8, tag="bv8p")
            ones82 = P.tile([128, 2, 64], f8, tag="ones82")
            ones8r = P.tile([1, 2 * 128], f8, tag="ones8r")
            z8row = P.tile([1, 2 * 260], f8, tag="z8row")
            wes = P.tile([128, DT * 2 * HL], fp32, tag="wes")
            bes = P.tile([HL, 2], fp32, tag="bes")
            gexp = P.tile([HL, DL], fp32, tag="gexp")
            ident = P.tile([128, 128], bf16, tag="ident")
            ones1 = P.tile([1, 1], fp32, tag="ones1")
            xmrow = P.tile([1, D], fp32, tag="xmrow")
            xm_col = P.tile([128, DT], fp32, tag="xm_col")
            gcol = P.tile([128, 2], fp32, tag="gcol")

            dma = nc.sync.dma_start

            def vview(t):
                return v[:].rearrange("p (t h c) -> p t h c", h=HL, c=65)[:, t]

            nc.vector.memset(ones82[:], 1.0)
            nc.vector.memset(ones8r[:], 1.0)
            nc.vector.memset(z8row[:], 0.0)
            nc.vector.memset(ones1[:], 1.0)
            # constant softmax-denominator columns of v
            nc.vector.memset(
                v[:].rearrange("p (t h c) -> p t h c", h=HL, c=65)[:, :, :, 64:65], 1.0
            )

            xt8v = xt8[:].rearrange("p (j s) -> p j s", s=S)      # [64, 16, S]
            wq8v = wq8[:].rearrange("p (j m) -> p j m", m=DL)     # [64, 16, DL]
            wk8v = wk8[:].rearrange("p (j m) -> p j m", m=DL)
            wv8v = wv8[:].rearrange("p (j m) -> p j m", m=DL)

            with (
                tc.tile_pool(name="x8l", bufs=8) as XL,
                tc.tile_pool(name="pm", bufs=3, space="PSUM") as PM,
                tc.tile_pool(name="pcps", bufs=2, space="PSUM") as PCS,
                tc.tile_pool(name="ets", bufs=26) as EX,
                tc.tile_pool(name="rc", bufs=8) as RC,
                tc.tile_pool(name="ob", bufs=2) as OB,
                tc.tile_pool(name="gs", bufs=1) as GS,
            ):
                xmp = [None, None]

                xbs = []

                def load_x_pair(pr):
                    """DMA natural-x pair pr via SWDGE (Pool) off the HWDGE path."""
                    xb = XL.tile([128, 2, D], f8, tag="xb", name=f"xb{pr}")
                    # SP HWDGE: queues naturally behind the critical xt8/w
                    # loads instead of jumping ahead from the idle Pool queue
                    nc.sync.dma_start(
                        out=xb[:],
                        in_=x8_d[:].rearrange("p (r c d) -> p r c d", c=2, d=D)[:, pr])
                    xbs.append(xb)

                def meanpool_mm(pr, xmps):
                    lhs = ones82[:]  # [128, 2, 64]: dual-fp8 ldweights needs wide M
                    for qh in range(4):
                        half, qq = qh // 2, qh % 2
                        first = (pr == 0 and qq == 0)
                        nc.tensor.matmul(
                            xmps[half][:, qq * 256:(qq + 1) * 256],
                            lhsT=lhs,
                            rhs=xbs[pr][:, :, qh * 256:(qh + 1) * 256],
                            start=first, stop=first,
                            skip_group_check=not first,
                            perf_mode=DRm,
                        )

                def qk_unit(w8v, dst8, bcol, i, sc, eng):
                    """Project one [128, SC] chunk of qT or kT (fp8 out)."""
                    pp = PM.tile([128, SC], fp32, tag="pm",
                                 name=f"pp{dst8.tensor.name}_{i}_{sc}")
                    for qq in range(2):
                        # x / weights live on 64 partitions x 16 d-tiles:
                        # dual-fp8 DR caps 2*K*M at the PE array size and the
                        # PE hangs if ldweights base-partition changes inside
                        # an accumulation chain, so every chain stays K=64.
                        for dp in range(DT):
                            first = (qq == 0 and dp == 0)
                            nc.tensor.matmul(
                                pp[:, qq * 256:(qq + 1) * 256],
                                lhsT=w8v[:, 2 * dp:2 * dp + 2, i * 128:(i + 1) * 128],
                                rhs=xt8v[:, 2 * dp:2 * dp + 2,
                                         sc * SC + qq * 256: sc * SC + (qq + 1) * 256],
                                start=first, stop=first,
                                skip_group_check=not first,
                                perf_mode=DRm,
                            )
                    dst = dst8[:, i * S + sc * SC: i * S + (sc + 1) * SC]
                    if eng == "A":
                        nc.scalar.activation(dst, pp[:], AF.Identity, bias=bcol)
                    else:
                        nc.vector.tensor_scalar(
                            out=dst, in0=pp[:], scalar1=bcol, scalar2=None,
                            op0=ALU.add,
                        )

                def v_unit(g, eng):
                    """Project v for s-tiles 2g, 2g+1 (one psum bank)."""
                    pv = PM.tile([128, 2 * DL], fp32, tag="pm", name=f"pv{g}")
                    for t2 in range(2):
                        t = 2 * g + t2
                        sl = pv[:, t2 * DL:(t2 + 1) * DL]
                        nc.tensor.matmul(
                            sl, lhsT=ones8r[:].rearrange("o (c m) -> o c m", c=2),
                            rhs=bv8p[:].rearrange("o (c m) -> o c m", c=2),
                            start=(t2 == 0), stop=(t2 == 0),
                            skip_group_check=(t2 == 1), perf_mode=DRm,
                        )
                        for dp in range(DT):
                            nc.tensor.matmul(
                                sl,
                                lhsT=xt8v[:, 2 * dp:2 * dp + 2, t * 128:(t + 1) * 128],
                                rhs=wv8v[:, 2 * dp:2 * dp + 2, :],
                                start=False, stop=False,
                                skip_group_check=True,
                                perf_mode=DRm,
                            )
                    for t2 in range(2):
                        t = 2 * g + t2
                        src = pv[:, t2 * DL:(t2 + 1) * DL].rearrange(
                            "p (h c) -> p h c", c=HD)
                        dst = vview(t)[:, :, 0:HD]
                        if eng == "A":
                            nc.scalar.copy(dst, src)
                        else:
                            nc.vector.tensor_copy(dst, src)

                def score_exp(i, sc, hh, skp, eng):
                    """Scores for sk-tiles (2skp, 2skp+1) x [sc*SC, (sc+1)*SC) of
                    head 2i+hh; one wide exp. psum holds 2*q.k (stride-0 DR)."""
                    r = hh * 64
                    sp = PM.tile([128, 2 * SC], fp32, tag="pm",
                                 name=f"sp{i}_{sc}_{hh}_{skp}")
                    for half in range(2):
                        sk = 2 * skp + half
                        lhsT = kT8[r:r + 64, i * S + sk * 128: i * S + (sk + 1) * 128] \
                            .unsqueeze(1).broadcast_to([64, 2, 128])
                        for qq in range(2):
                            rhs = qT8[r:r + 64,
                                      i * S + sc * SC + qq * 256: i * S + sc * SC + (qq + 1) * 256] \
                                .unsqueeze(1).broadcast_to([64, 2, 256])
                            # qq0 opens the bank's zero region; qq1 assigns
                            # into still-pending bytes (no second group)
                            nc.tensor.matmul(
                                sp[:, half * SC + qq * 256: half * SC + (qq + 1) * 256],
                                lhsT=lhsT, rhs=rhs, start=(qq == 0), stop=(qq == 0),
                                skip_group_check=(qq == 1),
                                perf_mode=DRm,
                            )
                    if eng == "A":
                        et = EX.tile([128, 2 * SC], bf16, tag="et",
                                     name=f"et{i}_{sc}_{hh}_{skp}")
                        nc.scalar.activation(et[:], sp[:], AF.Exp, scale=1.0 / 16.0)
                        return et[:]
                    et = EX.tile([128, 2 * SC], i16, tag="et",
                                 name=f"et{i}_{sc}_{hh}_{skp}")
                    nc.vector.tensor_scalar(
                        out=et[:], in0=sp[:], scalar1=A16, scalar2=B16,
                        op0=ALU.mult, op1=ALU.add,
                    )
                    return et[:].bitcast(bf16)

                def pv_mm(cps, ets, i, hh, sk):
                    h = 2 * i + hh
                    skp, half = sk // 2, sk % 2
                    for u in range(SC // 128):
                        nc.tensor.matmul(
                            cps[u][:, hh * 65:(hh + 1) * 65],
                            lhsT=ets[skp][:, half * SC + u * 128: half * SC + (u + 1) * 128],
                            rhs=v[:, sk * HL * 65 + h * 65: sk * HL * 65 + (h + 1) * 65],
                            start=False, stop=False, skip_group_check=True,
                        )

                def outproj_units(t):
                    def unit():
                        ot = OB.tile([128, D], bf16, tag="ot", name=f"ot{t}")
                        po = PM.tile([128, D], fp32, tag="pm", name=f"po{t}")
                        for n2 in range(2):
                            for i2 in range(2):
                                nc.tensor.matmul(
                                    po[:, n2 * 512:(n2 + 1) * 512],
                                    lhsT=ctxT[:, i2 * S + t * 128: i2 * S + (t + 1) * 128],
                                    rhs=wo[:, i2 * D + n2 * 512: i2 * D + (n2 + 1) * 512],
                                    start=(i2 == 0), stop=(i2 == 1),
                                )
                        if OUT_DRAIN[t % len(OUT_DRAIN)] == "A":
                            nc.scalar.copy(ot[:], po[:])
                        else:
                            nc.vector.tensor_copy(ot[:], po[:])
                        (nc.scalar if KB_NOSWDGE else nc.gpsimd).dma_start(
                            out=out_d[t * 128:(t + 1) * 128, :], in_=ot[:])
                    return [(1000, unit)]

                fillers = []

                def pop_fillers(budget_ns):
                    spent = 0
                    while fillers and spent < budget_ns:
                        ns, unit = fillers.pop(0)
                        unit()
                        spent += ns

                # ---------------- lead-in ----------------
                # warm the PE p-state while DMAs are in flight: dummy fp8
                # matmuls on memset data keep the array busy from ~0.6us so
                # the first real projections run at full clock, not the
                # 3x-slower cold state.
                wu = PM.tile([128, 128], fp32, tag="pm", name="warmup")
                o82f = ones82[:].rearrange("p c m -> p (c m)")
                for wi in range(40):
                    nc.tensor.matmul(
                        wu[:], lhsT=o82f, rhs=o82f,
                        start=True, stop=True, skip_group_check=True,
                    )
                # xt8 chunk 0 + wk8 first: they gate the whole pipeline
                def dma_xt(sc):
                    dma(xt8v[:, :, sc * SC:(sc + 1) * SC],
                        xt8_d[:].rearrange("p (j s) -> p j s", s=S)[:, :, sc * SC:(sc + 1) * SC])
                dma_xt(0)
                dma(wk8[:], wk_d[:])
                dma(bqk[:], bqk_d[:])
                dma(wq8[:], wq_d[:])
                dma_xt(1)
                dma(ident[:], id_d[:])
                dma_xt(2)
                dma_xt(3)
                dma(wv8[:], wv_d[:])
                dma(bv8p[:], bv8_d[:])
                dma(wes[:], wes_d[:])
                dma(bes[:], bes_d[:])
                dma(gexp[:], gexp_d[:])
                dma(wo[:], wo_d[:])
                # keep the pair transfers behind xt8-sc0/wk8/wq8 in the
                # shared DMA queue: they are not needed until the mean-pool
                with tc.tile_wait_until(0.0035):
                    for pr in range(ST // 2):
                        load_x_pair(pr)

                # minimal critical path: k i=0 sc=0 + q i=0 sc=0 lets chunk-0
                # scores start; the other k i=0 chunks interleave with the
                # first a-stretch (k-sc j emitted just before skp 2j).
                if KB_LEADIN >= 1:
                    qk_unit(wk8v, kT8, bqk[:, 2:3], 0, 0, QK_DRAIN[0])
                if KB_LEADIN >= 2:
                    qk_unit(wq8v, qT8, bqk[:, 0:1], 0, 0, QK_DRAIN[4])

                # deferred to fillers: all of i=1 (needed from chunk 4)
                for sc in range(NSC):
                    fillers.append((1100, (lambda s: lambda: qk_unit(
                        wk8v, kT8, bqk[:, 3:4], 1, s, QK_DRAIN[(8 + s) % 16]))(sc)))
                for sc in range(NSC):
                    fillers.append((1100, (lambda s: lambda: qk_unit(
                        wq8v, qT8, bqk[:, 1:2], 1, s, QK_DRAIN[(12 + s) % 16]))(sc)))

                def gates_block():
                    xcp = PCS.tile([128, DT], fp32, tag="cp", name="xcp")
                    for hf in range(2):
                        nc.vector.tensor_copy(
                            xmrow[:, hf * 512:(hf + 1) * 512], xmp[hf][0:1, :])
                    for j in range(DT):
                        nc.tensor.matmul(
                            xcp[:, j: j + 1],
                            lhsT=xmrow[:, j * 128:(j + 1) * 128],
                            rhs=ones1[:],
                            start=True, stop=True,
                        )
                    nc.vector.tensor_copy(xm_col[:], xcp[:])
                    gpe = PCS.tile([HL, 1], fp32, tag="cp", name="gpe")
                    gps = PCS.tile([HL, 1], fp32, tag="cp", name="gps")
                    for j in range(DT):
                        nc.tensor.matmul(
                            gpe[:], lhsT=wes[:, j * 8: j * 8 + 4],
                            rhs=xm_col[:, j: j + 1],
                            start=(j == 0), stop=(j == DT - 1),
                        )
                    for j in range(DT):
                        nc.tensor.matmul(
                            gps[:], lhsT=wes[:, j * 8 + 4: j * 8 + 8],
                            rhs=xm_col[:, j: j + 1],
                            start=(j == 0), stop=(j == DT - 1),
                        )
                    eth = GS.tile([HL, 1], fp32, tag="eth")
                    saf = GS.tile([HL, 1], fp32, tag="saf")
                    gate = GS.tile([HL, 1], fp32, tag="gate")
                    nc.scalar.activation(eth[:], gpe[:], AF.Sigmoid,
                                         bias=bes[:, 0:1], scale=1.0 / S)
                    nc.scalar.activation(saf[:], gps[:], AF.Sigmoid,
                                         bias=bes[:, 1:2], scale=1.0 / S)
                    nc.vector.tensor_mul(gate[:], eth[:], saf[:])
                    for i in range(2):
                        pgc = PCS.tile([128, 1], fp32, tag="cp", name=f"pgc{i}")
                        nc.tensor.matmul(
                            pgc[:], lhsT=gexp[:, i * 128:(i + 1) * 128], rhs=gate[:],
                            start=True, stop=True,
                        )
                        nc.vector.tensor_copy(gcol[:, i: i + 1], pgc[:])

                # ---------------- chunk loop ----------------
                def alloc_cps(i, sc):
                    # two 1-bank tiles, each holding two u-slots of [128, 130];
                    # a zero rank-1 matmul opens each bank's zero region so the
                    # interleaved PV accumulations need no group bookkeeping
                    pair = [PCS.tile([128, 2, 130], fp32, tag="cp",
                                     name=f"cp{i}_{sc}_{w}") for w in range(2)]
                    alloc_cps.pairs = pair
                    for w in range(2):
                        for a2 in range(2):
                            nc.tensor.matmul(
                                pair[w][:, a2, :],
                                lhsT=ones8r[:].rearrange("o (c m) -> o c m", c=2),
                                rhs=z8row[:].rearrange("o (c m) -> o c m", c=2)[:, :, 0:130],
                                start=True, stop=True,
                                skip_group_check=(a2 == 1),
                                perf_mode=DRm,
                            )
                    return [pair[u // 2][:, u % 2, :] for u in range(SC // 128)]

                def pv_mm_u(cps, ets, i, hh, u):
                    h = 2 * i + hh
                    for sk in range(ST):
                        skp, half = sk // 2, sk % 2
                        nc.tensor.matmul(
                            cps[u][:, hh * 65:(hh + 1) * 65],
                            lhsT=ets[skp][:, half * SC + u * 128: half * SC + (u + 1) * 128],
                            rhs=v[:, sk * HL * 65 + h * 65: sk * HL * 65 + (h + 1) * 65],
                            start=(sk == 0), stop=(sk == ST - 1),
                        )

                def make_tail_parts(i, sc, cps, last):
                    """Staggered per-u closures: partN(u) = DVE normalize only;
                    partT(u) = PE transpose (+ drain at odd u), emitted one
                    slot later so the transpose never queues on PE before its
                    normalize has finished on DVE. PV ran in the b-stretch."""
                    state = {"tp": None}

                    pairs = alloc_cps.pairs
                    recs = {}

                    def partN(u):
                        def f():
                            t = sc * (SC // 128) + u
                            if u % 2 == 0:
                                # one reciprocal covers both u-slots of the
                                # bank: all denominators are complete once the
                                # lagged PV pair has run
                                r4 = RC.tile([128, 2, 2], fp32, tag="rec",
                                             name=f"rec{i}_{t}")
                                dens = pairs[u // 2][:].rearrange(
                                    "p a (h c) -> p a h c", c=65)[:, :, :, 64]
                                nc.vector.reciprocal(r4[:], dens)
                                recs[u // 2] = r4
                            rec2 = recs[u // 2][:, u % 2, :]
                            cpv = cps[u].rearrange("p (h c) -> p h c", c=65)
                            with nc.allow_low_precision("softmax-normalized bf16 ctx"):
                                nc.vector.tensor_tensor(
                                    out=ctx[:, t * DL + i * 128: t * DL + (i + 1) * 128]
                                        .rearrange("p (h c) -> p h c", c=HD),
                                    in0=cpv[:, :, 0:HD],
                                    in1=rec2.unsqueeze(2).broadcast_to([128, 2, HD]),
                                    op=ALU.mult,
                                )
                        return f

                    def partT(u):
                        def f():
                            if u % 2 == 0:
                                # lives in the cps pool's idle window during the
                                # a-stretch, keeping all 3 PM slots for scores
                                state["tp"] = PCS.tile([128, 256], bf16, tag="cp",
                                                       name=f"tp{i}_{sc}_{u // 2}")
                            tp = state["tp"]
                            t = sc * (SC // 128) + u
                            nc.tensor.transpose(
                                tp[:, (u % 2) * 128:(u % 2 + 1) * 128],
                                ctx[:, t * DL + i * 128: t * DL + (i + 1) * 128],
                                ident[:],
                            )
                            if last:
                                # per-u drain so the final out-projs pipeline
                                nc.vector.tensor_scalar(
                                    out=ctxT[:, i * S + t * 128: i * S + (t + 1) * 128],
                                    in0=tp[:, (u % 2) * 128:(u % 2 + 1) * 128],
                                    scalar1=gcol[:, i: i + 1],
                                    scalar2=None,
                                    op0=ALU.mult,
                                )
                                for _, unit in outproj_units(t):
                                    unit()
                            elif u % 2 == 1:
                                nc.vector.tensor_scalar(
                                    out=ctxT[:, i * S + (t - 1) * 128: i * S + (t + 1) * 128],
                                    in0=tp[:],
                                    scalar1=gcol[:, i: i + 1],
                                    scalar2=None,
                                    op0=ALU.mult,
                                )
                                if u == SC // 128 - 1 and i == 1:
                                    for t2 in range(sc * 4, sc * 4 + 4):
                                        fillers.extend(outproj_units(t2))
                        return f

                    parts = []
                    nn = [partN(u) for u in range(SC // 128)]
                    tt = [partT(u) for u in range(SC // 128)]
                    parts.append(nn[0])
                    for u in range(1, SC // 128):
                        parts.append(lambda a=nn[u], b=tt[u - 1]: (a(), b()))
                    parts.append(tt[SC // 128 - 1])
                    return parts

                nchunks = [(i, sc) for i in range(2) for sc in range(NSC)][:KB_CHUNKS]
                pending_parts = []
                for n, (i, sc) in enumerate(nchunks):
                    first = (n == 0)
                    if first:
                        # mean-pool runs here: PE is otherwise idle during the
                        # first a-stretch and the PC psum pool is free.
                        xmp[0] = PCS.tile([64, 512], fp32, tag="cp", name="xmp0")
                        xmp[1] = PCS.tile([64, 512], fp32, tag="cp", name="xmp1")
                    ets_a = []
                    for skp in range(ST // 2):
                        if first and skp in (2, 4, 6):
                            # k i=0 chunk j just ahead of the scores needing it
                            qk_unit(wk8v, kT8, bqk[:, 2:3], 0, skp // 2,
                                    QK_DRAIN[skp // 2])
                        ets_a.append(score_exp(i, sc, 0, skp,
                                               EXP_PAT[n % len(EXP_PAT)][skp]))
                        if skp >= 1 and pending_parts:
                            pending_parts.pop(0)()
                            if skp >= 3 and not first:
                                pop_fillers(1000)
                        elif skp >= 2 and not first:
                            pop_fillers(FILLER_NS)
                    while pending_parts:
                        pending_parts.pop(0)()
                    if first:
                        # keep these off the critical lead-in: the scheduler
                        # would otherwise hoist them ahead of the k/q units
                        # and head-block PE on the slow x8-pair DMAs.
                        with tc.tile_wait_until(0.012):
                            for pr in range(ST // 2):
                                meanpool_mm(pr, xmp)
                        with tc.tile_wait_until(0.014):
                            gates_block()
                    cps = alloc_cps(i, sc)
                    ets_b = []
                    for skp in range(ST // 2):
                        ets_b.append(score_exp(i, sc, 1, skp,
                                               EXP_PAT[n % len(EXP_PAT)][8 + skp]))
                        if first:
                            v_unit(skp, V_DRAIN[skp % 8])
                        # PV for both heads streams through the b-stretch.
                        # Head b lags one skp so PE never queues behind the
                        # exp that was just issued for this skp.
                        pv_mm(cps, ets_a, i, 0, 2 * skp)
                        pv_mm(cps, ets_a, i, 0, 2 * skp + 1)
                        if skp >= 1:
                            pv_mm(cps, ets_b, i, 1, 2 * (skp - 1))
                            pv_mm(cps, ets_b, i, 1, 2 * (skp - 1) + 1)
                        if skp == 5 and i == 0 and sc < NSC - 1:
                            # q i=0 chunk sc+1 mid-b-stretch, off the boundary
                            qk_unit(wq8v, qT8, bqk[:, 0:1], 0, sc + 1,
                                    QK_DRAIN[4 + sc + 1])
                        if not first:
                            pop_fillers(FILLER_NS)
                    pending_parts = make_tail_parts(
                        i, sc, cps, last=(n == len(nchunks) - 1))
                    # the last head-b PV pair rides into the next a-stretch so
                    # the chunk boundary never waits on the final exp
                    def last_pv(cps=cps, ets_b=ets_b, i=i):
                        pv_mm(cps, ets_b, i, 1, ST - 2)
                        pv_mm(cps, ets_b, i, 1, ST - 1)
                    pending_parts.insert(0, last_pv)
                if KB_CHUNKS == 8:
                    for p in pending_parts:
                        p()
                    pop_fillers(10**9)
                else:
                    pending_parts.clear()
                    fillers.clear()
                    # touch out so the output DMA graph exists
                    ot = OB.tile([128, D], bf16, tag="ot", name="ot_stub")
                    nc.vector.memset(ot[:], 0.0)
                    nc.sync.dma_start(out=out_d[0:128, :], in_=ot[:])

    _split_multi_waits(nc)
    return nc


def _split_multi_waits(nc, skip=("InstEventSemaphore",)):
    """Hoist extra sync waits onto preceding same-engine NoOps.

    Walrus codegen can attach only one sync wait to some instruction
    encodings, so any instruction carrying N>1 waits is rewritten as N-1
    single-wait NoOps followed by the instruction with the last wait.
    """
    import concourse.mybir as mybir

    eng = {
        "EngineType.PE": nc.tensor,
        "EngineType.DVE": nc.vector,
        "EngineType.Activation": nc.scalar,
        "EngineType.Pool": nc.gpsimd,
        "EngineType.SP": nc.sync,
    }

    def fresh_nop(engine_key):
        nop = eng[engine_key].nop(hint="wsplit").ins
        for fn in nc.m.functions:
            for bb in fn.blocks:
                for i, ins in enumerate(bb.instructions):
                    if ins.name == nop.name:
                        del bb.instructions[i]
                        return nop
        raise RuntimeError("fresh nop not found")

    for fn in nc.m.functions:
        for bb in fn.blocks:
            insertions = []
            for idx, ins in enumerate(bb.instructions):
                if type(ins).__name__ in skip:
                    continue
                si = ins.sync_info
                if si is None or len(si.on_wait) <= 1:
                    continue
                waits = list(si.on_wait)
                nops = []
                for w in waits[:-1]:
                    nop = fresh_nop(str(ins.engine))
                    nop.sync_info = mybir.SyncInfo(on_wait=[w], on_update=[])
                    nops.append(nop)
                ins.sync_info = mybir.SyncInfo(
                    on_wait=[waits[-1]], on_update=list(si.on_update)
                )
                insertions.append((idx, nops))
            for idx, nops in reversed(insertions):
                bb.instructions[idx:idx] = nops


def _in_maps(inputs):
    x = np.ascontiguousarray(inputs["x"], np.float32)
    maps = []
    ident = np.eye(128, dtype=np.float32)
    gexp = np.zeros((HL, DL), np.float32)
    for h in range(HL):
        gexp[h, h * HD:(h + 1) * HD] = 1.0
    x8 = x.astype(F8)          # [B, S, D]
    for c in range(NCORES):
        b, g = c // HG, c % HG
        sl = slice(g * DL, (g + 1) * DL)
        hsl = slice(g * HL, (g + 1) * HL)
        # xT fp8 on 64 partitions: [64, jj, s] = x[b][s, jj*64+p]
        xt8 = np.ascontiguousarray(
            x8[b].T.reshape(2 * DT, 64, S).transpose(1, 0, 2).reshape(64, 2 * DT * S))
        # natural-x pairs: [128, pr, c2, col] = x[b][pr*256 + c2*128 + p, col]
        x8n = np.ascontiguousarray(
            x8[b].reshape(ST // 2, 2, 128, D).transpose(2, 0, 1, 3).reshape(128, ST * D))
        def wtile(w):
            return np.ascontiguousarray(
                w[:, sl].reshape(2 * DT, 64, DL).transpose(1, 0, 2).reshape(64, 2 * DT * DL).astype(F8))
        wo = inputs["Wo"][sl, :].reshape(2, 128, D).transpose(1, 0, 2).reshape(128, 2 * D)
        bqk = np.stack([
            inputs["bq"][sl][0:128], inputs["bq"][sl][128:256],
            inputs["bk"][sl][0:128], inputs["bk"][sl][128:256],
        ], axis=1).astype(np.float32)
        bv8p = np.zeros((1, 2 * DL), F8)
        bv8p[0, 0:DL] = inputs["bv"][sl].astype(F8)
        wes = np.concatenate([inputs["We"][:, hsl], inputs["Ws"][:, hsl]], axis=1)
        wes = wes.reshape(DT, 128, 2 * HL).transpose(1, 0, 2).reshape(128, DT * 2 * HL)
        bes = np.stack([inputs["be"][hsl], inputs["bs"][hsl]], axis=1)
        maps.append({
            "xt8": xt8,
            "x8": x8n,
            "wq8": wtile(inputs["Wq"]),
            "wk8": wtile(inputs["Wk"]),
            "wv8": wtile(inputs["Wv"]),
            "wo": np.ascontiguousarray(wo.astype(BF16)),
            "bqk": np.ascontiguousarray(bqk),
            "bv8p": bv8p,
            "wes": np.ascontiguousarray(wes.astype(np.float32)),
            "bes": np.ascontiguousarray(bes.astype(np.float32)),
            "gexp": gexp,
            "ident": ident.astype(BF16),
        })
    return maps


def kernel(**inputs):
    if "nc" not in _CACHE:
        _CACHE["nc"] = _build_nc()
    nc = _CACHE["nc"]
    maps = _in_maps({k: np.asarray(v) for k, v in inputs.items()})

    from concourse.bass_utils import run_bass_kernel_spmd

    trace = bool(int(os.environ.get("KERNEL_TRACE", "0")))
    res = run_bass_kernel_spmd(
        nc, maps, list(range(NCORES)), trace=trace,
        tmpdir=os.environ.get("KERNEL_TRACE_DIR") if trace else None,
    )
    _CACHE["last_result"] = res
    bo = np.asarray(inputs["bo"], np.float32)
    out = np.zeros((B, S, D), np.float32)
    for b in range(B):
        acc = np.zeros((S, D), np.float32)
        for g in range(HG):
            acc += res.results[b * HG + g]["out"].astype(np.float32)
        out[b] = acc + bo
    return out
